# revision 1
# baseline (speedup 1.0000x reference)
"""GraphSAGE-max (3 layers + 2 heads) on 8 Trainium2 NeuronCores.

Strategy: data-parallel over dst-node partitions (the "graph partition +
replicated weights" scheme). Nodes are dealt to the 8 cores snake-wise by
in-degree, then re-sorted inside each core by (in-degree, lo-half-degree)
so a dense ELL gather schedule has little padding. Features live in
replicated DRAM tables of bf16 rows; the per-core table block carries its
own -inf pad row so both the lo half (cores 0-3) and the hi half (cores
4-7) of the table are addressable with int16 dma_gather indices.

Each layer, per 128-node tile:
  - dma_gather neighbor rows (two calls: lo table view, hi table view)
    -> [128, K*F] bf16, tree-max down to [128, F], upcast to f32
  - PE-transpose to feature-major, then f32 matmuls:
      yT = relu(Wl.T @ aggT + Wr.T @ hT + b)
  - PE-transpose back to node-major, cast bf16, store to the core's block
  - AllGather blocks across the 8 cores -> next layer's gather table
The two output heads share the third aggregation.
"""

import numpy as np
import ml_dtypes

import concourse.bass as bass
import concourse.bacc as bacc
import concourse.mybir as mybir
import concourse.tile as tile
from concourse.masks import make_identity
from concourse.bass_utils import run_bass_kernel_spmd

N = 50000
E = 800000
F_IN = 128
H = 256
NCOR = 8
NLOC = N // NCOR             # 6250
BLOCK = NLOC + 1             # 6251 rows per core block (last = -inf pad)
HALF = 4 * BLOCK             # 25004 rows per table half
TILES = (NLOC + 127) // 128  # 49
PADN = TILES * 128           # 6272
NEG = float(np.finfo(np.float32).min)
KCAP = 8                     # max gather columns per dma_gather call
CHUNK = 4                    # node tiles per matmul chunk (N free = 512)
PADIDX = NLOC                # pad row local index inside a table half

_LAST = {}                   # stash for the test harness


# ----------------------------------------------------------------------------
# host-side graph preprocessing
# ----------------------------------------------------------------------------

def _wrap_idx(ilist):
    """ilist [NCOR, num] int -> dma_gather wrapped layout [NCOR, 128*ceil(num/16)]
    (16-partition wrap, replicated to 128 partitions)."""
    num = ilist.shape[1]
    cols = (num + 15) // 16
    w = np.zeros((NCOR, 16, cols), np.int16)
    i = np.arange(num)
    w[:, i % 16, i // 16] = ilist
    w = np.tile(w, (1, 8, 1))                 # [NCOR, 128, cols]
    return w.reshape(NCOR, 128 * cols)


def _preprocess(edge_index):
    src = np.asarray(edge_index[0], np.int64)
    dst = np.asarray(edge_index[1], np.int64)
    deg = np.bincount(dst, minlength=N)

    # deal nodes (by degree desc) to cores snake-wise -> owner per old id
    order = np.argsort(-deg, kind="stable")
    ranks = np.arange(N)
    pos = ranks % NCOR
    core_of_rank = np.where((ranks // NCOR) % 2 == 0, pos, NCOR - 1 - pos)
    owner = np.empty(N, np.int64)
    owner[order] = core_of_rank

    # lo half = nodes owned by cores 0-3
    lo_of_old = owner < 4
    deg_lo = np.bincount(dst[lo_of_old[src]], minlength=N)

    # within-core order: (deg desc, deg_lo desc) -> tight two-phase ELL
    old_of_new = np.empty(N, np.int64)
    for m in range(NCOR):
        nodes = np.where(owner == m)[0]
        key = np.lexsort((-deg_lo[nodes], -deg[nodes]))
        old_of_new[m * NLOC:(m + 1) * NLOC] = nodes[key]
    new_of_old = np.empty(N, np.int64)
    new_of_old[old_of_new] = np.arange(N)

    # local index within the table half, per old id
    m_of_old = new_of_old // NLOC
    r_of_old = new_of_old % NLOC
    tloc_of_old = np.where(m_of_old < 4, m_of_old, m_of_old - 4) * BLOCK + r_of_old

    # per-dst phase-split neighbor slots
    nd = new_of_old[dst]
    ph = (~lo_of_old[src]).astype(np.int64)           # 0 = lo, 1 = hi
    stloc = tloc_of_old[src]
    gk = nd * 2 + ph
    eorder = np.argsort(gk, kind="stable")
    gk_s = gk[eorder]
    stloc_s = stloc[eorder]
    starts = np.searchsorted(gk_s, np.arange(2 * N))
    slot = np.arange(E) - starts[gk_s]
    cnt = np.bincount(gk, minlength=2 * N)
    dlo = cnt[0::2]                  # lo-degree, indexed by new id
    dhi = cnt[1::2]

    # shared compile-time K schedule per (tile, phase): max over cores
    def ktile(d):
        ks = np.zeros(TILES, np.int64)
        for m in range(NCOR):
            dm = d[m * NLOC:(m + 1) * NLOC]
            for t in range(TILES):
                blk = dm[t * 128:(t + 1) * 128]
                if blk.size:
                    ks[t] = max(ks[t], int(blk.max()))
        return np.maximum(ks, 1)
    klo = ktile(dlo)
    khi = ktile(dhi)

    # dense ELL per phase [NCOR, PADN, kmax]
    def ell_of(phase, kmax):
        ell = np.full((NCOR, PADN, kmax), PADIDX, np.int16)
        sel = ph[eorder] == phase
        nd_e = nd[eorder][sel]
        ell[nd_e // NLOC, nd_e % NLOC, slot[sel]] = stloc_s[sel].astype(np.int16)
        return ell
    ells = {0: ell_of(0, int(klo.max())), 1: ell_of(1, int(khi.max()))}

    # gather call schedule + wrapped int16 index stream.
    # Calls are grouped per matmul chunk (CHUNK node tiles): one idx DMA
    # loads the whole chunk's wrapped indices; each call slices columns.
    NCH = (TILES + CHUNK - 1) // CHUNK
    sched = []           # (tile, phase, col0, kn, chunk, cidx0_cols)
    chunks = []          # (flat_off, cols) per chunk
    blocks = []
    off = 0
    for c in range(NCH):
        cblocks = []
        ccols = 0
        for t in range(c * CHUNK, min((c + 1) * CHUNK, TILES)):
            col0 = 0
            for phase, ks in ((0, klo), (1, khi)):
                k0 = 0
                while k0 < int(ks[t]):
                    kn = min(KCAP, int(ks[t]) - k0)
                    blk = ells[phase][:, t * 128:(t + 1) * 128, k0:k0 + kn]
                    ilist = blk.transpose(0, 2, 1).reshape(NCOR, kn * 128)
                    w = _wrap_idx(ilist).reshape(NCOR, 128, 8 * kn)
                    cblocks.append(w)
                    sched.append((t, phase, col0, kn, c, ccols))
                    ccols += 8 * kn
                    k0 += kn
                    col0 += kn
        cb = np.concatenate(cblocks, axis=2)      # [NCOR, 128, ccols]
        blocks.append(cb.reshape(NCOR, 128 * ccols))
        chunks.append((off, ccols))
        off += 128 * ccols
    idx_flat = np.concatenate(blocks, axis=1)
    ktot = klo + khi

    return dict(new_of_old=new_of_old, old_of_new=old_of_new,
                sched=sched, chunks=chunks, totslot=off, idx_flat=idx_flat,
                ktot=ktot, isolated=bool((deg == 0).any()))


# ----------------------------------------------------------------------------
# device program
# ----------------------------------------------------------------------------

def _tree_max(nc, g, k, F):
    """In-place max over k column groups of width F; result in g[:, :F]."""
    while k > 1:
        if k % 2 == 1:
            nc.vector.tensor_tensor(out=g[:, 0:F], in0=g[:, 0:F],
                                    in1=g[:, (k - 1) * F:k * F],
                                    op=mybir.AluOpType.max)
            k -= 1
            if k == 1:
                break
        half = k // 2
        nc.vector.tensor_tensor(out=g[:, 0:half * F], in0=g[:, 0:half * F],
                                in1=g[:, half * F:2 * half * F],
                                op=mybir.AluOpType.max)
        k = half


def _build_program(sched, chunks, totslot, isolated, stages=5):
    """stages: 0=L1 gathers only, 1=L1 only, 2=+AG1, 3=+L2, 4=+AG2, 5=full."""
    nc = bacc.Bacc("TRN2", target_bir_lowering=False, debug=False,
                   num_devices=NCOR)
    f32, bf16, i16 = mybir.dt.float32, mybir.dt.bfloat16, mybir.dt.int16

    t_xtab = nc.dram_tensor("xtab", [2 * HALF, F_IN], bf16,
                            kind="ExternalInput")
    t_xT = nc.dram_tensor("xT", [F_IN, PADN], f32, kind="ExternalInput")
    t_idx = nc.dram_tensor("idx", [totslot], i16, kind="ExternalInput")
    wnames = ["Wl1", "Wr1", "Wl2", "Wr2", "Wla", "Wra", "Wlm", "Wrm"]
    wshapes = {"Wl1": (F_IN, H), "Wr1": (F_IN, H)}
    t_w = {w: nc.dram_tensor(w, list(wshapes.get(w, (H, H))), f32,
                             kind="ExternalInput") for w in wnames}
    t_b = {b: nc.dram_tensor(b, [H, 1], f32, kind="ExternalInput")
           for b in ["bl1", "bl2", "bla", "blm"]}
    t_wh = {w: nc.dram_tensor(w, [H, 1], f32, kind="ExternalInput")
            for w in ["Wa", "Wm"]}
    t_bh = {b: nc.dram_tensor(b, [1, 1], f32, kind="ExternalInput")
            for b in ["ba", "bm"]}
    t_out = nc.dram_tensor("out", [2, NLOC], f32, kind="ExternalOutput")

    NCH = (TILES + CHUNK - 1) // CHUNK
    cw_of = lambda c: min(CHUNK, TILES - c * CHUNK) * 128

    sched_of_tile = {}
    for (t, phase, col0, kn, c, cidx0) in sched:
        sched_of_tile.setdefault(t, []).append((phase, col0, kn, cidx0))
    CMAX = max(cols for (_, cols) in chunks)

    with tile.TileContext(nc) as tc:
        with tc.tile_pool(name="const", bufs=1) as cpool, \
             tc.tile_pool(name="hT", bufs=1) as hpool, \
             tc.tile_pool(name="work", bufs=2) as wk, \
             tc.tile_pool(name="psT", bufs=4, space="PSUM") as psT, \
             tc.tile_pool(name="psY", bufs=2, space="PSUM") as psY, \
             tc.tile_pool(name="dram", bufs=1, space="DRAM") as dram:

            ident = cpool.tile([128, 128], f32, name="ident")
            make_identity(nc, ident[:])

            w_sb = {}
            for w in wnames:
                fi = wshapes.get(w, (H, H))[0]
                fh = fi // 128
                ws = cpool.tile([128, fh * H], f32, name=f"sb_{w}")
                for h in range(fh):
                    nc.sync.dma_start(ws[:, h * H:(h + 1) * H],
                                      t_w[w][h * 128:(h + 1) * 128, :])
                w_sb[w] = ws
            b_sb = {}
            for b in t_b:
                bs = cpool.tile([128, 2], f32, name=f"sb_{b}")
                for h in range(2):
                    nc.sync.dma_start(bs[:, h:h + 1],
                                      t_b[b][h * 128:(h + 1) * 128, :])
                b_sb[b] = bs
            wh_sb = {}
            for w in t_wh:
                ws = cpool.tile([128, 2], f32, name=f"sb_{w}")
                for h in range(2):
                    nc.sync.dma_start(ws[:, h:h + 1],
                                      t_wh[w][h * 128:(h + 1) * 128, :])
                wh_sb[w] = ws
            bh_sb = {}
            for b in t_bh:
                bs = cpool.tile([1, 1], f32, name=f"sb_{b}")
                nc.sync.dma_start(bs[:], t_bh[b][:])
                bh_sb[b] = bs

            xT_sb = hpool.tile([128, PADN], f32, name="xT_sb")
            nc.sync.dma_start(xT_sb[:], t_xT[:])
            h1T = hpool.tile([128, 2 * PADN], f32, name="h1T")
            h2T = hpool.tile([128, 2 * PADN], f32, name="h2T")

            h1tab = dram.tile([2 * HALF, H], bf16, name="h1tab",
                              addr_space="Shared")
            h2tab = dram.tile([2 * HALF, H], bf16, name="h2tab",
                              addr_space="Shared")
            blk1 = dram.tile([BLOCK, H], bf16, name="blk1")
            blk2 = dram.tile([BLOCK, H], bf16, name="blk2")

            # each core's block ends with a -inf pad row
            padrow = cpool.tile([1, H], bf16, name="padrow")
            nc.vector.memset(padrow[:], NEG)
            nc.sync.dma_start(blk1[NLOC:NLOC + 1, :], padrow[:])
            nc.sync.dma_start(blk2[NLOC:NLOC + 1, :], padrow[:])

            def load_idx_chunk(c, tag):
                off, cols = chunks[c]
                idxc = wk.tile([128, CMAX], i16, name=f"idxc_{tag}",
                               tag="idxc", bufs=3)
                nc.sync.dma_start(
                    idxc[:, :cols],
                    t_idx[off:off + 128 * cols].rearrange("(p s) -> p s",
                                                          p=128))
                return idxc

            def aggregate_tile(t, table, F, tag, idxc):
                """two-phase gather + tree-max + upcast for node tile t.
                Returns an f32 [128, F] tile."""
                agg16 = wk.tile([128, H], bf16, name=f"agg16_{tag}",
                                tag="agg16")
                first = True
                for (phase, col0, kn, cidx0) in sched_of_tile[t]:
                    cols = 8 * kn
                    g = wk.tile([128, KCAP * H], bf16, name=f"g_{tag}",
                                tag="gather", bufs=3)
                    view = table[0:HALF, :] if phase == 0 \
                        else table[HALF:2 * HALF, :]
                    nc.gpsimd.dma_gather(
                        out_ap=g[:, :kn * F].rearrange("p (k f) -> p k f",
                                                       f=F),
                        in_ap=view, idxs_ap=idxc[:, cidx0:cidx0 + cols],
                        num_idxs=128 * kn, num_idxs_reg=128 * kn,
                        elem_size=F, single_packet=False)
                    _tree_max(nc, g, kn, F)
                    if first:
                        nc.vector.tensor_copy(agg16[:, :F], g[:, :F])
                        first = False
                    else:
                        nc.vector.tensor_tensor(out=agg16[:, :F],
                                                in0=agg16[:, :F],
                                                in1=g[:, :F],
                                                op=mybir.AluOpType.max)
                agg32 = wk.tile([128, H], f32, name=f"agg32_{tag}",
                                tag="agg32")
                nc.vector.tensor_copy(agg32[:, :F], agg16[:, :F])
                return agg32

            def transpose_into(srcap, dst, col, tag):
                tp = psT.tile([128, 128], f32, name=f"tp_{tag}", tag="tp")
                nc.tensor.transpose(tp[:], srcap, ident[:])
                nc.vector.tensor_copy(dst[:, col:col + 128], tp[:])

            def layer(table, selfT, F, Wl, Wr, bl, outT, blkout, tag):
                fh_in = F // 128
                if stages == 0:
                    for c in range(NCH):
                        idxc = load_idx_chunk(c, f"{tag}_{c}")
                        for i in range(cw_of(c) // 128):
                            t = c * CHUNK + i
                            agg32 = aggregate_tile(t, table, F, f"{tag}_{t}",
                                                   idxc)
                            rows = min(128, NLOC - t * 128)
                            nc.sync.dma_start(
                                blkout[t * 128:t * 128 + rows, 0:F],
                                agg32[:rows, :F])
                    return
                for c in range(NCH):
                    cw = cw_of(c)
                    ntile = cw // 128
                    idxc = load_idx_chunk(c, f"{tag}_{c}")
                    aggT = wk.tile([128, fh_in * 512], f32,
                                   name=f"aggT_{tag}", tag="aggT")
                    for i in range(ntile):
                        t = c * CHUNK + i
                        agg32 = aggregate_tile(t, table, F, f"{tag}_{t}",
                                               idxc)
                        for fh in range(fh_in):
                            transpose_into(agg32[:, fh * 128:(fh + 1) * 128],
                                           aggT, fh * 512 + i * 128,
                                           f"{tag}_{t}_{fh}")
                    for hh in range(2):
                        psy = psY.tile([128, 512], f32, name=f"psy_{tag}",
                                       tag="psy")
                        nmm = 2 * fh_in
                        i = 0
                        for fh in range(fh_in):
                            nc.tensor.matmul(
                                psy[:, :cw],
                                w_sb[Wl][:, fh * H + hh * 128:
                                         fh * H + (hh + 1) * 128],
                                aggT[:, fh * 512:fh * 512 + cw],
                                start=(i == 0), stop=(i == nmm - 1))
                            i += 1
                            nc.tensor.matmul(
                                psy[:, :cw],
                                w_sb[Wr][:, fh * H + hh * 128:
                                         fh * H + (hh + 1) * 128],
                                selfT[:, fh * PADN + c * CHUNK * 128:
                                      fh * PADN + c * CHUNK * 128 + cw],
                                start=(i == 0), stop=(i == nmm - 1))
                            i += 1
                        nc.scalar.activation(
                            outT[:, hh * PADN + c * CHUNK * 128:
                                 hh * PADN + c * CHUNK * 128 + cw],
                            psy[:, :cw],
                            mybir.ActivationFunctionType.Relu,
                            bias=b_sb[bl][:, hh:hh + 1])
                    for i in range(ntile):
                        t = c * CHUNK + i
                        ynode = wk.tile([128, H], bf16, name=f"yn_{tag}",
                                        tag="ynode")
                        for hh in range(2):
                            tp = psT.tile([128, 128], f32,
                                          name=f"tpo_{tag}", tag="tp")
                            nc.tensor.transpose(
                                tp[:],
                                outT[:, hh * PADN + t * 128:
                                     hh * PADN + (t + 1) * 128],
                                ident[:])
                            nc.vector.tensor_copy(
                                ynode[:, hh * 128:(hh + 1) * 128], tp[:])
                        rows = min(128, NLOC - t * 128)
                        nc.sync.dma_start(blkout[t * 128:t * 128 + rows, :],
                                          ynode[:rows, :])

            layer(t_xtab, xT_sb, F_IN, "Wl1", "Wr1", "bl1", h1T, blk1, "l1")
            if stages == 0:
                pass
            if stages >= 2:
                nc.gpsimd.collective_compute(
                    "AllGather", mybir.AluOpType.bypass,
                    replica_groups=[list(range(NCOR))],
                    ins=[blk1.opt()], outs=[h1tab.opt()])
            if stages >= 3:
                layer(h1tab, h1T, H, "Wl2", "Wr2", "bl2", h2T, blk2, "l2")
            if stages >= 4:
                nc.gpsimd.collective_compute(
                    "AllGather", mybir.AluOpType.bypass,
                    replica_groups=[list(range(NCOR))],
                    ins=[blk2.opt()], outs=[h2tab.opt()])

            # layer 3: two branches + heads
            for c in range(NCH if stages >= 5 else 0):
                cw = cw_of(c)
                ntile = cw // 128
                idxc = load_idx_chunk(c, f"l3_{c}")
                aggT = wk.tile([128, 2 * 512], f32, name="aggT_l3",
                               tag="aggT")
                for i in range(ntile):
                    t = c * CHUNK + i
                    agg32 = aggregate_tile(t, h2tab, H, f"l3_{t}", idxc)
                    for fh in range(2):
                        transpose_into(agg32[:, fh * 128:(fh + 1) * 128],
                                       aggT, fh * 512 + i * 128,
                                       f"l3_{t}_{fh}")
                out_sbs = [wk.tile([1, 512], f32, name=f"out_sb{bi}",
                                   tag=f"out_sb{bi}") for bi in range(2)]
                for bi, (Wl, Wr, bl, Wh, bh) in enumerate(
                        [("Wla", "Wra", "bla", "Wa", "ba"),
                         ("Wlm", "Wrm", "blm", "Wm", "bm")]):
                    brT = wk.tile([128, 2 * 512], f32, name=f"brT{bi}",
                                  tag="brT")
                    for hh in range(2):
                        psy = psY.tile([128, 512], f32, name=f"psy3_{bi}",
                                       tag="psy")
                        for fh in range(2):
                            nc.tensor.matmul(
                                psy[:, :cw],
                                w_sb[Wl][:, fh * H + hh * 128:
                                         fh * H + (hh + 1) * 128],
                                aggT[:, fh * 512:fh * 512 + cw],
                                start=(fh == 0), stop=False)
                            nc.tensor.matmul(
                                psy[:, :cw],
                                w_sb[Wr][:, fh * H + hh * 128:
                                         fh * H + (hh + 1) * 128],
                                h2T[:, fh * PADN + c * CHUNK * 128:
                                    fh * PADN + c * CHUNK * 128 + cw],
                                start=False, stop=(fh == 1))
                        nc.scalar.activation(
                            brT[:, hh * 512:hh * 512 + cw], psy[:, :cw],
                            mybir.ActivationFunctionType.Relu,
                            bias=b_sb[bl][:, hh:hh + 1])
                    psh = psY.tile([1, 512], f32, name=f"psh{bi}", tag="psh")
                    for hh in range(2):
                        nc.tensor.matmul(psh[:, :cw],
                                         wh_sb[Wh][:, hh:hh + 1],
                                         brT[:, hh * 512:hh * 512 + cw],
                                         start=(hh == 0), stop=(hh == 1))
                    nc.scalar.activation(out_sbs[bi][:, :cw],
                                         psh[:, :cw],
                                         mybir.ActivationFunctionType.Identity,
                                         bias=bh_sb[bh][:])
                live = min(cw, NLOC - c * CHUNK * 128)
                for bi in range(2):
                    nc.sync.dma_start(
                        t_out[bi:bi + 1,
                              c * CHUNK * 128:c * CHUNK * 128 + live],
                        out_sbs[bi][:, :live])

    nc.compile()
    return nc


# ----------------------------------------------------------------------------
# entry point
# ----------------------------------------------------------------------------

def kernel(x, edge_index, Wl1, bl1, Wr1, Wl2, bl2, Wr2,
           Wla, bla, Wra, Wa, ba, Wlm, blm, Wrm, Wm, bm):
    x = np.asarray(x, np.float32)
    pp = _preprocess(edge_index)
    old_of_new = pp["old_of_new"]

    # x gather table in block layout: per core 6250 rows + one -inf pad row
    xp = x[old_of_new]
    xtab = np.empty((2 * HALF, F_IN), np.float32)
    for m in range(NCOR):
        base = m * BLOCK if m < 4 else HALF + (m - 4) * BLOCK
        xtab[base:base + NLOC] = xp[m * NLOC:(m + 1) * NLOC]
        xtab[base + NLOC] = NEG
    xtab = xtab.astype(ml_dtypes.bfloat16)

    nc = _build_program(pp["sched"], pp["chunks"], pp["totslot"],
                        pp["isolated"])

    def f32(a):
        return np.ascontiguousarray(np.asarray(a, np.float32))

    in_maps = []
    for m in range(NCOR):
        blk = xp[m * NLOC:(m + 1) * NLOC]
        xT = np.zeros((F_IN, PADN), np.float32)
        xT[:, :NLOC] = blk.T
        in_maps.append({
            "xtab": xtab, "xT": xT, "idx": pp["idx_flat"][m],
            "Wl1": f32(Wl1), "Wr1": f32(Wr1),
            "Wl2": f32(Wl2), "Wr2": f32(Wr2),
            "Wla": f32(Wla), "Wra": f32(Wra),
            "Wlm": f32(Wlm), "Wrm": f32(Wrm),
            "bl1": f32(bl1).reshape(H, 1), "bl2": f32(bl2).reshape(H, 1),
            "bla": f32(bla).reshape(H, 1), "blm": f32(blm).reshape(H, 1),
            "Wa": f32(Wa).reshape(H, 1), "Wm": f32(Wm).reshape(H, 1),
            "ba": f32(ba).reshape(1, 1), "bm": f32(bm).reshape(1, 1),
        })

    res = run_bass_kernel_spmd(nc, in_maps, core_ids=list(range(NCOR)))

    rt = np.empty(N, np.float32)
    mv = np.empty(N, np.float32)
    for m in range(NCOR):
        out = res.results[m]["out"]
        rt[m * NLOC:(m + 1) * NLOC] = out[0]
        mv[m * NLOC:(m + 1) * NLOC] = out[1]
    rt_o = np.empty(N, np.float32)
    mv_o = np.empty(N, np.float32)
    rt_o[old_of_new] = rt
    mv_o[old_of_new] = mv

    _LAST.update(nc=nc, in_maps=in_maps, pp=pp)
    return (rt_o, mv_o)



# revision 4
# speedup vs baseline: 1022.3388x; 1022.3388x over previous
"""GraphSAGE-max (3 layers + 2 heads) on 8 Trainium2 NeuronCores.

Strategy: data-parallel over dst-node partitions with replicated bf16 feature
tables in DRAM. The critical resource is GPSIMD descriptor generation for
dma_gather (~7.8ns per gathered row), so the schedule minimizes gathered
rows:

  - int16 gather indices can only span 32767 table rows, so the 50008-row
    table is covered by two OVERLAPPING views: view0 = rows [0, 32767),
    view1 = rows [17241, 50008). Sources in the 15526-row overlap can be
    fetched by either phase; high out-degree nodes are steered into the
    overlap (owner cores 3,4) so ~39% of edges are free-choice.
  - Per dst-node the free edges are split between the phases to equalize the
    per-tile max slot counts: K0[t]+K1[t] = max(max_deg, max_f0+max_f1),
    bringing ELL padding from 1.39x down to ~1.20x.

Each layer, per 128-node tile: dma_gather neighbor rows (one call per phase
per 16-slot group) -> tree-max on DVE -> bf16 matmuls (weights stationary)
-> relu -> PE-transpose back to node-major -> store to the core's block ->
AllGather blocks -> next layer's table.
"""

import numpy as np
import ml_dtypes

import concourse.bass as bass
import concourse.bacc as bacc
import concourse.mybir as mybir
import concourse.tile as tile
from concourse.masks import make_identity
from concourse.bass_utils import run_bass_kernel_spmd

N = 50000
E = 800000
F_IN = 128
H = 256
NCOR = 8
NLOC = N // NCOR             # 6250
BLOCK = NLOC + 1             # 6251 rows per core block (last = -inf pad)
TAB = NCOR * BLOCK           # 50008 table rows
VSIZE = 32767                # int16-addressable view size
V1_START = TAB - VSIZE       # 17241; overlap = [V1_START, VSIZE)
PAD0 = NLOC                  # view0 pad idx: B0's pad row (table row 6250)
PAD1 = VSIZE - 1             # view1 pad idx: B7's pad row (50007-17241)
TILES = (NLOC + 127) // 128  # 49
PADN = TILES * 128           # 6272
NEG = float(np.finfo(np.float32).min)
KCAP = 16                    # max gather columns per dma_gather call
CHUNK = 4                    # node tiles per matmul chunk (N free = 512)

_LAST = {}                   # stash for the test harness


# ----------------------------------------------------------------------------
# host-side graph preprocessing
# ----------------------------------------------------------------------------

def _wrap_idx(ilist):
    """ilist [NCOR, num] int -> dma_gather wrapped layout [NCOR, 128*ceil(num/16)]
    (16-partition wrap, replicated to 128 partitions)."""
    num = ilist.shape[1]
    cols = (num + 15) // 16
    w = np.zeros((NCOR, 16, cols), np.int16)
    i = np.arange(num)
    w[:, i % 16, i // 16] = ilist
    w = np.tile(w, (1, 8, 1))                 # [NCOR, 128, cols]
    return w.reshape(NCOR, 128 * cols)


def _preprocess(edge_index):
    src = np.asarray(edge_index[0], np.int64)
    dst = np.asarray(edge_index[1], np.int64)
    deg = np.bincount(dst, minlength=N)
    odeg = np.bincount(src, minlength=N)

    # node -> core: groups of 8 by in-degree desc (keeps per-core in-degree
    # profiles aligned); within each group the highest out-degree nodes go
    # to cores 3,4 whose blocks sit fully inside the view overlap.
    order = np.argsort(-deg, kind="stable")
    CORE_PREF = np.array([3, 4, 2, 5, 1, 6, 0, 7])
    owner = np.empty(N, np.int64)
    for g in range(N // 8):
        grp = order[g * 8:(g + 1) * 8]
        sub = grp[np.argsort(-odeg[grp], kind="stable")]
        owner[sub] = CORE_PREF

    def positions(keys):
        old_of_new = np.empty(N, np.int64)
        for m in range(NCOR):
            nodes = np.where(owner == m)[0]
            k = np.lexsort(tuple(reversed([kk[nodes] for kk in keys])))
            old_of_new[m * NLOC:(m + 1) * NLOC] = nodes[k]
        new_of_old = np.empty(N, np.int64)
        new_of_old[old_of_new] = np.arange(N)
        return old_of_new, new_of_old

    def table_pos(new_of_old):
        return (new_of_old // NLOC) * BLOCK + new_of_old % NLOC

    def forced_counts(tpos):
        sp = tpos[src]
        f0 = np.bincount(dst[sp < V1_START], minlength=N)
        f1 = np.bincount(dst[sp >= VSIZE], minlength=N)
        return f0, f1

    # pass 1: in-degree sort -> forced counts; pass 2: secondary sort by f0
    # clusters similar phase-0 loads into the same tile.
    _, new1 = positions([-deg])
    f0a, _ = forced_counts(table_pos(new1))
    old_of_new, new_of_old = positions([-deg, -f0a])
    tpos = table_pos(new_of_old)
    f0, f1 = forced_counts(tpos)

    # per-tile phase budgets (shared across cores)
    rank = new_of_old % NLOC
    tilei = rank // 128
    K0 = np.zeros(TILES, np.int64)
    K1 = np.zeros(TILES, np.int64)
    for t in range(TILES):
        sel = tilei == t
        mf0, mf1, mk = f0[sel].max(), f1[sel].max(), deg[sel].max()
        Ts = max(mk, mf0 + mf1)
        a = min(max((Ts + 1) // 2, mf0), Ts - mf1)
        K0[t], K1[t] = max(int(a), 1), max(int(Ts - a), 1)

    # per-dst phase-0 quota, then slot assignment
    k0cap = K0[tilei]
    k1cap = K1[tilei]
    freec = deg - f0 - f1
    dlo = np.maximum(np.minimum(f0 + freec, k0cap), deg - k1cap)

    nd = new_of_old[dst]
    sp = tpos[src]
    is_f0 = sp < V1_START
    is_f1 = sp >= VSIZE
    # order edges per dst: forced0 first, then free, then forced1; the first
    # dlo[d] edges go to phase 0.
    cls = np.where(is_f0, 0, np.where(is_f1, 2, 1)).astype(np.int64)
    ekey = nd * 4 + cls
    eorder = np.argsort(ekey, kind="stable")
    nd_s = nd[eorder]
    sp_s = sp[eorder]
    starts = np.searchsorted(nd_s, np.arange(N))
    slot_in_dst = np.arange(E) - starts[nd_s]
    ph_s = (slot_in_dst >= dlo[old_of_new[nd_s]]).astype(np.int64)
    # sanity: forced edges must land in their required phase
    # (guaranteed by construction: dlo >= f0, deg-dlo >= f1, and the
    #  forced0-free-forced1 ordering)
    idx0 = sp_s - 0
    idx1 = sp_s - V1_START
    loc = np.where(ph_s == 0, idx0, idx1)
    slot_in_ph = np.where(ph_s == 0, slot_in_dst,
                          slot_in_dst - dlo[old_of_new[nd_s]])

    # dense ELL per phase [NCOR, PADN, Kmax]
    def ell_of(phase, kcol, padidx):
        ell = np.full((NCOR, PADN, int(kcol.max())), padidx, np.int16)
        sel = ph_s == phase
        nde = nd_s[sel]
        ell[nde // NLOC, nde % NLOC, slot_in_ph[sel]] = loc[sel].astype(np.int16)
        return ell
    ells = {0: ell_of(0, K0, PAD0), 1: ell_of(1, K1, PAD1)}

    # gather call schedule + wrapped int16 index stream, grouped per chunk
    NCH = (TILES + CHUNK - 1) // CHUNK
    sched = []           # (tile, phase, kn, chunk, cidx0)
    chunks = []          # (flat_off, cols) per chunk
    blocks = []
    off = 0
    for c in range(NCH):
        cblocks = []
        ccols = 0
        for t in range(c * CHUNK, min((c + 1) * CHUNK, TILES)):
            for phase, ks in ((0, K0), (1, K1)):
                k0 = 0
                while k0 < int(ks[t]):
                    kn = min(KCAP, int(ks[t]) - k0)
                    blk = ells[phase][:, t * 128:(t + 1) * 128, k0:k0 + kn]
                    ilist = blk.transpose(0, 2, 1).reshape(NCOR, kn * 128)
                    w = _wrap_idx(ilist).reshape(NCOR, 128, 8 * kn)
                    cblocks.append(w)
                    sched.append((t, phase, kn, c, ccols))
                    ccols += 8 * kn
                    k0 += kn
        cb = np.concatenate(cblocks, axis=2)      # [NCOR, 128, ccols]
        blocks.append(cb.reshape(NCOR, 128 * ccols))
        chunks.append((off, ccols))
        off += 128 * ccols
    idx_flat = np.concatenate(blocks, axis=1)

    return dict(new_of_old=new_of_old, old_of_new=old_of_new,
                sched=sched, chunks=chunks, totslot=off, idx_flat=idx_flat)


# ----------------------------------------------------------------------------
# device program
# ----------------------------------------------------------------------------

def _tree_max(nc, g, k, F):
    """In-place max over k column groups of width F; result in g[:, :F]."""
    while k > 1:
        if k % 2 == 1:
            nc.vector.tensor_tensor(out=g[:, 0:F], in0=g[:, 0:F],
                                    in1=g[:, (k - 1) * F:k * F],
                                    op=mybir.AluOpType.max)
            k -= 1
            if k == 1:
                break
        half = k // 2
        nc.vector.tensor_tensor(out=g[:, 0:half * F], in0=g[:, 0:half * F],
                                in1=g[:, half * F:2 * half * F],
                                op=mybir.AluOpType.max)
        k = half


def _build_program(sched, chunks, totslot):
    nc = bacc.Bacc("TRN2", target_bir_lowering=False, debug=False,
                   num_devices=NCOR)
    f32, bf16, i16 = mybir.dt.float32, mybir.dt.bfloat16, mybir.dt.int16

    t_xtab = nc.dram_tensor("xtab", [TAB, F_IN], bf16, kind="ExternalInput")
    t_xT = nc.dram_tensor("xT", [F_IN, PADN], bf16, kind="ExternalInput")
    t_idx = nc.dram_tensor("idx", [totslot], i16, kind="ExternalInput")
    wnames = ["Wl1", "Wr1", "Wl2", "Wr2", "Wla", "Wra", "Wlm", "Wrm"]
    wshapes = {"Wl1": (F_IN, H), "Wr1": (F_IN, H)}
    t_w = {w: nc.dram_tensor(w, list(wshapes.get(w, (H, H))), bf16,
                             kind="ExternalInput") for w in wnames}
    t_b = {b: nc.dram_tensor(b, [H, 1], f32, kind="ExternalInput")
           for b in ["bl1", "bl2", "bla", "blm"]}
    t_wh = {w: nc.dram_tensor(w, [H, 1], bf16, kind="ExternalInput")
            for w in ["Wa", "Wm"]}
    t_bh = {b: nc.dram_tensor(b, [1, 1], f32, kind="ExternalInput")
            for b in ["ba", "bm"]}
    t_out = nc.dram_tensor("out", [2, NLOC], f32, kind="ExternalOutput")

    NCH = (TILES + CHUNK - 1) // CHUNK
    cw_of = lambda c: min(CHUNK, TILES - c * CHUNK) * 128

    sched_of_tile = {}
    for (t, phase, kn, c, cidx0) in sched:
        sched_of_tile.setdefault(t, []).append((phase, kn, cidx0))
    CMAX = max(cols for (_, cols) in chunks)

    with tile.TileContext(nc) as tc:
        with tc.tile_pool(name="const", bufs=1) as cpool, \
             tc.tile_pool(name="hT", bufs=1) as hpool, \
             tc.tile_pool(name="work", bufs=2) as wk, \
             tc.tile_pool(name="psT", bufs=4, space="PSUM") as psT, \
             tc.tile_pool(name="psY", bufs=2, space="PSUM") as psY, \
             tc.tile_pool(name="dram", bufs=1, space="DRAM") as dram:

            ident = cpool.tile([128, 128], bf16, name="ident")
            make_identity(nc, ident[:])

            w_sb = {}
            for w in wnames:
                fi = wshapes.get(w, (H, H))[0]
                fh = fi // 128
                ws = cpool.tile([128, fh * H], bf16, name=f"sb_{w}")
                for h in range(fh):
                    nc.sync.dma_start(ws[:, h * H:(h + 1) * H],
                                      t_w[w][h * 128:(h + 1) * 128, :])
                w_sb[w] = ws
            b_sb = {}
            for b in t_b:
                bs = cpool.tile([128, 2], f32, name=f"sb_{b}")
                for h in range(2):
                    nc.sync.dma_start(bs[:, h:h + 1],
                                      t_b[b][h * 128:(h + 1) * 128, :])
                b_sb[b] = bs
            wh_sb = {}
            for w in t_wh:
                ws = cpool.tile([128, 2], bf16, name=f"sb_{w}")
                for h in range(2):
                    nc.sync.dma_start(ws[:, h:h + 1],
                                      t_wh[w][h * 128:(h + 1) * 128, :])
                wh_sb[w] = ws
            bh_sb = {}
            for b in t_bh:
                bs = cpool.tile([1, 1], f32, name=f"sb_{b}")
                nc.sync.dma_start(bs[:], t_bh[b][:])
                bh_sb[b] = bs

            xT_sb = hpool.tile([128, PADN], bf16, name="xT_sb")
            nc.sync.dma_start(xT_sb[:], t_xT[:])
            h1T = hpool.tile([128, 2 * PADN], bf16, name="h1T")
            h2T = hpool.tile([128, 2 * PADN], bf16, name="h2T")

            h1tab = dram.tile([TAB, H], bf16, name="h1tab",
                              addr_space="Shared")
            h2tab = dram.tile([TAB, H], bf16, name="h2tab",
                              addr_space="Shared")
            blk1 = dram.tile([BLOCK, H], bf16, name="blk1")
            blk2 = dram.tile([BLOCK, H], bf16, name="blk2")

            # each core's block ends with a -inf pad row
            padrow = cpool.tile([1, H], bf16, name="padrow")
            nc.vector.memset(padrow[:], NEG)
            nc.sync.dma_start(blk1[NLOC:NLOC + 1, :], padrow[:])
            nc.sync.dma_start(blk2[NLOC:NLOC + 1, :], padrow[:])

            def load_idx_chunk(c, tag):
                off, cols = chunks[c]
                idxc = wk.tile([128, CMAX], i16, name=f"idxc_{tag}",
                               tag="idxc", bufs=3)
                nc.sync.dma_start(
                    idxc[:, :cols],
                    t_idx[off:off + 128 * cols].rearrange("(p s) -> p s",
                                                          p=128))
                return idxc

            def aggregate_tile(t, table, F, tag, idxc):
                """two-phase gather + tree-max for node tile t -> bf16 [128, F]."""
                agg16 = wk.tile([128, H], bf16, name=f"agg16_{tag}",
                                tag="agg16", bufs=3)
                first = True
                for (phase, kn, cidx0) in sched_of_tile[t]:
                    cols = 8 * kn
                    g = wk.tile([128, KCAP * H], bf16, name=f"g_{tag}",
                                tag="gather", bufs=4)
                    view = table[0:VSIZE, :] if phase == 0 \
                        else table[V1_START:TAB, :]
                    nc.gpsimd.dma_gather(
                        out_ap=g[:, :kn * F].rearrange("p (k f) -> p k f",
                                                       f=F),
                        in_ap=view, idxs_ap=idxc[:, cidx0:cidx0 + cols],
                        num_idxs=128 * kn, num_idxs_reg=128 * kn,
                        elem_size=F, single_packet=False)
                    _tree_max(nc, g, kn, F)
                    if first:
                        nc.vector.tensor_copy(agg16[:, :F], g[:, :F])
                        first = False
                    else:
                        nc.vector.tensor_tensor(out=agg16[:, :F],
                                                in0=agg16[:, :F],
                                                in1=g[:, :F],
                                                op=mybir.AluOpType.max)
                return agg16

            def transpose_into(srcap, dst, col, tag):
                tp = psT.tile([128, 128], bf16, name=f"tp_{tag}", tag="tp")
                nc.tensor.transpose(tp[:], srcap, ident[:])
                nc.vector.tensor_copy(dst[:, col:col + 128], tp[:])

            def layer(table, selfT, F, Wl, Wr, bl, outT, blkout, tag):
                fh_in = F // 128
                for c in range(NCH):
                    cw = cw_of(c)
                    ntile = cw // 128
                    idxc = load_idx_chunk(c, f"{tag}_{c}")
                    aggT = wk.tile([128, fh_in * 512], bf16,
                                   name=f"aggT_{tag}", tag="aggT")
                    for i in range(ntile):
                        t = c * CHUNK + i
                        agg16 = aggregate_tile(t, table, F, f"{tag}_{t}",
                                               idxc)
                        for fh in range(fh_in):
                            transpose_into(agg16[:, fh * 128:(fh + 1) * 128],
                                           aggT, fh * 512 + i * 128,
                                           f"{tag}_{t}_{fh}")
                    for hh in range(2):
                        psy = psY.tile([128, 512], f32, name=f"psy_{tag}",
                                       tag="psy")
                        nmm = 2 * fh_in
                        i = 0
                        for fh in range(fh_in):
                            nc.tensor.matmul(
                                psy[:, :cw],
                                w_sb[Wl][:, fh * H + hh * 128:
                                         fh * H + (hh + 1) * 128],
                                aggT[:, fh * 512:fh * 512 + cw],
                                start=(i == 0), stop=(i == nmm - 1))
                            i += 1
                            nc.tensor.matmul(
                                psy[:, :cw],
                                w_sb[Wr][:, fh * H + hh * 128:
                                         fh * H + (hh + 1) * 128],
                                selfT[:, fh * PADN + c * CHUNK * 128:
                                      fh * PADN + c * CHUNK * 128 + cw],
                                start=(i == 0), stop=(i == nmm - 1))
                            i += 1
                        nc.scalar.activation(
                            outT[:, hh * PADN + c * CHUNK * 128:
                                 hh * PADN + c * CHUNK * 128 + cw],
                            psy[:, :cw],
                            mybir.ActivationFunctionType.Relu,
                            bias=b_sb[bl][:, hh:hh + 1])
                    for i in range(ntile):
                        t = c * CHUNK + i
                        ynode = wk.tile([128, H], bf16, name=f"yn_{tag}",
                                        tag="ynode", bufs=3)
                        for hh in range(2):
                            tp = psT.tile([128, 128], bf16,
                                          name=f"tpo_{tag}", tag="tp")
                            nc.tensor.transpose(
                                tp[:],
                                outT[:, hh * PADN + t * 128:
                                     hh * PADN + (t + 1) * 128],
                                ident[:])
                            nc.vector.tensor_copy(
                                ynode[:, hh * 128:(hh + 1) * 128], tp[:])
                        rows = min(128, NLOC - t * 128)
                        nc.sync.dma_start(blkout[t * 128:t * 128 + rows, :],
                                          ynode[:rows, :])

            layer(t_xtab, xT_sb, F_IN, "Wl1", "Wr1", "bl1", h1T, blk1, "l1")
            nc.gpsimd.collective_compute(
                "AllGather", mybir.AluOpType.bypass,
                replica_groups=[list(range(NCOR))],
                ins=[blk1.opt()], outs=[h1tab.opt()])
            layer(h1tab, h1T, H, "Wl2", "Wr2", "bl2", h2T, blk2, "l2")
            nc.gpsimd.collective_compute(
                "AllGather", mybir.AluOpType.bypass,
                replica_groups=[list(range(NCOR))],
                ins=[blk2.opt()], outs=[h2tab.opt()])

            # layer 3: two branches + heads
            for c in range(NCH):
                cw = cw_of(c)
                ntile = cw // 128
                idxc = load_idx_chunk(c, f"l3_{c}")
                aggT = wk.tile([128, 2 * 512], bf16, name="aggT_l3",
                               tag="aggT")
                for i in range(ntile):
                    t = c * CHUNK + i
                    agg16 = aggregate_tile(t, h2tab, H, f"l3_{t}", idxc)
                    for fh in range(2):
                        transpose_into(agg16[:, fh * 128:(fh + 1) * 128],
                                       aggT, fh * 512 + i * 128,
                                       f"l3_{t}_{fh}")
                out_sbs = [wk.tile([1, 512], f32, name=f"out_sb{bi}",
                                   tag=f"out_sb{bi}") for bi in range(2)]
                for bi, (Wl, Wr, bl, Wh, bh) in enumerate(
                        [("Wla", "Wra", "bla", "Wa", "ba"),
                         ("Wlm", "Wrm", "blm", "Wm", "bm")]):
                    brT = wk.tile([128, 2 * 512], bf16, name=f"brT{bi}",
                                  tag="brT")
                    for hh in range(2):
                        psy = psY.tile([128, 512], f32, name=f"psy3_{bi}",
                                       tag="psy")
                        for fh in range(2):
                            nc.tensor.matmul(
                                psy[:, :cw],
                                w_sb[Wl][:, fh * H + hh * 128:
                                         fh * H + (hh + 1) * 128],
                                aggT[:, fh * 512:fh * 512 + cw],
                                start=(fh == 0), stop=False)
                            nc.tensor.matmul(
                                psy[:, :cw],
                                w_sb[Wr][:, fh * H + hh * 128:
                                         fh * H + (hh + 1) * 128],
                                h2T[:, fh * PADN + c * CHUNK * 128:
                                    fh * PADN + c * CHUNK * 128 + cw],
                                start=False, stop=(fh == 1))
                        nc.scalar.activation(
                            brT[:, hh * 512:hh * 512 + cw], psy[:, :cw],
                            mybir.ActivationFunctionType.Relu,
                            bias=b_sb[bl][:, hh:hh + 1])
                    psh = psY.tile([1, 512], f32, name=f"psh{bi}", tag="psh")
                    for hh in range(2):
                        nc.tensor.matmul(psh[:, :cw],
                                         wh_sb[Wh][:, hh:hh + 1],
                                         brT[:, hh * 512:hh * 512 + cw],
                                         start=(hh == 0), stop=(hh == 1))
                    nc.scalar.activation(out_sbs[bi][:, :cw],
                                         psh[:, :cw],
                                         mybir.ActivationFunctionType.Identity,
                                         bias=bh_sb[bh][:])
                live = min(cw, NLOC - c * CHUNK * 128)
                for bi in range(2):
                    nc.sync.dma_start(
                        t_out[bi:bi + 1,
                              c * CHUNK * 128:c * CHUNK * 128 + live],
                        out_sbs[bi][:, :live])

    nc.compile()
    return nc


# ----------------------------------------------------------------------------
# entry point
# ----------------------------------------------------------------------------

def kernel(x, edge_index, Wl1, bl1, Wr1, Wl2, bl2, Wr2,
           Wla, bla, Wra, Wa, ba, Wlm, blm, Wrm, Wm, bm):
    x = np.asarray(x, np.float32)
    pp = _preprocess(edge_index)
    old_of_new = pp["old_of_new"]

    # x gather table in block layout: per core 6250 rows + one -inf pad row
    xp = x[old_of_new]
    xtab = np.empty((TAB, F_IN), np.float32)
    for m in range(NCOR):
        base = m * BLOCK
        xtab[base:base + NLOC] = xp[m * NLOC:(m + 1) * NLOC]
        xtab[base + NLOC] = NEG
    xtab = xtab.astype(ml_dtypes.bfloat16)

    nc = _build_program(pp["sched"], pp["chunks"], pp["totslot"])

    def f32(a):
        return np.ascontiguousarray(np.asarray(a, np.float32))

    def b16(a):
        return np.ascontiguousarray(
            np.asarray(a, np.float32).astype(ml_dtypes.bfloat16))

    in_maps = []
    for m in range(NCOR):
        blk = xp[m * NLOC:(m + 1) * NLOC]
        xT = np.zeros((F_IN, PADN), np.float32)
        xT[:, :NLOC] = blk.T
        in_maps.append({
            "xtab": xtab, "xT": xT.astype(ml_dtypes.bfloat16),
            "idx": pp["idx_flat"][m],
            "Wl1": b16(Wl1), "Wr1": b16(Wr1),
            "Wl2": b16(Wl2), "Wr2": b16(Wr2),
            "Wla": b16(Wla), "Wra": b16(Wra),
            "Wlm": b16(Wlm), "Wrm": b16(Wrm),
            "bl1": f32(bl1).reshape(H, 1), "bl2": f32(bl2).reshape(H, 1),
            "bla": f32(bla).reshape(H, 1), "blm": f32(blm).reshape(H, 1),
            "Wa": b16(Wa).reshape(H, 1), "Wm": b16(Wm).reshape(H, 1),
            "ba": f32(ba).reshape(1, 1), "bm": f32(bm).reshape(1, 1),
        })

    res = run_bass_kernel_spmd(nc, in_maps, core_ids=list(range(NCOR)))

    rt = np.empty(N, np.float32)
    mv = np.empty(N, np.float32)
    for m in range(NCOR):
        out = res.results[m]["out"]
        rt[m * NLOC:(m + 1) * NLOC] = out[0]
        mv[m * NLOC:(m + 1) * NLOC] = out[1]
    rt_o = np.empty(N, np.float32)
    mv_o = np.empty(N, np.float32)
    rt_o[old_of_new] = rt
    mv_o[old_of_new] = mv

    _LAST.update(nc=nc, in_maps=in_maps, pp=pp)
    return (rt_o, mv_o)


# revision 6
# speedup vs baseline: 1099.5306x; 1.0755x over previous
"""GraphSAGE-max (3 layers + 2 heads) on 8 Trainium2 NeuronCores.

Strategy: data-parallel over dst-node partitions with replicated bf16 feature
tables in DRAM. The critical resource is GPSIMD descriptor generation for
dma_gather (~7.8ns per gathered row), so the schedule minimizes gathered
rows:

  - int16 gather indices can only span 32767 table rows, so the 50008-row
    table is covered by two OVERLAPPING views: view0 = rows [0, 32767),
    view1 = rows [17241, 50008). Sources in the 15526-row overlap can be
    fetched by either phase; high out-degree nodes are steered into the
    overlap (owner cores 3,4) so ~39% of edges are free-choice.
  - Per dst-node the free edges are split between the phases to equalize the
    per-tile max slot counts: K0[t]+K1[t] = max(max_deg, max_f0+max_f1),
    bringing ELL padding from 1.39x down to ~1.20x.

Each layer, per 128-node tile: dma_gather neighbor rows (one call per phase
per 16-slot group) -> tree-max on DVE -> bf16 matmuls (weights stationary)
-> relu -> PE-transpose back to node-major -> store to the core's block ->
AllGather blocks -> next layer's table.
"""

import numpy as np
import ml_dtypes

import concourse.bass as bass
import concourse.bacc as bacc
import concourse.mybir as mybir
import concourse.tile as tile
from concourse.masks import make_identity
from concourse.bass_utils import run_bass_kernel_spmd

N = 50000
E = 800000
F_IN = 128
H = 256
NCOR = 8
NLOC = N // NCOR             # 6250
BLOCK = NLOC + 1             # 6251 rows per core block (last = -inf pad)
TAB = NCOR * BLOCK           # 50008 table rows
VSIZE = 32767                # int16-addressable view size
V1_START = TAB - VSIZE       # 17241; overlap = [V1_START, VSIZE)
PAD0 = NLOC                  # view0 pad idx: B0's pad row (table row 6250)
PAD1 = VSIZE - 1             # view1 pad idx: B7's pad row (50007-17241)
TILES = (NLOC + 127) // 128  # 49
PADN = TILES * 128           # 6272
NEG = float(np.finfo(np.float32).min)
KCAP = 8                     # max gather columns per dma_gather call
CHUNK = 4                    # node tiles per matmul chunk (N free = 512)

_LAST = {}                   # stash for the test harness


# ----------------------------------------------------------------------------
# host-side graph preprocessing
# ----------------------------------------------------------------------------

def _wrap_idx(ilist):
    """ilist [NCOR, num] int -> dma_gather wrapped layout [NCOR, 128*ceil(num/16)]
    (16-partition wrap, replicated to 128 partitions)."""
    num = ilist.shape[1]
    cols = (num + 15) // 16
    w = np.zeros((NCOR, 16, cols), np.int16)
    i = np.arange(num)
    w[:, i % 16, i // 16] = ilist
    w = np.tile(w, (1, 8, 1))                 # [NCOR, 128, cols]
    return w.reshape(NCOR, 128 * cols)


def _preprocess(edge_index):
    src = np.asarray(edge_index[0], np.int64)
    dst = np.asarray(edge_index[1], np.int64)
    deg = np.bincount(dst, minlength=N)
    odeg = np.bincount(src, minlength=N)

    # node -> core: groups of 8 by in-degree desc (keeps per-core in-degree
    # profiles aligned); within each group the highest out-degree nodes go
    # to cores 3,4 whose blocks sit fully inside the view overlap.
    order = np.argsort(-deg, kind="stable")
    CORE_PREF = np.array([3, 4, 2, 5, 1, 6, 0, 7])
    owner = np.empty(N, np.int64)
    for g in range(N // 8):
        grp = order[g * 8:(g + 1) * 8]
        sub = grp[np.argsort(-odeg[grp], kind="stable")]
        owner[sub] = CORE_PREF

    def positions(keys):
        old_of_new = np.empty(N, np.int64)
        for m in range(NCOR):
            nodes = np.where(owner == m)[0]
            k = np.lexsort(tuple(reversed([kk[nodes] for kk in keys])))
            old_of_new[m * NLOC:(m + 1) * NLOC] = nodes[k]
        new_of_old = np.empty(N, np.int64)
        new_of_old[old_of_new] = np.arange(N)
        return old_of_new, new_of_old

    def table_pos(new_of_old):
        return (new_of_old // NLOC) * BLOCK + new_of_old % NLOC

    def forced_counts(tpos):
        sp = tpos[src]
        f0 = np.bincount(dst[sp < V1_START], minlength=N)
        f1 = np.bincount(dst[sp >= VSIZE], minlength=N)
        return f0, f1

    # pass 1: in-degree sort -> forced counts; pass 2: secondary sort by f0
    # clusters similar phase-0 loads into the same tile.
    _, new1 = positions([-deg])
    f0a, _ = forced_counts(table_pos(new1))
    old_of_new, new_of_old = positions([-deg, -f0a])
    tpos = table_pos(new_of_old)
    f0, f1 = forced_counts(tpos)

    # per-tile phase budgets (shared across cores)
    rank = new_of_old % NLOC
    tilei = rank // 128
    K0 = np.zeros(TILES, np.int64)
    K1 = np.zeros(TILES, np.int64)
    for t in range(TILES):
        sel = tilei == t
        mf0, mf1, mk = f0[sel].max(), f1[sel].max(), deg[sel].max()
        Ts = max(mk, mf0 + mf1)
        a = min(max((Ts + 1) // 2, mf0), Ts - mf1)
        K0[t], K1[t] = max(int(a), 1), max(int(Ts - a), 1)

    # per-dst phase-0 quota, then slot assignment
    k0cap = K0[tilei]
    k1cap = K1[tilei]
    freec = deg - f0 - f1
    dlo = np.maximum(np.minimum(f0 + freec, k0cap), deg - k1cap)

    nd = new_of_old[dst]
    sp = tpos[src]
    is_f0 = sp < V1_START
    is_f1 = sp >= VSIZE
    # order edges per dst: forced0 first, then free, then forced1; the first
    # dlo[d] edges go to phase 0.
    cls = np.where(is_f0, 0, np.where(is_f1, 2, 1)).astype(np.int64)
    ekey = nd * 4 + cls
    eorder = np.argsort(ekey, kind="stable")
    nd_s = nd[eorder]
    sp_s = sp[eorder]
    starts = np.searchsorted(nd_s, np.arange(N))
    slot_in_dst = np.arange(E) - starts[nd_s]
    ph_s = (slot_in_dst >= dlo[old_of_new[nd_s]]).astype(np.int64)
    # sanity: forced edges must land in their required phase
    # (guaranteed by construction: dlo >= f0, deg-dlo >= f1, and the
    #  forced0-free-forced1 ordering)
    idx0 = sp_s - 0
    idx1 = sp_s - V1_START
    loc = np.where(ph_s == 0, idx0, idx1)
    slot_in_ph = np.where(ph_s == 0, slot_in_dst,
                          slot_in_dst - dlo[old_of_new[nd_s]])

    # dense ELL per phase [NCOR, PADN, Kmax]
    def ell_of(phase, kcol, padidx):
        ell = np.full((NCOR, PADN, int(kcol.max())), padidx, np.int16)
        sel = ph_s == phase
        nde = nd_s[sel]
        ell[nde // NLOC, nde % NLOC, slot_in_ph[sel]] = loc[sel].astype(np.int16)
        return ell
    ells = {0: ell_of(0, K0, PAD0), 1: ell_of(1, K1, PAD1)}

    # gather call schedule + wrapped int16 index stream, grouped per chunk
    NCH = (TILES + CHUNK - 1) // CHUNK
    sched = []           # (tile, phase, kn, chunk, cidx0)
    chunks = []          # (flat_off, cols) per chunk
    blocks = []
    off = 0
    for c in range(NCH):
        cblocks = []
        ccols = 0
        for t in range(c * CHUNK, min((c + 1) * CHUNK, TILES)):
            for phase, ks in ((0, K0), (1, K1)):
                k0 = 0
                while k0 < int(ks[t]):
                    kn = min(KCAP, int(ks[t]) - k0)
                    blk = ells[phase][:, t * 128:(t + 1) * 128, k0:k0 + kn]
                    ilist = blk.transpose(0, 2, 1).reshape(NCOR, kn * 128)
                    w = _wrap_idx(ilist).reshape(NCOR, 128, 8 * kn)
                    cblocks.append(w)
                    sched.append((t, phase, kn, c, ccols))
                    ccols += 8 * kn
                    k0 += kn
        cb = np.concatenate(cblocks, axis=2)      # [NCOR, 128, ccols]
        blocks.append(cb.reshape(NCOR, 128 * ccols))
        chunks.append((off, ccols))
        off += 128 * ccols
    idx_flat = np.concatenate(blocks, axis=1)

    return dict(new_of_old=new_of_old, old_of_new=old_of_new,
                sched=sched, chunks=chunks, totslot=off, idx_flat=idx_flat)


# ----------------------------------------------------------------------------
# device program
# ----------------------------------------------------------------------------

def _tree_max(nc, g, k, F):
    """In-place max over k column groups of width F; result in g[:, :F]."""
    while k > 1:
        if k % 2 == 1:
            nc.vector.tensor_tensor(out=g[:, 0:F], in0=g[:, 0:F],
                                    in1=g[:, (k - 1) * F:k * F],
                                    op=mybir.AluOpType.max)
            k -= 1
            if k == 1:
                break
        half = k // 2
        nc.vector.tensor_tensor(out=g[:, 0:half * F], in0=g[:, 0:half * F],
                                in1=g[:, half * F:2 * half * F],
                                op=mybir.AluOpType.max)
        k = half


def _build_program(sched, chunks, totslot):
    nc = bacc.Bacc("TRN2", target_bir_lowering=False, debug=False,
                   num_devices=NCOR)
    f32, bf16, i16 = mybir.dt.float32, mybir.dt.bfloat16, mybir.dt.int16

    t_xtab = nc.dram_tensor("xtab", [TAB, F_IN], bf16, kind="ExternalInput")
    t_xT = nc.dram_tensor("xT", [F_IN, PADN], bf16, kind="ExternalInput")
    t_idx = nc.dram_tensor("idx", [totslot], i16, kind="ExternalInput")
    wnames = ["Wl1", "Wr1", "Wl2", "Wr2", "Wla", "Wra", "Wlm", "Wrm"]
    wshapes = {"Wl1": (F_IN, H), "Wr1": (F_IN, H)}
    t_w = {w: nc.dram_tensor(w, list(wshapes.get(w, (H, H))), bf16,
                             kind="ExternalInput") for w in wnames}
    t_b = {b: nc.dram_tensor(b, [H, 1], f32, kind="ExternalInput")
           for b in ["bl1", "bl2", "bla", "blm"]}
    t_wh = {w: nc.dram_tensor(w, [H, 1], bf16, kind="ExternalInput")
            for w in ["Wa", "Wm"]}
    t_bh = {b: nc.dram_tensor(b, [1, 1], f32, kind="ExternalInput")
            for b in ["ba", "bm"]}
    t_out = nc.dram_tensor("out", [2, NLOC], f32, kind="ExternalOutput")

    NCH = (TILES + CHUNK - 1) // CHUNK
    cw_of = lambda c: min(CHUNK, TILES - c * CHUNK) * 128

    sched_of_tile = {}
    for (t, phase, kn, c, cidx0) in sched:
        sched_of_tile.setdefault(t, []).append((phase, kn, cidx0))
    CMAX = max(cols for (_, cols) in chunks)

    with tile.TileContext(nc) as tc:
        with tc.tile_pool(name="const", bufs=1) as cpool, \
             tc.tile_pool(name="hT", bufs=1) as hpool, \
             tc.tile_pool(name="work", bufs=2) as wk, \
             tc.tile_pool(name="psT", bufs=2, space="PSUM") as psT, \
             tc.tile_pool(name="psY", bufs=2, space="PSUM") as psY, \
             tc.tile_pool(name="dram", bufs=1, space="DRAM") as dram:

            ident = cpool.tile([128, 128], f32, name="ident")
            make_identity(nc, ident[:])
            ident16 = cpool.tile([128, 128], bf16, name="ident16")
            make_identity(nc, ident16[:])

            w_sb = {}
            for w in wnames:
                fi = wshapes.get(w, (H, H))[0]
                fh = fi // 128
                ws = cpool.tile([128, fh * H], bf16, name=f"sb_{w}")
                for h in range(fh):
                    nc.sync.dma_start(ws[:, h * H:(h + 1) * H],
                                      t_w[w][h * 128:(h + 1) * 128, :])
                w_sb[w] = ws
            b_sb = {}
            for b in t_b:
                bs = cpool.tile([128, 2], f32, name=f"sb_{b}")
                for h in range(2):
                    nc.sync.dma_start(bs[:, h:h + 1],
                                      t_b[b][h * 128:(h + 1) * 128, :])
                b_sb[b] = bs
            wh_sb = {}
            for w in t_wh:
                ws = cpool.tile([128, 2], bf16, name=f"sb_{w}")
                for h in range(2):
                    nc.sync.dma_start(ws[:, h:h + 1],
                                      t_wh[w][h * 128:(h + 1) * 128, :])
                wh_sb[w] = ws
            bh_sb = {}
            for b in t_bh:
                bs = cpool.tile([1, 1], f32, name=f"sb_{b}")
                nc.sync.dma_start(bs[:], t_bh[b][:])
                bh_sb[b] = bs

            xT_sb = hpool.tile([128, PADN], bf16, name="xT_sb")
            nc.sync.dma_start(xT_sb[:], t_xT[:])
            h1T = hpool.tile([128, 2 * PADN], bf16, name="h1T")
            h2T = hpool.tile([128, 2 * PADN], bf16, name="h2T")

            h1tab = dram.tile([TAB, H], bf16, name="h1tab",
                              addr_space="Shared")
            h2tab = dram.tile([TAB, H], bf16, name="h2tab",
                              addr_space="Shared")
            blk1 = dram.tile([BLOCK, H], bf16, name="blk1")
            blk2 = dram.tile([BLOCK, H], bf16, name="blk2")

            # each core's block ends with a -inf pad row
            padrow = cpool.tile([1, H], bf16, name="padrow")
            nc.vector.memset(padrow[:], NEG)
            nc.sync.dma_start(blk1[NLOC:NLOC + 1, :], padrow[:])
            nc.sync.dma_start(blk2[NLOC:NLOC + 1, :], padrow[:])

            def load_idx_chunk(c, tag):
                off, cols = chunks[c]
                idxc = wk.tile([128, CMAX], i16, name=f"idxc_{tag}",
                               tag="idxc", bufs=3)
                nc.sync.dma_start(
                    idxc[:, :cols],
                    t_idx[off:off + 128 * cols].rearrange("(p s) -> p s",
                                                          p=128))
                return idxc

            def aggregate_tile(t, table, F, tag, idxc):
                """two-phase gather + tree-max for node tile t -> bf16 [128, F]."""
                agg16 = wk.tile([128, H], bf16, name=f"agg16_{tag}",
                                tag="agg16", bufs=3)
                first = True
                for (phase, kn, cidx0) in sched_of_tile[t]:
                    cols = 8 * kn
                    g = wk.tile([128, KCAP * H], bf16, name=f"g_{tag}",
                                tag="gather", bufs=4)
                    view = table[0:VSIZE, :] if phase == 0 \
                        else table[V1_START:TAB, :]
                    nc.gpsimd.dma_gather(
                        out_ap=g[:, :kn * F].rearrange("p (k f) -> p k f",
                                                       f=F),
                        in_ap=view, idxs_ap=idxc[:, cidx0:cidx0 + cols],
                        num_idxs=128 * kn, num_idxs_reg=128 * kn,
                        elem_size=F, single_packet=False)
                    _tree_max(nc, g, kn, F)
                    if first:
                        nc.vector.tensor_copy(agg16[:, :F], g[:, :F])
                        first = False
                    else:
                        nc.vector.tensor_tensor(out=agg16[:, :F],
                                                in0=agg16[:, :F],
                                                in1=g[:, :F],
                                                op=mybir.AluOpType.max)
                return agg16

            def transpose_into(srcap, dst, col, tag):
                tp = psT.tile([128, 128], f32, name=f"tp_{tag}", tag="tpf")
                nc.tensor.transpose(tp[:], srcap, ident[:])
                nc.vector.tensor_copy(dst[:, col:col + 128], tp[:])

            def layer(table, selfT, F, Wl, Wr, bl, outT, blkout, tag):
                fh_in = F // 128
                for c in range(NCH):
                    cw = cw_of(c)
                    ntile = cw // 128
                    idxc = load_idx_chunk(c, f"{tag}_{c}")
                    aggT = wk.tile([128, fh_in * 512], bf16,
                                   name=f"aggT_{tag}", tag="aggT")
                    for i in range(ntile):
                        t = c * CHUNK + i
                        agg16 = aggregate_tile(t, table, F, f"{tag}_{t}",
                                               idxc)
                        agg32 = wk.tile([128, H], f32, name=f"a32_{tag}",
                                        tag="agg32", bufs=3)
                        nc.vector.tensor_copy(agg32[:, :F], agg16[:, :F])
                        for fh in range(fh_in):
                            transpose_into(agg32[:, fh * 128:(fh + 1) * 128],
                                           aggT, fh * 512 + i * 128,
                                           f"{tag}_{t}_{fh}")
                    for hh in range(2):
                        psy = psY.tile([128, 512], f32, name=f"psy_{tag}",
                                       tag="psy")
                        nmm = 2 * fh_in
                        i = 0
                        for fh in range(fh_in):
                            nc.tensor.matmul(
                                psy[:, :cw],
                                w_sb[Wl][:, fh * H + hh * 128:
                                         fh * H + (hh + 1) * 128],
                                aggT[:, fh * 512:fh * 512 + cw],
                                start=(i == 0), stop=(i == nmm - 1))
                            i += 1
                            nc.tensor.matmul(
                                psy[:, :cw],
                                w_sb[Wr][:, fh * H + hh * 128:
                                         fh * H + (hh + 1) * 128],
                                selfT[:, fh * PADN + c * CHUNK * 128:
                                      fh * PADN + c * CHUNK * 128 + cw],
                                start=(i == 0), stop=(i == nmm - 1))
                            i += 1
                        nc.scalar.activation(
                            outT[:, hh * PADN + c * CHUNK * 128:
                                 hh * PADN + c * CHUNK * 128 + cw],
                            psy[:, :cw],
                            mybir.ActivationFunctionType.Relu,
                            bias=b_sb[bl][:, hh:hh + 1])
                    for i in range(ntile):
                        t = c * CHUNK + i
                        ynode = wk.tile([128, H], bf16, name=f"yn_{tag}",
                                        tag="ynode", bufs=3)
                        for hh in range(2):
                            tp = psT.tile([128, 128], bf16,
                                          name=f"tpo_{tag}", tag="tp")
                            nc.tensor.transpose(
                                tp[:],
                                outT[:, hh * PADN + t * 128:
                                     hh * PADN + (t + 1) * 128],
                                ident16[:])
                            nc.vector.tensor_copy(
                                ynode[:, hh * 128:(hh + 1) * 128], tp[:])
                        rows = min(128, NLOC - t * 128)
                        nc.sync.dma_start(blkout[t * 128:t * 128 + rows, :],
                                          ynode[:rows, :])

            layer(t_xtab, xT_sb, F_IN, "Wl1", "Wr1", "bl1", h1T, blk1, "l1")
            nc.gpsimd.collective_compute(
                "AllGather", mybir.AluOpType.bypass,
                replica_groups=[list(range(NCOR))],
                ins=[blk1.opt()], outs=[h1tab.opt()])
            layer(h1tab, h1T, H, "Wl2", "Wr2", "bl2", h2T, blk2, "l2")
            nc.gpsimd.collective_compute(
                "AllGather", mybir.AluOpType.bypass,
                replica_groups=[list(range(NCOR))],
                ins=[blk2.opt()], outs=[h2tab.opt()])

            # layer 3: two branches + heads
            for c in range(NCH):
                cw = cw_of(c)
                ntile = cw // 128
                idxc = load_idx_chunk(c, f"l3_{c}")
                aggT = wk.tile([128, 2 * 512], bf16, name="aggT_l3",
                               tag="aggT")
                for i in range(ntile):
                    t = c * CHUNK + i
                    agg16 = aggregate_tile(t, h2tab, H, f"l3_{t}", idxc)
                    agg32 = wk.tile([128, H], f32, name="a32_l3",
                                    tag="agg32", bufs=3)
                    nc.vector.tensor_copy(agg32[:, :H], agg16[:, :H])
                    for fh in range(2):
                        transpose_into(agg32[:, fh * 128:(fh + 1) * 128],
                                       aggT, fh * 512 + i * 128,
                                       f"l3_{t}_{fh}")
                out_sbs = [wk.tile([1, 512], f32, name=f"out_sb{bi}",
                                   tag=f"out_sb{bi}") for bi in range(2)]
                for bi, (Wl, Wr, bl, Wh, bh) in enumerate(
                        [("Wla", "Wra", "bla", "Wa", "ba"),
                         ("Wlm", "Wrm", "blm", "Wm", "bm")]):
                    brT = wk.tile([128, 2 * 512], bf16, name=f"brT{bi}",
                                  tag="brT")
                    for hh in range(2):
                        psy = psY.tile([128, 512], f32, name=f"psy3_{bi}",
                                       tag="psy")
                        for fh in range(2):
                            nc.tensor.matmul(
                                psy[:, :cw],
                                w_sb[Wl][:, fh * H + hh * 128:
                                         fh * H + (hh + 1) * 128],
                                aggT[:, fh * 512:fh * 512 + cw],
                                start=(fh == 0), stop=False)
                            nc.tensor.matmul(
                                psy[:, :cw],
                                w_sb[Wr][:, fh * H + hh * 128:
                                         fh * H + (hh + 1) * 128],
                                h2T[:, fh * PADN + c * CHUNK * 128:
                                    fh * PADN + c * CHUNK * 128 + cw],
                                start=False, stop=(fh == 1))
                        nc.scalar.activation(
                            brT[:, hh * 512:hh * 512 + cw], psy[:, :cw],
                            mybir.ActivationFunctionType.Relu,
                            bias=b_sb[bl][:, hh:hh + 1])
                    psh = psY.tile([1, 512], f32, name=f"psh{bi}", tag="psh")
                    for hh in range(2):
                        nc.tensor.matmul(psh[:, :cw],
                                         wh_sb[Wh][:, hh:hh + 1],
                                         brT[:, hh * 512:hh * 512 + cw],
                                         start=(hh == 0), stop=(hh == 1))
                    nc.scalar.activation(out_sbs[bi][:, :cw],
                                         psh[:, :cw],
                                         mybir.ActivationFunctionType.Identity,
                                         bias=bh_sb[bh][:])
                live = min(cw, NLOC - c * CHUNK * 128)
                for bi in range(2):
                    nc.sync.dma_start(
                        t_out[bi:bi + 1,
                              c * CHUNK * 128:c * CHUNK * 128 + live],
                        out_sbs[bi][:, :live])

    nc.compile()
    return nc


# ----------------------------------------------------------------------------
# entry point
# ----------------------------------------------------------------------------

def kernel(x, edge_index, Wl1, bl1, Wr1, Wl2, bl2, Wr2,
           Wla, bla, Wra, Wa, ba, Wlm, blm, Wrm, Wm, bm):
    x = np.asarray(x, np.float32)
    pp = _preprocess(edge_index)
    old_of_new = pp["old_of_new"]

    # x gather table in block layout: per core 6250 rows + one -inf pad row
    xp = x[old_of_new]
    xtab = np.empty((TAB, F_IN), np.float32)
    for m in range(NCOR):
        base = m * BLOCK
        xtab[base:base + NLOC] = xp[m * NLOC:(m + 1) * NLOC]
        xtab[base + NLOC] = NEG
    xtab = xtab.astype(ml_dtypes.bfloat16)

    nc = _build_program(pp["sched"], pp["chunks"], pp["totslot"])

    def f32(a):
        return np.ascontiguousarray(np.asarray(a, np.float32))

    def b16(a):
        return np.ascontiguousarray(
            np.asarray(a, np.float32).astype(ml_dtypes.bfloat16))

    in_maps = []
    for m in range(NCOR):
        blk = xp[m * NLOC:(m + 1) * NLOC]
        xT = np.zeros((F_IN, PADN), np.float32)
        xT[:, :NLOC] = blk.T
        in_maps.append({
            "xtab": xtab, "xT": xT.astype(ml_dtypes.bfloat16),
            "idx": pp["idx_flat"][m],
            "Wl1": b16(Wl1), "Wr1": b16(Wr1),
            "Wl2": b16(Wl2), "Wr2": b16(Wr2),
            "Wla": b16(Wla), "Wra": b16(Wra),
            "Wlm": b16(Wlm), "Wrm": b16(Wrm),
            "bl1": f32(bl1).reshape(H, 1), "bl2": f32(bl2).reshape(H, 1),
            "bla": f32(bla).reshape(H, 1), "blm": f32(blm).reshape(H, 1),
            "Wa": b16(Wa).reshape(H, 1), "Wm": b16(Wm).reshape(H, 1),
            "ba": f32(ba).reshape(1, 1), "bm": f32(bm).reshape(1, 1),
        })

    res = run_bass_kernel_spmd(nc, in_maps, core_ids=list(range(NCOR)))

    rt = np.empty(N, np.float32)
    mv = np.empty(N, np.float32)
    for m in range(NCOR):
        out = res.results[m]["out"]
        rt[m * NLOC:(m + 1) * NLOC] = out[0]
        mv[m * NLOC:(m + 1) * NLOC] = out[1]
    rt_o = np.empty(N, np.float32)
    mv_o = np.empty(N, np.float32)
    rt_o[old_of_new] = rt
    mv_o[old_of_new] = mv

    _LAST.update(nc=nc, in_maps=in_maps, pp=pp)
    return (rt_o, mv_o)


# revision 7
# speedup vs baseline: 1412.8100x; 1.2849x over previous
"""GraphSAGE-max (3 layers + 2 heads) on 8 Trainium2 NeuronCores.

Strategy: data-parallel over dst-node partitions with replicated bf16 feature
tables in DRAM. The critical resource is GPSIMD descriptor generation for
dma_gather (~7.8ns per gathered row), so the schedule minimizes gathered
rows:

  - int16 gather indices can only span 32767 table rows, so the 50008-row
    table is covered by two OVERLAPPING views: view0 = rows [0, 32767),
    view1 = rows [17241, 50008). Sources in the 15526-row overlap can be
    fetched by either phase; high out-degree nodes are steered into the
    overlap (owner cores 3,4) so ~39% of edges are free-choice.
  - Per dst-node the free edges are split between the phases to equalize the
    per-tile max slot counts: K0[t]+K1[t] = max(max_deg, max_f0+max_f1),
    bringing ELL padding from 1.39x down to ~1.20x.

Each layer, per 128-node tile: dma_gather neighbor rows (one call per phase
per 16-slot group) -> tree-max on DVE -> bf16 matmuls (weights stationary)
-> relu -> PE-transpose back to node-major -> store to the core's block ->
AllGather blocks -> next layer's table.
"""

import numpy as np
import ml_dtypes

import concourse.bass as bass
import concourse.bacc as bacc
import concourse.mybir as mybir
import concourse.tile as tile
from concourse.masks import make_identity
from concourse.bass_utils import run_bass_kernel_spmd

N = 50000
E = 800000
F_IN = 128
H = 256
NCOR = 8
NLOC = N // NCOR             # 6250
BLOCK = NLOC + 1             # 6251 rows per core block (last = -inf pad)
TAB = NCOR * BLOCK           # 50008 table rows
VSIZE = 32767                # int16-addressable view size
V1_START = TAB - VSIZE       # 17241; overlap = [V1_START, VSIZE)
PAD0 = NLOC                  # view0 pad idx: B0's pad row (table row 6250)
PAD1 = VSIZE - 1             # view1 pad idx: B7's pad row (50007-17241)
TILES = (NLOC + 127) // 128  # 49
PADN = TILES * 128           # 6272
NEG = float(np.finfo(np.float32).min)
KCAP = 8                     # max gather columns per dma_gather call
CHUNK = 4                    # node tiles per matmul chunk (N free = 512)

_LAST = {}                   # stash for the test harness


# ----------------------------------------------------------------------------
# host-side graph preprocessing
# ----------------------------------------------------------------------------

def _wrap_idx(ilist):
    """ilist [NCOR, num] int -> dma_gather wrapped layout [NCOR, 128*ceil(num/16)]
    (16-partition wrap, replicated to 128 partitions)."""
    num = ilist.shape[1]
    cols = (num + 15) // 16
    w = np.zeros((NCOR, 16, cols), np.int16)
    i = np.arange(num)
    w[:, i % 16, i // 16] = ilist
    w = np.tile(w, (1, 8, 1))                 # [NCOR, 128, cols]
    return w.reshape(NCOR, 128 * cols)


def _preprocess(edge_index):
    src = np.asarray(edge_index[0], np.int64)
    dst = np.asarray(edge_index[1], np.int64)
    deg = np.bincount(dst, minlength=N)
    odeg = np.bincount(src, minlength=N)

    # node -> core: groups of 8 by in-degree desc (keeps per-core in-degree
    # profiles aligned); within each group the highest out-degree nodes go
    # to cores 3,4 whose blocks sit fully inside the view overlap.
    order = np.argsort(-deg, kind="stable")
    CORE_PREF = np.array([3, 4, 2, 5, 1, 6, 0, 7])
    owner = np.empty(N, np.int64)
    for g in range(N // 8):
        grp = order[g * 8:(g + 1) * 8]
        sub = grp[np.argsort(-odeg[grp], kind="stable")]
        owner[sub] = CORE_PREF

    def positions(keys):
        old_of_new = np.empty(N, np.int64)
        for m in range(NCOR):
            nodes = np.where(owner == m)[0]
            k = np.lexsort(tuple(reversed([kk[nodes] for kk in keys])))
            old_of_new[m * NLOC:(m + 1) * NLOC] = nodes[k]
        new_of_old = np.empty(N, np.int64)
        new_of_old[old_of_new] = np.arange(N)
        return old_of_new, new_of_old

    def table_pos(new_of_old):
        return (new_of_old // NLOC) * BLOCK + new_of_old % NLOC

    def forced_counts(tpos):
        sp = tpos[src]
        f0 = np.bincount(dst[sp < V1_START], minlength=N)
        f1 = np.bincount(dst[sp >= VSIZE], minlength=N)
        return f0, f1

    # pass 1: in-degree sort -> forced counts; pass 2: secondary sort by f0
    # clusters similar phase-0 loads into the same tile.
    _, new1 = positions([-deg])
    f0a, _ = forced_counts(table_pos(new1))
    old_of_new, new_of_old = positions([-deg, -f0a])
    tpos = table_pos(new_of_old)
    f0, f1 = forced_counts(tpos)

    # per-tile phase budgets (shared across cores)
    rank = new_of_old % NLOC
    tilei = rank // 128
    K0 = np.zeros(TILES, np.int64)
    K1 = np.zeros(TILES, np.int64)
    for t in range(TILES):
        sel = tilei == t
        mf0, mf1, mk = f0[sel].max(), f1[sel].max(), deg[sel].max()
        Ts = max(mk, mf0 + mf1)
        a = min(max((Ts + 1) // 2, mf0), Ts - mf1)
        K0[t], K1[t] = max(int(a), 1), max(int(Ts - a), 1)

    # per-dst phase-0 quota, then slot assignment
    k0cap = K0[tilei]
    k1cap = K1[tilei]
    freec = deg - f0 - f1
    dlo = np.maximum(np.minimum(f0 + freec, k0cap), deg - k1cap)

    nd = new_of_old[dst]
    sp = tpos[src]
    is_f0 = sp < V1_START
    is_f1 = sp >= VSIZE
    # order edges per dst: forced0 first, then free, then forced1; the first
    # dlo[d] edges go to phase 0.
    cls = np.where(is_f0, 0, np.where(is_f1, 2, 1)).astype(np.int64)
    ekey = nd * 4 + cls
    eorder = np.argsort(ekey, kind="stable")
    nd_s = nd[eorder]
    sp_s = sp[eorder]
    starts = np.searchsorted(nd_s, np.arange(N))
    slot_in_dst = np.arange(E) - starts[nd_s]
    ph_s = (slot_in_dst >= dlo[old_of_new[nd_s]]).astype(np.int64)
    # sanity: forced edges must land in their required phase
    # (guaranteed by construction: dlo >= f0, deg-dlo >= f1, and the
    #  forced0-free-forced1 ordering)
    idx0 = sp_s - 0
    idx1 = sp_s - V1_START
    loc = np.where(ph_s == 0, idx0, idx1)
    slot_in_ph = np.where(ph_s == 0, slot_in_dst,
                          slot_in_dst - dlo[old_of_new[nd_s]])

    # dense ELL per phase [NCOR, PADN, Kmax]
    def ell_of(phase, kcol, padidx):
        ell = np.full((NCOR, PADN, int(kcol.max())), padidx, np.int16)
        sel = ph_s == phase
        nde = nd_s[sel]
        ell[nde // NLOC, nde % NLOC, slot_in_ph[sel]] = loc[sel].astype(np.int16)
        return ell
    ells = {0: ell_of(0, K0, PAD0), 1: ell_of(1, K1, PAD1)}

    # gather call schedule + wrapped int16 index stream, grouped per chunk
    NCH = (TILES + CHUNK - 1) // CHUNK
    sched = []           # (tile, phase, kn, chunk, cidx0, xoff)
    chunks = []          # (flat_off, cols) per chunk
    blocks = []
    ell_abs = []         # absolute table rows per call [NCOR, kn*128]
    off = 0
    xoff = 0
    for c in range(NCH):
        cblocks = []
        ccols = 0
        for t in range(c * CHUNK, min((c + 1) * CHUNK, TILES)):
            for phase, ks in ((0, K0), (1, K1)):
                k0 = 0
                while k0 < int(ks[t]):
                    kn = min(KCAP, int(ks[t]) - k0)
                    blk = ells[phase][:, t * 128:(t + 1) * 128, k0:k0 + kn]
                    ilist = blk.transpose(0, 2, 1).reshape(NCOR, kn * 128)
                    base = 0 if phase == 0 else V1_START
                    ell_abs.append(ilist.astype(np.int64) + base)
                    w = _wrap_idx(ilist).reshape(NCOR, 128, 8 * kn)
                    cblocks.append(w)
                    sched.append((t, phase, kn, c, ccols, xoff))
                    ccols += 8 * kn
                    xoff += kn
                    k0 += kn
        cb = np.concatenate(cblocks, axis=2)      # [NCOR, 128, ccols]
        blocks.append(cb.reshape(NCOR, 128 * ccols))
        chunks.append((off, ccols))
        off += 128 * ccols
    idx_flat = np.concatenate(blocks, axis=1)
    ell_abs = np.concatenate(ell_abs, axis=1)     # [NCOR, totxslots*128]

    return dict(new_of_old=new_of_old, old_of_new=old_of_new,
                sched=sched, chunks=chunks, totslot=off, idx_flat=idx_flat,
                ell_abs=ell_abs, totx=xoff)


# ----------------------------------------------------------------------------
# device program
# ----------------------------------------------------------------------------

def _tree_max(nc, g, k, F):
    """In-place max over k column groups of width F; result in g[:, :F]."""
    while k > 1:
        if k % 2 == 1:
            nc.vector.tensor_tensor(out=g[:, 0:F], in0=g[:, 0:F],
                                    in1=g[:, (k - 1) * F:k * F],
                                    op=mybir.AluOpType.max)
            k -= 1
            if k == 1:
                break
        half = k // 2
        nc.vector.tensor_tensor(out=g[:, 0:half * F], in0=g[:, 0:half * F],
                                in1=g[:, half * F:2 * half * F],
                                op=mybir.AluOpType.max)
        k = half


def _build_program(sched, chunks, totslot, totx):
    nc = bacc.Bacc("TRN2", target_bir_lowering=False, debug=False,
                   num_devices=NCOR)
    f32, bf16, i16 = mybir.dt.float32, mybir.dt.bfloat16, mybir.dt.int16

    t_xell = nc.dram_tensor("xell", [totx * 128, F_IN], bf16,
                            kind="ExternalInput")
    t_xT = nc.dram_tensor("xT", [F_IN, PADN], bf16, kind="ExternalInput")
    t_idx = nc.dram_tensor("idx", [totslot], i16, kind="ExternalInput")
    wnames = ["Wl1", "Wr1", "Wl2", "Wr2", "Wla", "Wra", "Wlm", "Wrm"]
    wshapes = {"Wl1": (F_IN, H), "Wr1": (F_IN, H)}
    t_w = {w: nc.dram_tensor(w, list(wshapes.get(w, (H, H))), bf16,
                             kind="ExternalInput") for w in wnames}
    t_b = {b: nc.dram_tensor(b, [H, 1], f32, kind="ExternalInput")
           for b in ["bl1", "bl2", "bla", "blm"]}
    t_wh = {w: nc.dram_tensor(w, [H, 1], bf16, kind="ExternalInput")
            for w in ["Wa", "Wm"]}
    t_bh = {b: nc.dram_tensor(b, [1, 1], f32, kind="ExternalInput")
            for b in ["ba", "bm"]}
    t_out = nc.dram_tensor("out", [2, NLOC], f32, kind="ExternalOutput")

    NCH = (TILES + CHUNK - 1) // CHUNK
    cw_of = lambda c: min(CHUNK, TILES - c * CHUNK) * 128

    sched_of_tile = {}
    for (t, phase, kn, c, cidx0, xoff) in sched:
        sched_of_tile.setdefault(t, []).append((phase, kn, cidx0, xoff))
    CMAX = max(cols for (_, cols) in chunks)

    with tile.TileContext(nc) as tc:
        with tc.tile_pool(name="const", bufs=1) as cpool, \
             tc.tile_pool(name="hT", bufs=1) as hpool, \
             tc.tile_pool(name="work", bufs=2) as wk, \
             tc.tile_pool(name="psT", bufs=2, space="PSUM") as psT, \
             tc.tile_pool(name="psY", bufs=2, space="PSUM") as psY, \
             tc.tile_pool(name="dram", bufs=1, space="DRAM") as dram:

            ident = cpool.tile([128, 128], f32, name="ident")
            make_identity(nc, ident[:])
            ident16 = cpool.tile([128, 128], bf16, name="ident16")
            make_identity(nc, ident16[:])

            w_sb = {}
            for w in wnames:
                fi = wshapes.get(w, (H, H))[0]
                fh = fi // 128
                ws = cpool.tile([128, fh * H], bf16, name=f"sb_{w}")
                for h in range(fh):
                    nc.sync.dma_start(ws[:, h * H:(h + 1) * H],
                                      t_w[w][h * 128:(h + 1) * 128, :])
                w_sb[w] = ws
            b_sb = {}
            for b in t_b:
                bs = cpool.tile([128, 2], f32, name=f"sb_{b}")
                for h in range(2):
                    nc.sync.dma_start(bs[:, h:h + 1],
                                      t_b[b][h * 128:(h + 1) * 128, :])
                b_sb[b] = bs
            wh_sb = {}
            for w in t_wh:
                ws = cpool.tile([128, 2], bf16, name=f"sb_{w}")
                for h in range(2):
                    nc.sync.dma_start(ws[:, h:h + 1],
                                      t_wh[w][h * 128:(h + 1) * 128, :])
                wh_sb[w] = ws
            bh_sb = {}
            for b in t_bh:
                bs = cpool.tile([1, 1], f32, name=f"sb_{b}")
                nc.sync.dma_start(bs[:], t_bh[b][:])
                bh_sb[b] = bs

            xT_sb = hpool.tile([128, PADN], bf16, name="xT_sb")
            nc.sync.dma_start(xT_sb[:], t_xT[:])
            h1T = hpool.tile([128, 2 * PADN], bf16, name="h1T")
            h2T = hpool.tile([128, 2 * PADN], bf16, name="h2T")

            h1tab = dram.tile([TAB, H], bf16, name="h1tab",
                              addr_space="Shared")
            h2tab = dram.tile([TAB, H], bf16, name="h2tab",
                              addr_space="Shared")
            blk1 = dram.tile([BLOCK, H], bf16, name="blk1")
            blk2 = dram.tile([BLOCK, H], bf16, name="blk2")

            # each core's block ends with a -inf pad row
            padrow = cpool.tile([1, H], bf16, name="padrow")
            nc.vector.memset(padrow[:], NEG)
            nc.sync.dma_start(blk1[NLOC:NLOC + 1, :], padrow[:])
            nc.sync.dma_start(blk2[NLOC:NLOC + 1, :], padrow[:])

            def load_idx_chunk(c, tag):
                off, cols = chunks[c]
                idxc = wk.tile([128, CMAX], i16, name=f"idxc_{tag}",
                               tag="idxc", bufs=3)
                nc.sync.dma_start(
                    idxc[:, :cols],
                    t_idx[off:off + 128 * cols].rearrange("(p s) -> p s",
                                                          p=128))
                return idxc

            def aggregate_tile(t, table, F, tag, idxc):
                """two-phase gather + tree-max for node tile t -> bf16 [128, F].

                table=None: layer-1 mode, rows stream from the host
                pre-gathered t_xell instead of dma_gather."""
                agg16 = wk.tile([128, H], bf16, name=f"agg16_{tag}",
                                tag="agg16", bufs=3)
                first = True
                for (phase, kn, cidx0, xoff) in sched_of_tile[t]:
                    cols = 8 * kn
                    g = wk.tile([128, KCAP * H], bf16, name=f"g_{tag}",
                                tag="gather", bufs=4)
                    if table is None:
                        nc.sync.dma_start(
                            g[:, :kn * F].rearrange("p (k f) -> p k f", f=F),
                            t_xell[xoff * 128:(xoff + kn) * 128, :].rearrange(
                                "(k p) f -> p k f", p=128))
                    else:
                        view = table[0:VSIZE, :] if phase == 0 \
                            else table[V1_START:TAB, :]
                        nc.gpsimd.dma_gather(
                            out_ap=g[:, :kn * F].rearrange("p (k f) -> p k f",
                                                           f=F),
                            in_ap=view, idxs_ap=idxc[:, cidx0:cidx0 + cols],
                            num_idxs=128 * kn, num_idxs_reg=128 * kn,
                            elem_size=F, single_packet=False)
                    _tree_max(nc, g, kn, F)
                    if first:
                        nc.vector.tensor_copy(agg16[:, :F], g[:, :F])
                        first = False
                    else:
                        nc.vector.tensor_tensor(out=agg16[:, :F],
                                                in0=agg16[:, :F],
                                                in1=g[:, :F],
                                                op=mybir.AluOpType.max)
                return agg16

            def transpose_into(srcap, dst, col, tag):
                tp = psT.tile([128, 128], f32, name=f"tp_{tag}", tag="tpf")
                nc.tensor.transpose(tp[:], srcap, ident[:])
                nc.vector.tensor_copy(dst[:, col:col + 128], tp[:])

            def layer(table, selfT, F, Wl, Wr, bl, outT, blkout, tag):
                fh_in = F // 128
                for c in range(NCH):
                    cw = cw_of(c)
                    ntile = cw // 128
                    idxc = load_idx_chunk(c, f"{tag}_{c}") \
                        if table is not None else None
                    aggT = wk.tile([128, fh_in * 512], bf16,
                                   name=f"aggT_{tag}", tag="aggT")
                    for i in range(ntile):
                        t = c * CHUNK + i
                        agg16 = aggregate_tile(t, table, F, f"{tag}_{t}",
                                               idxc)
                        agg32 = wk.tile([128, H], f32, name=f"a32_{tag}",
                                        tag="agg32", bufs=3)
                        nc.vector.tensor_copy(agg32[:, :F], agg16[:, :F])
                        for fh in range(fh_in):
                            transpose_into(agg32[:, fh * 128:(fh + 1) * 128],
                                           aggT, fh * 512 + i * 128,
                                           f"{tag}_{t}_{fh}")
                    for hh in range(2):
                        psy = psY.tile([128, 512], f32, name=f"psy_{tag}",
                                       tag="psy")
                        nmm = 2 * fh_in
                        i = 0
                        for fh in range(fh_in):
                            nc.tensor.matmul(
                                psy[:, :cw],
                                w_sb[Wl][:, fh * H + hh * 128:
                                         fh * H + (hh + 1) * 128],
                                aggT[:, fh * 512:fh * 512 + cw],
                                start=(i == 0), stop=(i == nmm - 1))
                            i += 1
                            nc.tensor.matmul(
                                psy[:, :cw],
                                w_sb[Wr][:, fh * H + hh * 128:
                                         fh * H + (hh + 1) * 128],
                                selfT[:, fh * PADN + c * CHUNK * 128:
                                      fh * PADN + c * CHUNK * 128 + cw],
                                start=(i == 0), stop=(i == nmm - 1))
                            i += 1
                        nc.scalar.activation(
                            outT[:, hh * PADN + c * CHUNK * 128:
                                 hh * PADN + c * CHUNK * 128 + cw],
                            psy[:, :cw],
                            mybir.ActivationFunctionType.Relu,
                            bias=b_sb[bl][:, hh:hh + 1])
                    for i in range(ntile):
                        t = c * CHUNK + i
                        ynode = wk.tile([128, H], bf16, name=f"yn_{tag}",
                                        tag="ynode", bufs=3)
                        for hh in range(2):
                            tp = psT.tile([128, 128], bf16,
                                          name=f"tpo_{tag}", tag="tp")
                            nc.tensor.transpose(
                                tp[:],
                                outT[:, hh * PADN + t * 128:
                                     hh * PADN + (t + 1) * 128],
                                ident16[:])
                            nc.vector.tensor_copy(
                                ynode[:, hh * 128:(hh + 1) * 128], tp[:])
                        rows = min(128, NLOC - t * 128)
                        nc.sync.dma_start(blkout[t * 128:t * 128 + rows, :],
                                          ynode[:rows, :])

            layer(None, xT_sb, F_IN, "Wl1", "Wr1", "bl1", h1T, blk1, "l1")
            nc.gpsimd.collective_compute(
                "AllGather", mybir.AluOpType.bypass,
                replica_groups=[list(range(NCOR))],
                ins=[blk1.opt()], outs=[h1tab.opt()])
            layer(h1tab, h1T, H, "Wl2", "Wr2", "bl2", h2T, blk2, "l2")
            nc.gpsimd.collective_compute(
                "AllGather", mybir.AluOpType.bypass,
                replica_groups=[list(range(NCOR))],
                ins=[blk2.opt()], outs=[h2tab.opt()])

            # layer 3: two branches + heads
            for c in range(NCH):
                cw = cw_of(c)
                ntile = cw // 128
                idxc = load_idx_chunk(c, f"l3_{c}")
                aggT = wk.tile([128, 2 * 512], bf16, name="aggT_l3",
                               tag="aggT")
                for i in range(ntile):
                    t = c * CHUNK + i
                    agg16 = aggregate_tile(t, h2tab, H, f"l3_{t}", idxc)
                    agg32 = wk.tile([128, H], f32, name="a32_l3",
                                    tag="agg32", bufs=3)
                    nc.vector.tensor_copy(agg32[:, :H], agg16[:, :H])
                    for fh in range(2):
                        transpose_into(agg32[:, fh * 128:(fh + 1) * 128],
                                       aggT, fh * 512 + i * 128,
                                       f"l3_{t}_{fh}")
                out_sbs = [wk.tile([1, 512], f32, name=f"out_sb{bi}",
                                   tag=f"out_sb{bi}") for bi in range(2)]
                for bi, (Wl, Wr, bl, Wh, bh) in enumerate(
                        [("Wla", "Wra", "bla", "Wa", "ba"),
                         ("Wlm", "Wrm", "blm", "Wm", "bm")]):
                    brT = wk.tile([128, 2 * 512], bf16, name=f"brT{bi}",
                                  tag="brT")
                    for hh in range(2):
                        psy = psY.tile([128, 512], f32, name=f"psy3_{bi}",
                                       tag="psy")
                        for fh in range(2):
                            nc.tensor.matmul(
                                psy[:, :cw],
                                w_sb[Wl][:, fh * H + hh * 128:
                                         fh * H + (hh + 1) * 128],
                                aggT[:, fh * 512:fh * 512 + cw],
                                start=(fh == 0), stop=False)
                            nc.tensor.matmul(
                                psy[:, :cw],
                                w_sb[Wr][:, fh * H + hh * 128:
                                         fh * H + (hh + 1) * 128],
                                h2T[:, fh * PADN + c * CHUNK * 128:
                                    fh * PADN + c * CHUNK * 128 + cw],
                                start=False, stop=(fh == 1))
                        nc.scalar.activation(
                            brT[:, hh * 512:hh * 512 + cw], psy[:, :cw],
                            mybir.ActivationFunctionType.Relu,
                            bias=b_sb[bl][:, hh:hh + 1])
                    psh = psY.tile([1, 512], f32, name=f"psh{bi}", tag="psh")
                    for hh in range(2):
                        nc.tensor.matmul(psh[:, :cw],
                                         wh_sb[Wh][:, hh:hh + 1],
                                         brT[:, hh * 512:hh * 512 + cw],
                                         start=(hh == 0), stop=(hh == 1))
                    nc.scalar.activation(out_sbs[bi][:, :cw],
                                         psh[:, :cw],
                                         mybir.ActivationFunctionType.Identity,
                                         bias=bh_sb[bh][:])
                live = min(cw, NLOC - c * CHUNK * 128)
                for bi in range(2):
                    nc.sync.dma_start(
                        t_out[bi:bi + 1,
                              c * CHUNK * 128:c * CHUNK * 128 + live],
                        out_sbs[bi][:, :live])

    nc.compile()
    return nc


# ----------------------------------------------------------------------------
# entry point
# ----------------------------------------------------------------------------

def kernel(x, edge_index, Wl1, bl1, Wr1, Wl2, bl2, Wr2,
           Wla, bla, Wra, Wa, ba, Wlm, blm, Wrm, Wm, bm):
    x = np.asarray(x, np.float32)
    pp = _preprocess(edge_index)
    old_of_new = pp["old_of_new"]

    # x gather table in block layout: per core 6250 rows + one -inf pad row
    xp = x[old_of_new]
    xtab = np.empty((TAB, F_IN), np.float32)
    for m in range(NCOR):
        base = m * BLOCK
        xtab[base:base + NLOC] = xp[m * NLOC:(m + 1) * NLOC]
        xtab[base + NLOC] = NEG
    xtab = xtab.astype(ml_dtypes.bfloat16)

    nc = _build_program(pp["sched"], pp["chunks"], pp["totslot"],
                        pp["totx"])

    def f32(a):
        return np.ascontiguousarray(np.asarray(a, np.float32))

    def b16(a):
        return np.ascontiguousarray(
            np.asarray(a, np.float32).astype(ml_dtypes.bfloat16))

    in_maps = []
    for m in range(NCOR):
        blk = xp[m * NLOC:(m + 1) * NLOC]
        xT = np.zeros((F_IN, PADN), np.float32)
        xT[:, :NLOC] = blk.T
        xell = xtab[pp["ell_abs"][m]]
        in_maps.append({
            "xell": xell, "xT": xT.astype(ml_dtypes.bfloat16),
            "idx": pp["idx_flat"][m],
            "Wl1": b16(Wl1), "Wr1": b16(Wr1),
            "Wl2": b16(Wl2), "Wr2": b16(Wr2),
            "Wla": b16(Wla), "Wra": b16(Wra),
            "Wlm": b16(Wlm), "Wrm": b16(Wrm),
            "bl1": f32(bl1).reshape(H, 1), "bl2": f32(bl2).reshape(H, 1),
            "bla": f32(bla).reshape(H, 1), "blm": f32(blm).reshape(H, 1),
            "Wa": b16(Wa).reshape(H, 1), "Wm": b16(Wm).reshape(H, 1),
            "ba": f32(ba).reshape(1, 1), "bm": f32(bm).reshape(1, 1),
        })

    res = run_bass_kernel_spmd(nc, in_maps, core_ids=list(range(NCOR)))

    rt = np.empty(N, np.float32)
    mv = np.empty(N, np.float32)
    for m in range(NCOR):
        out = res.results[m]["out"]
        rt[m * NLOC:(m + 1) * NLOC] = out[0]
        mv[m * NLOC:(m + 1) * NLOC] = out[1]
    rt_o = np.empty(N, np.float32)
    mv_o = np.empty(N, np.float32)
    rt_o[old_of_new] = rt
    mv_o[old_of_new] = mv

    _LAST.update(nc=nc, in_maps=in_maps, pp=pp)
    return (rt_o, mv_o)


# revision 8
# speedup vs baseline: 1530.2127x; 1.0831x over previous
"""GraphSAGE-max (3 layers + 2 heads) on 8 Trainium2 NeuronCores.

Strategy: data-parallel over dst-node partitions with replicated bf16 feature
tables in DRAM. The critical resource is GPSIMD descriptor generation for
dma_gather (~7.8ns per gathered row), so the schedule minimizes gathered
rows:

  - int16 gather indices can only span 32767 table rows, so the 50008-row
    table is covered by two OVERLAPPING views: view0 = rows [0, 32767),
    view1 = rows [17241, 50008). Sources in the 15526-row overlap can be
    fetched by either phase; high out-degree nodes are steered into the
    overlap (owner cores 3,4) so ~39% of edges are free-choice.
  - Per dst-node the free edges are split between the phases to equalize the
    per-tile max slot counts: K0[t]+K1[t] = max(max_deg, max_f0+max_f1),
    bringing ELL padding from 1.39x down to ~1.20x.

Each layer, per 128-node tile: dma_gather neighbor rows (one call per phase
per 16-slot group) -> tree-max on DVE -> bf16 matmuls (weights stationary)
-> relu -> PE-transpose back to node-major -> store to the core's block ->
AllGather blocks -> next layer's table.
"""

import numpy as np
import ml_dtypes

import concourse.bass as bass
import concourse.bacc as bacc
import concourse.mybir as mybir
import concourse.tile as tile
from concourse.masks import make_identity
from concourse.bass_utils import run_bass_kernel_spmd

N = 50000
E = 800000
F_IN = 128
H = 256
NCOR = 8
NLOC = N // NCOR             # 6250
BLOCK = NLOC + 1             # 6251 rows per core block (last = -inf pad)
TAB = NCOR * BLOCK           # 50008 table rows
VSIZE = 32767                # int16-addressable view size
V1_START = TAB - VSIZE       # 17241; overlap = [V1_START, VSIZE)
PAD0 = NLOC                  # view0 pad idx: B0's pad row (table row 6250)
PAD1 = VSIZE - 1             # view1 pad idx: B7's pad row (50007-17241)
TILES = (NLOC + 127) // 128  # 49
PADN = TILES * 128           # 6272
NEG = float(np.finfo(np.float32).min)
KCAP = 8                     # max gather columns per dma_gather call
CHUNK = 4                    # node tiles per matmul chunk (N free = 512)

_LAST = {}                   # stash for the test harness


# ----------------------------------------------------------------------------
# host-side graph preprocessing
# ----------------------------------------------------------------------------

def _wrap_idx(ilist):
    """ilist [NCOR, num] int -> dma_gather wrapped layout [NCOR, 128*ceil(num/16)]
    (16-partition wrap, replicated to 128 partitions)."""
    num = ilist.shape[1]
    cols = (num + 15) // 16
    w = np.zeros((NCOR, 16, cols), np.int16)
    i = np.arange(num)
    w[:, i % 16, i // 16] = ilist
    w = np.tile(w, (1, 8, 1))                 # [NCOR, 128, cols]
    return w.reshape(NCOR, 128 * cols)


def _preprocess(edge_index):
    src = np.asarray(edge_index[0], np.int64)
    dst = np.asarray(edge_index[1], np.int64)
    deg = np.bincount(dst, minlength=N)
    odeg = np.bincount(src, minlength=N)

    # node -> core: groups of 8 by in-degree desc (keeps per-core in-degree
    # profiles aligned); within each group the highest out-degree nodes go
    # to cores 3,4 whose blocks sit fully inside the view overlap.
    order = np.argsort(-deg, kind="stable")
    CORE_PREF = np.array([3, 4, 2, 5, 1, 6, 0, 7])
    owner = np.empty(N, np.int64)
    for g in range(N // 8):
        grp = order[g * 8:(g + 1) * 8]
        sub = grp[np.argsort(-odeg[grp], kind="stable")]
        owner[sub] = CORE_PREF

    def positions(keys):
        old_of_new = np.empty(N, np.int64)
        for m in range(NCOR):
            nodes = np.where(owner == m)[0]
            k = np.lexsort(tuple(reversed([kk[nodes] for kk in keys])))
            old_of_new[m * NLOC:(m + 1) * NLOC] = nodes[k]
        new_of_old = np.empty(N, np.int64)
        new_of_old[old_of_new] = np.arange(N)
        return old_of_new, new_of_old

    def table_pos(new_of_old):
        return (new_of_old // NLOC) * BLOCK + new_of_old % NLOC

    def forced_counts(tpos):
        sp = tpos[src]
        f0 = np.bincount(dst[sp < V1_START], minlength=N)
        f1 = np.bincount(dst[sp >= VSIZE], minlength=N)
        return f0, f1

    # pass 1: in-degree sort -> forced counts; pass 2: secondary sort by f0
    # clusters similar phase-0 loads into the same tile.
    _, new1 = positions([-deg])
    f0a, _ = forced_counts(table_pos(new1))
    old_of_new, new_of_old = positions([-deg, -f0a])
    tpos = table_pos(new_of_old)
    f0, f1 = forced_counts(tpos)

    # per-tile phase budgets (shared across cores)
    rank = new_of_old % NLOC
    tilei = rank // 128
    K0 = np.zeros(TILES, np.int64)
    K1 = np.zeros(TILES, np.int64)
    for t in range(TILES):
        sel = tilei == t
        mf0, mf1, mk = f0[sel].max(), f1[sel].max(), deg[sel].max()
        Ts = max(mk, mf0 + mf1)
        a = min(max((Ts + 1) // 2, mf0), Ts - mf1)
        K0[t], K1[t] = max(int(a), 1), max(int(Ts - a), 1)

    # per-dst phase-0 quota, then slot assignment
    k0cap = K0[tilei]
    k1cap = K1[tilei]
    freec = deg - f0 - f1
    dlo = np.maximum(np.minimum(f0 + freec, k0cap), deg - k1cap)

    nd = new_of_old[dst]
    sp = tpos[src]
    is_f0 = sp < V1_START
    is_f1 = sp >= VSIZE
    # order edges per dst: forced0 first, then free, then forced1; the first
    # dlo[d] edges go to phase 0.
    cls = np.where(is_f0, 0, np.where(is_f1, 2, 1)).astype(np.int64)
    ekey = nd * 4 + cls
    eorder = np.argsort(ekey, kind="stable")
    nd_s = nd[eorder]
    sp_s = sp[eorder]
    starts = np.searchsorted(nd_s, np.arange(N))
    slot_in_dst = np.arange(E) - starts[nd_s]
    ph_s = (slot_in_dst >= dlo[old_of_new[nd_s]]).astype(np.int64)
    # sanity: forced edges must land in their required phase
    # (guaranteed by construction: dlo >= f0, deg-dlo >= f1, and the
    #  forced0-free-forced1 ordering)
    idx0 = sp_s - 0
    idx1 = sp_s - V1_START
    loc = np.where(ph_s == 0, idx0, idx1)
    slot_in_ph = np.where(ph_s == 0, slot_in_dst,
                          slot_in_dst - dlo[old_of_new[nd_s]])

    # dense ELL per phase [NCOR, PADN, Kmax]
    def ell_of(phase, kcol, padidx):
        ell = np.full((NCOR, PADN, int(kcol.max())), padidx, np.int16)
        sel = ph_s == phase
        nde = nd_s[sel]
        ell[nde // NLOC, nde % NLOC, slot_in_ph[sel]] = loc[sel].astype(np.int16)
        return ell
    ells = {0: ell_of(0, K0, PAD0), 1: ell_of(1, K1, PAD1)}

    # gather call schedule + wrapped int16 index stream, grouped per chunk
    NCH = (TILES + CHUNK - 1) // CHUNK
    sched = []           # (tile, phase, kn, chunk, cidx0, xoff)
    chunks = []          # (flat_off, cols) per chunk
    blocks = []
    ell_abs = []         # absolute table rows per call [NCOR, kn*128]
    off = 0
    xoff = 0
    for c in range(NCH):
        cblocks = []
        ccols = 0
        for t in range(c * CHUNK, min((c + 1) * CHUNK, TILES)):
            for phase, ks in ((0, K0), (1, K1)):
                k0 = 0
                while k0 < int(ks[t]):
                    kn = min(KCAP, int(ks[t]) - k0)
                    blk = ells[phase][:, t * 128:(t + 1) * 128, k0:k0 + kn]
                    ilist = blk.transpose(0, 2, 1).reshape(NCOR, kn * 128)
                    base = 0 if phase == 0 else V1_START
                    ell_abs.append(ilist.astype(np.int64) + base)
                    w = _wrap_idx(ilist).reshape(NCOR, 128, 8 * kn)
                    cblocks.append(w)
                    sched.append((t, phase, kn, c, ccols, xoff))
                    ccols += 8 * kn
                    xoff += kn
                    k0 += kn
        cb = np.concatenate(cblocks, axis=2)      # [NCOR, 128, ccols]
        blocks.append(cb.reshape(NCOR, 128 * ccols))
        chunks.append((off, ccols))
        off += 128 * ccols
    idx_flat = np.concatenate(blocks, axis=1)
    ell_abs = np.concatenate(ell_abs, axis=1)     # [NCOR, totxslots*128]

    return dict(new_of_old=new_of_old, old_of_new=old_of_new,
                sched=sched, chunks=chunks, totslot=off, idx_flat=idx_flat,
                ell_abs=ell_abs, totx=xoff)


# ----------------------------------------------------------------------------
# device program
# ----------------------------------------------------------------------------

def _tree_max(nc, g, k, F):
    """In-place max over k column groups of width F; result in g[:, :F]."""
    while k > 1:
        if k % 2 == 1:
            nc.vector.tensor_tensor(out=g[:, 0:F], in0=g[:, 0:F],
                                    in1=g[:, (k - 1) * F:k * F],
                                    op=mybir.AluOpType.max)
            k -= 1
            if k == 1:
                break
        half = k // 2
        nc.vector.tensor_tensor(out=g[:, 0:half * F], in0=g[:, 0:half * F],
                                in1=g[:, half * F:2 * half * F],
                                op=mybir.AluOpType.max)
        k = half


def _build_program(sched, chunks, totslot, totx):
    nc = bacc.Bacc("TRN2", target_bir_lowering=False, debug=False,
                   num_devices=NCOR)
    f32, bf16, i16 = mybir.dt.float32, mybir.dt.bfloat16, mybir.dt.int16

    t_xell = nc.dram_tensor("xell", [totx * 128, F_IN], bf16,
                            kind="ExternalInput")
    t_xT = nc.dram_tensor("xT", [F_IN, PADN], bf16, kind="ExternalInput")
    t_idx = nc.dram_tensor("idx", [totslot], i16, kind="ExternalInput")
    wnames = ["Wl1", "Wr1", "Wl2", "Wr2", "Wla", "Wra", "Wlm", "Wrm"]
    wshapes = {"Wl1": (F_IN, H), "Wr1": (F_IN, H)}
    t_w = {w: nc.dram_tensor(w, list(wshapes.get(w, (H, H))), bf16,
                             kind="ExternalInput") for w in wnames}
    t_b = {b: nc.dram_tensor(b, [H, 1], f32, kind="ExternalInput")
           for b in ["bl1", "bl2", "bla", "blm"]}
    t_wh = {w: nc.dram_tensor(w, [H, 1], bf16, kind="ExternalInput")
            for w in ["Wa", "Wm"]}
    t_bh = {b: nc.dram_tensor(b, [1, 1], f32, kind="ExternalInput")
            for b in ["ba", "bm"]}
    t_out = nc.dram_tensor("out", [2, NLOC], f32, kind="ExternalOutput")

    NCH = (TILES + CHUNK - 1) // CHUNK
    cw_of = lambda c: min(CHUNK, TILES - c * CHUNK) * 128

    sched_of_tile = {}
    for (t, phase, kn, c, cidx0, xoff) in sched:
        sched_of_tile.setdefault(t, []).append((phase, kn, cidx0, xoff))
    CMAX = max(cols for (_, cols) in chunks)

    with tile.TileContext(nc) as tc:
        with tc.tile_pool(name="const", bufs=1) as cpool, \
             tc.tile_pool(name="hT", bufs=1) as hpool, \
             tc.tile_pool(name="work", bufs=2) as wk, \
             tc.tile_pool(name="psT", bufs=2, space="PSUM") as psT, \
             tc.tile_pool(name="psY", bufs=2, space="PSUM") as psY, \
             tc.tile_pool(name="dram", bufs=1, space="DRAM") as dram:

            ident = cpool.tile([128, 128], f32, name="ident")
            make_identity(nc, ident[:])
            ident16 = cpool.tile([128, 128], bf16, name="ident16")
            make_identity(nc, ident16[:])

            w_sb = {}
            for w in wnames:
                fi = wshapes.get(w, (H, H))[0]
                fh = fi // 128
                ws = cpool.tile([128, fh * H], bf16, name=f"sb_{w}")
                for h in range(fh):
                    nc.sync.dma_start(ws[:, h * H:(h + 1) * H],
                                      t_w[w][h * 128:(h + 1) * 128, :])
                w_sb[w] = ws
            b_sb = {}
            for b in t_b:
                bs = cpool.tile([128, 2], f32, name=f"sb_{b}")
                for h in range(2):
                    nc.sync.dma_start(bs[:, h:h + 1],
                                      t_b[b][h * 128:(h + 1) * 128, :])
                b_sb[b] = bs
            wh_sb = {}
            for w in t_wh:
                ws = cpool.tile([128, 2], bf16, name=f"sb_{w}")
                for h in range(2):
                    nc.sync.dma_start(ws[:, h:h + 1],
                                      t_wh[w][h * 128:(h + 1) * 128, :])
                wh_sb[w] = ws
            bh_sb = {}
            for b in t_bh:
                bs = cpool.tile([1, 1], f32, name=f"sb_{b}")
                nc.sync.dma_start(bs[:], t_bh[b][:])
                bh_sb[b] = bs

            xT_sb = hpool.tile([128, PADN], bf16, name="xT_sb")
            nc.sync.dma_start(xT_sb[:], t_xT[:])
            h1T = hpool.tile([128, 2 * PADN], bf16, name="h1T")
            h2T = hpool.tile([128, 2 * PADN], bf16, name="h2T")

            h1tab = dram.tile([TAB, H], bf16, name="h1tab",
                              addr_space="Shared")
            h2tab = dram.tile([TAB, H], bf16, name="h2tab",
                              addr_space="Shared")
            blk1 = dram.tile([BLOCK, H], bf16, name="blk1")
            blk2 = dram.tile([BLOCK, H], bf16, name="blk2")

            # each core's block ends with a -inf pad row
            padrow = cpool.tile([1, H], bf16, name="padrow")
            nc.vector.memset(padrow[:], NEG)
            nc.sync.dma_start(blk1[NLOC:NLOC + 1, :], padrow[:])
            nc.sync.dma_start(blk2[NLOC:NLOC + 1, :], padrow[:])

            def load_idx_chunk(c, tag):
                off, cols = chunks[c]
                idxc = wk.tile([128, CMAX], i16, name=f"idxc_{tag}",
                               tag="idxc", bufs=4)
                nc.sync.dma_start(
                    idxc[:, :cols],
                    t_idx[off:off + 128 * cols].rearrange("(p s) -> p s",
                                                          p=128))
                return idxc

            def aggregate_tile(t, table, F, tag, idxc):
                """two-phase gather + tree-max for node tile t -> bf16 [128, F].

                table=None: layer-1 mode, rows stream from the host
                pre-gathered t_xell instead of dma_gather."""
                agg16 = wk.tile([128, H], bf16, name=f"agg16_{tag}",
                                tag="agg16", bufs=4)
                first = True
                for (phase, kn, cidx0, xoff) in sched_of_tile[t]:
                    cols = 8 * kn
                    g = wk.tile([128, KCAP * H], bf16, name=f"g_{tag}",
                                tag="gather", bufs=6)
                    if table is None:
                        nc.sync.dma_start(
                            g[:, :kn * F].rearrange("p (k f) -> p k f", f=F),
                            t_xell[xoff * 128:(xoff + kn) * 128, :].rearrange(
                                "(k p) f -> p k f", p=128))
                    else:
                        view = table[0:VSIZE, :] if phase == 0 \
                            else table[V1_START:TAB, :]
                        nc.gpsimd.dma_gather(
                            out_ap=g[:, :kn * F].rearrange("p (k f) -> p k f",
                                                           f=F),
                            in_ap=view, idxs_ap=idxc[:, cidx0:cidx0 + cols],
                            num_idxs=128 * kn, num_idxs_reg=128 * kn,
                            elem_size=F, single_packet=False)
                    _tree_max(nc, g, kn, F)
                    if first:
                        nc.vector.tensor_copy(agg16[:, :F], g[:, :F])
                        first = False
                    else:
                        nc.vector.tensor_tensor(out=agg16[:, :F],
                                                in0=agg16[:, :F],
                                                in1=g[:, :F],
                                                op=mybir.AluOpType.max)
                return agg16

            def transpose_into(srcap, dst, col, tag):
                tp = psT.tile([128, 128], f32, name=f"tp_{tag}", tag="tpf")
                nc.tensor.transpose(tp[:], srcap, ident[:])
                nc.vector.tensor_copy(dst[:, col:col + 128], tp[:])

            def layer(table, selfT, F, Wl, Wr, bl, outT, blkout, tag):
                fh_in = F // 128
                for c in range(NCH):
                    cw = cw_of(c)
                    ntile = cw // 128
                    idxc = load_idx_chunk(c, f"{tag}_{c}") \
                        if table is not None else None
                    aggT = wk.tile([128, fh_in * 512], bf16,
                                   name=f"aggT_{tag}", tag="aggT")
                    for i in range(ntile):
                        t = c * CHUNK + i
                        agg16 = aggregate_tile(t, table, F, f"{tag}_{t}",
                                               idxc)
                        agg32 = wk.tile([128, H], f32, name=f"a32_{tag}",
                                        tag="agg32", bufs=4)
                        nc.any.tensor_copy(agg32[:, :F], agg16[:, :F])
                        for fh in range(fh_in):
                            transpose_into(agg32[:, fh * 128:(fh + 1) * 128],
                                           aggT, fh * 512 + i * 128,
                                           f"{tag}_{t}_{fh}")
                    for hh in range(2):
                        psy = psY.tile([128, 512], f32, name=f"psy_{tag}",
                                       tag="psy")
                        nmm = 2 * fh_in
                        i = 0
                        for fh in range(fh_in):
                            nc.tensor.matmul(
                                psy[:, :cw],
                                w_sb[Wl][:, fh * H + hh * 128:
                                         fh * H + (hh + 1) * 128],
                                aggT[:, fh * 512:fh * 512 + cw],
                                start=(i == 0), stop=(i == nmm - 1))
                            i += 1
                            nc.tensor.matmul(
                                psy[:, :cw],
                                w_sb[Wr][:, fh * H + hh * 128:
                                         fh * H + (hh + 1) * 128],
                                selfT[:, fh * PADN + c * CHUNK * 128:
                                      fh * PADN + c * CHUNK * 128 + cw],
                                start=(i == 0), stop=(i == nmm - 1))
                            i += 1
                        nc.scalar.activation(
                            outT[:, hh * PADN + c * CHUNK * 128:
                                 hh * PADN + c * CHUNK * 128 + cw],
                            psy[:, :cw],
                            mybir.ActivationFunctionType.Relu,
                            bias=b_sb[bl][:, hh:hh + 1])
                    for i in range(ntile):
                        t = c * CHUNK + i
                        ynode = wk.tile([128, H], bf16, name=f"yn_{tag}",
                                        tag="ynode", bufs=3)
                        for hh in range(2):
                            tp = psT.tile([128, 128], bf16,
                                          name=f"tpo_{tag}", tag="tp")
                            nc.tensor.transpose(
                                tp[:],
                                outT[:, hh * PADN + t * 128:
                                     hh * PADN + (t + 1) * 128],
                                ident16[:])
                            nc.vector.tensor_copy(
                                ynode[:, hh * 128:(hh + 1) * 128], tp[:])
                        rows = min(128, NLOC - t * 128)
                        nc.sync.dma_start(blkout[t * 128:t * 128 + rows, :],
                                          ynode[:rows, :])

            layer(None, xT_sb, F_IN, "Wl1", "Wr1", "bl1", h1T, blk1, "l1")
            nc.gpsimd.collective_compute(
                "AllGather", mybir.AluOpType.bypass,
                replica_groups=[list(range(NCOR))],
                ins=[blk1.opt()], outs=[h1tab.opt()])
            layer(h1tab, h1T, H, "Wl2", "Wr2", "bl2", h2T, blk2, "l2")
            nc.gpsimd.collective_compute(
                "AllGather", mybir.AluOpType.bypass,
                replica_groups=[list(range(NCOR))],
                ins=[blk2.opt()], outs=[h2tab.opt()])

            # layer 3: two branches + heads
            for c in range(NCH):
                cw = cw_of(c)
                ntile = cw // 128
                idxc = load_idx_chunk(c, f"l3_{c}")
                aggT = wk.tile([128, 2 * 512], bf16, name="aggT_l3",
                               tag="aggT")
                for i in range(ntile):
                    t = c * CHUNK + i
                    agg16 = aggregate_tile(t, h2tab, H, f"l3_{t}", idxc)
                    agg32 = wk.tile([128, H], f32, name="a32_l3",
                                    tag="agg32", bufs=4)
                    nc.any.tensor_copy(agg32[:, :H], agg16[:, :H])
                    for fh in range(2):
                        transpose_into(agg32[:, fh * 128:(fh + 1) * 128],
                                       aggT, fh * 512 + i * 128,
                                       f"l3_{t}_{fh}")
                out_sbs = [wk.tile([1, 512], f32, name=f"out_sb{bi}",
                                   tag=f"out_sb{bi}") for bi in range(2)]
                for bi, (Wl, Wr, bl, Wh, bh) in enumerate(
                        [("Wla", "Wra", "bla", "Wa", "ba"),
                         ("Wlm", "Wrm", "blm", "Wm", "bm")]):
                    brT = wk.tile([128, 2 * 512], bf16, name=f"brT{bi}",
                                  tag="brT")
                    for hh in range(2):
                        psy = psY.tile([128, 512], f32, name=f"psy3_{bi}",
                                       tag="psy")
                        for fh in range(2):
                            nc.tensor.matmul(
                                psy[:, :cw],
                                w_sb[Wl][:, fh * H + hh * 128:
                                         fh * H + (hh + 1) * 128],
                                aggT[:, fh * 512:fh * 512 + cw],
                                start=(fh == 0), stop=False)
                            nc.tensor.matmul(
                                psy[:, :cw],
                                w_sb[Wr][:, fh * H + hh * 128:
                                         fh * H + (hh + 1) * 128],
                                h2T[:, fh * PADN + c * CHUNK * 128:
                                    fh * PADN + c * CHUNK * 128 + cw],
                                start=False, stop=(fh == 1))
                        nc.scalar.activation(
                            brT[:, hh * 512:hh * 512 + cw], psy[:, :cw],
                            mybir.ActivationFunctionType.Relu,
                            bias=b_sb[bl][:, hh:hh + 1])
                    psh = psY.tile([1, 512], f32, name=f"psh{bi}", tag="psh")
                    for hh in range(2):
                        nc.tensor.matmul(psh[:, :cw],
                                         wh_sb[Wh][:, hh:hh + 1],
                                         brT[:, hh * 512:hh * 512 + cw],
                                         start=(hh == 0), stop=(hh == 1))
                    nc.scalar.activation(out_sbs[bi][:, :cw],
                                         psh[:, :cw],
                                         mybir.ActivationFunctionType.Identity,
                                         bias=bh_sb[bh][:])
                live = min(cw, NLOC - c * CHUNK * 128)
                for bi in range(2):
                    nc.sync.dma_start(
                        t_out[bi:bi + 1,
                              c * CHUNK * 128:c * CHUNK * 128 + live],
                        out_sbs[bi][:, :live])

    nc.compile()
    return nc


# ----------------------------------------------------------------------------
# entry point
# ----------------------------------------------------------------------------

def kernel(x, edge_index, Wl1, bl1, Wr1, Wl2, bl2, Wr2,
           Wla, bla, Wra, Wa, ba, Wlm, blm, Wrm, Wm, bm):
    x = np.asarray(x, np.float32)
    pp = _preprocess(edge_index)
    old_of_new = pp["old_of_new"]

    # x gather table in block layout: per core 6250 rows + one -inf pad row
    xp = x[old_of_new]
    xtab = np.empty((TAB, F_IN), np.float32)
    for m in range(NCOR):
        base = m * BLOCK
        xtab[base:base + NLOC] = xp[m * NLOC:(m + 1) * NLOC]
        xtab[base + NLOC] = NEG
    xtab = xtab.astype(ml_dtypes.bfloat16)

    nc = _build_program(pp["sched"], pp["chunks"], pp["totslot"],
                        pp["totx"])

    def f32(a):
        return np.ascontiguousarray(np.asarray(a, np.float32))

    def b16(a):
        return np.ascontiguousarray(
            np.asarray(a, np.float32).astype(ml_dtypes.bfloat16))

    in_maps = []
    for m in range(NCOR):
        blk = xp[m * NLOC:(m + 1) * NLOC]
        xT = np.zeros((F_IN, PADN), np.float32)
        xT[:, :NLOC] = blk.T
        xell = xtab[pp["ell_abs"][m]]
        in_maps.append({
            "xell": xell, "xT": xT.astype(ml_dtypes.bfloat16),
            "idx": pp["idx_flat"][m],
            "Wl1": b16(Wl1), "Wr1": b16(Wr1),
            "Wl2": b16(Wl2), "Wr2": b16(Wr2),
            "Wla": b16(Wla), "Wra": b16(Wra),
            "Wlm": b16(Wlm), "Wrm": b16(Wrm),
            "bl1": f32(bl1).reshape(H, 1), "bl2": f32(bl2).reshape(H, 1),
            "bla": f32(bla).reshape(H, 1), "blm": f32(blm).reshape(H, 1),
            "Wa": b16(Wa).reshape(H, 1), "Wm": b16(Wm).reshape(H, 1),
            "ba": f32(ba).reshape(1, 1), "bm": f32(bm).reshape(1, 1),
        })

    res = run_bass_kernel_spmd(nc, in_maps, core_ids=list(range(NCOR)))

    rt = np.empty(N, np.float32)
    mv = np.empty(N, np.float32)
    for m in range(NCOR):
        out = res.results[m]["out"]
        rt[m * NLOC:(m + 1) * NLOC] = out[0]
        mv[m * NLOC:(m + 1) * NLOC] = out[1]
    rt_o = np.empty(N, np.float32)
    mv_o = np.empty(N, np.float32)
    rt_o[old_of_new] = rt
    mv_o[old_of_new] = mv

    _LAST.update(nc=nc, in_maps=in_maps, pp=pp)
    return (rt_o, mv_o)


# revision 9
# speedup vs baseline: 1536.8296x; 1.0043x over previous
"""GraphSAGE-max (3 layers + 2 heads) on 8 Trainium2 NeuronCores.

Strategy: data-parallel over dst-node partitions with replicated bf16 feature
tables in DRAM. The critical resource is GPSIMD descriptor generation for
dma_gather (~7.8ns per gathered row), so the schedule minimizes gathered
rows:

  - int16 gather indices can only span 32767 table rows, so the 50008-row
    table is covered by two OVERLAPPING views: view0 = rows [0, 32767),
    view1 = rows [17241, 50008). Sources in the 15526-row overlap can be
    fetched by either phase; high out-degree nodes are steered into the
    overlap (owner cores 3,4) so ~39% of edges are free-choice.
  - Per dst-node the free edges are split between the phases to equalize the
    per-tile max slot counts: K0[t]+K1[t] = max(max_deg, max_f0+max_f1),
    bringing ELL padding from 1.39x down to ~1.20x.

Layer 1's neighbor rows depend only on host inputs, so its ELL stream is
pre-gathered on the host (x[src] in call order) and the device reads it
with plain HWDGE DMAs — no Q7 descriptor generation at all. Layers 2/3
dma_gather from the AllGathered hidden-state tables with calls capped at
8 slot columns (1024 indices), which runs at ~7.8ns/row; 16-column calls
measure ~9.3ns/row.

Each layer, per 128-node tile: fetch neighbor rows -> tree-max on DVE ->
upcast f32 -> PE-transpose to feature-major -> bf16 matmuls (weights
stationary) -> relu -> PE-transpose back to node-major -> store to the
core's block -> AllGather blocks -> next layer's table.
"""

import numpy as np
import ml_dtypes

import concourse.bass as bass
import concourse.bacc as bacc
import concourse.mybir as mybir
import concourse.tile as tile
from concourse.masks import make_identity
from concourse.bass_utils import run_bass_kernel_spmd

N = 50000
E = 800000
F_IN = 128
H = 256
NCOR = 8
NLOC = N // NCOR             # 6250
BLOCK = NLOC + 1             # 6251 rows per core block (last = -inf pad)
TAB = NCOR * BLOCK           # 50008 table rows
VSIZE = 32767                # int16-addressable view size
V1_START = TAB - VSIZE       # 17241; overlap = [V1_START, VSIZE)
PAD0 = NLOC                  # view0 pad idx: B0's pad row (table row 6250)
PAD1 = VSIZE - 1             # view1 pad idx: B7's pad row (50007-17241)
TILES = (NLOC + 127) // 128  # 49
PADN = TILES * 128           # 6272
NEG = float(np.finfo(np.float32).min)
KCAP = 8                     # max gather columns per dma_gather call
CHUNK = 4                    # node tiles per matmul chunk (N free = 512)

_LAST = {}                   # stash for the test harness


# ----------------------------------------------------------------------------
# host-side graph preprocessing
# ----------------------------------------------------------------------------

def _wrap_idx(ilist):
    """ilist [NCOR, num] int -> dma_gather wrapped layout [NCOR, 128*ceil(num/16)]
    (16-partition wrap, replicated to 128 partitions)."""
    num = ilist.shape[1]
    cols = (num + 15) // 16
    w = np.zeros((NCOR, 16, cols), np.int16)
    i = np.arange(num)
    w[:, i % 16, i // 16] = ilist
    w = np.tile(w, (1, 8, 1))                 # [NCOR, 128, cols]
    return w.reshape(NCOR, 128 * cols)


def _preprocess(edge_index):
    src = np.asarray(edge_index[0], np.int64)
    dst = np.asarray(edge_index[1], np.int64)
    deg = np.bincount(dst, minlength=N)
    odeg = np.bincount(src, minlength=N)

    # node -> core: groups of 8 by in-degree desc (keeps per-core in-degree
    # profiles aligned); within each group the highest out-degree nodes go
    # to cores 3,4 whose blocks sit fully inside the view overlap.
    order = np.argsort(-deg, kind="stable")
    CORE_PREF = np.array([3, 4, 2, 5, 1, 6, 0, 7])
    owner = np.empty(N, np.int64)
    for g in range(N // 8):
        grp = order[g * 8:(g + 1) * 8]
        sub = grp[np.argsort(-odeg[grp], kind="stable")]
        owner[sub] = CORE_PREF

    def positions(keys):
        old_of_new = np.empty(N, np.int64)
        for m in range(NCOR):
            nodes = np.where(owner == m)[0]
            k = np.lexsort(tuple(reversed([kk[nodes] for kk in keys])))
            old_of_new[m * NLOC:(m + 1) * NLOC] = nodes[k]
        new_of_old = np.empty(N, np.int64)
        new_of_old[old_of_new] = np.arange(N)
        return old_of_new, new_of_old

    def table_pos(new_of_old):
        return (new_of_old // NLOC) * BLOCK + new_of_old % NLOC

    def forced_counts(tpos):
        sp = tpos[src]
        f0 = np.bincount(dst[sp < V1_START], minlength=N)
        f1 = np.bincount(dst[sp >= VSIZE], minlength=N)
        return f0, f1

    # pass 1: in-degree sort -> forced counts; pass 2: secondary sort by f0
    # clusters similar phase-0 loads into the same tile.
    _, new1 = positions([-deg])
    f0a, _ = forced_counts(table_pos(new1))
    old_of_new, new_of_old = positions([-deg, -f0a])
    tpos = table_pos(new_of_old)
    f0, f1 = forced_counts(tpos)

    # per-tile phase budgets (shared across cores)
    rank = new_of_old % NLOC
    tilei = rank // 128
    K0 = np.zeros(TILES, np.int64)
    K1 = np.zeros(TILES, np.int64)
    for t in range(TILES):
        sel = tilei == t
        mf0, mf1, mk = f0[sel].max(), f1[sel].max(), deg[sel].max()
        Ts = max(mk, mf0 + mf1)
        a = min(max((Ts + 1) // 2, mf0), Ts - mf1)
        K0[t], K1[t] = max(int(a), 1), max(int(Ts - a), 1)

    # per-dst phase-0 quota, then slot assignment
    k0cap = K0[tilei]
    k1cap = K1[tilei]
    freec = deg - f0 - f1
    dlo = np.maximum(np.minimum(f0 + freec, k0cap), deg - k1cap)

    nd = new_of_old[dst]
    sp = tpos[src]
    is_f0 = sp < V1_START
    is_f1 = sp >= VSIZE
    # order edges per dst: forced0 first, then free, then forced1; the first
    # dlo[d] edges go to phase 0.
    cls = np.where(is_f0, 0, np.where(is_f1, 2, 1)).astype(np.int64)
    ekey = nd * 4 + cls
    eorder = np.argsort(ekey, kind="stable")
    nd_s = nd[eorder]
    sp_s = sp[eorder]
    starts = np.searchsorted(nd_s, np.arange(N))
    slot_in_dst = np.arange(E) - starts[nd_s]
    ph_s = (slot_in_dst >= dlo[old_of_new[nd_s]]).astype(np.int64)
    # sanity: forced edges must land in their required phase
    # (guaranteed by construction: dlo >= f0, deg-dlo >= f1, and the
    #  forced0-free-forced1 ordering)
    idx0 = sp_s - 0
    idx1 = sp_s - V1_START
    loc = np.where(ph_s == 0, idx0, idx1)
    slot_in_ph = np.where(ph_s == 0, slot_in_dst,
                          slot_in_dst - dlo[old_of_new[nd_s]])

    # dense ELL per phase [NCOR, PADN, Kmax]
    def ell_of(phase, kcol, padidx):
        ell = np.full((NCOR, PADN, int(kcol.max())), padidx, np.int16)
        sel = ph_s == phase
        nde = nd_s[sel]
        ell[nde // NLOC, nde % NLOC, slot_in_ph[sel]] = loc[sel].astype(np.int16)
        return ell
    ells = {0: ell_of(0, K0, PAD0), 1: ell_of(1, K1, PAD1)}

    # gather call schedule + wrapped int16 index stream, grouped per chunk
    NCH = (TILES + CHUNK - 1) // CHUNK
    sched = []           # (tile, phase, kn, chunk, cidx0, xoff)
    chunks = []          # (flat_off, cols) per chunk
    blocks = []
    ell_abs = []         # absolute table rows per call [NCOR, kn*128]
    off = 0
    xoff = 0
    for c in range(NCH):
        cblocks = []
        ccols = 0
        for t in range(c * CHUNK, min((c + 1) * CHUNK, TILES)):
            for phase, ks in ((0, K0), (1, K1)):
                k0 = 0
                while k0 < int(ks[t]):
                    kn = min(KCAP, int(ks[t]) - k0)
                    blk = ells[phase][:, t * 128:(t + 1) * 128, k0:k0 + kn]
                    ilist = blk.transpose(0, 2, 1).reshape(NCOR, kn * 128)
                    base = 0 if phase == 0 else V1_START
                    ell_abs.append(ilist.astype(np.int64) + base)
                    w = _wrap_idx(ilist).reshape(NCOR, 128, 8 * kn)
                    cblocks.append(w)
                    sched.append((t, phase, kn, c, ccols, xoff))
                    ccols += 8 * kn
                    xoff += kn
                    k0 += kn
        cb = np.concatenate(cblocks, axis=2)      # [NCOR, 128, ccols]
        blocks.append(cb.reshape(NCOR, 128 * ccols))
        chunks.append((off, ccols))
        off += 128 * ccols
    idx_flat = np.concatenate(blocks, axis=1)
    ell_abs = np.concatenate(ell_abs, axis=1)     # [NCOR, totxslots*128]

    return dict(new_of_old=new_of_old, old_of_new=old_of_new,
                sched=sched, chunks=chunks, totslot=off, idx_flat=idx_flat,
                ell_abs=ell_abs, totx=xoff)


# ----------------------------------------------------------------------------
# device program
# ----------------------------------------------------------------------------

def _tree_max(nc, g, k, F):
    """In-place max over k column groups of width F; result in g[:, :F]."""
    while k > 1:
        if k % 2 == 1:
            nc.vector.tensor_tensor(out=g[:, 0:F], in0=g[:, 0:F],
                                    in1=g[:, (k - 1) * F:k * F],
                                    op=mybir.AluOpType.max)
            k -= 1
            if k == 1:
                break
        half = k // 2
        nc.vector.tensor_tensor(out=g[:, 0:half * F], in0=g[:, 0:half * F],
                                in1=g[:, half * F:2 * half * F],
                                op=mybir.AluOpType.max)
        k = half


def _build_program(sched, chunks, totslot, totx):
    nc = bacc.Bacc("TRN2", target_bir_lowering=False, debug=False,
                   num_devices=NCOR)
    f32, bf16, i16 = mybir.dt.float32, mybir.dt.bfloat16, mybir.dt.int16

    t_xell = nc.dram_tensor("xell", [totx * 128, F_IN], bf16,
                            kind="ExternalInput")
    t_xT = nc.dram_tensor("xT", [F_IN, PADN], bf16, kind="ExternalInput")
    t_idx = nc.dram_tensor("idx", [totslot], i16, kind="ExternalInput")
    wnames = ["Wl1", "Wr1", "Wl2", "Wr2", "Wla", "Wra", "Wlm", "Wrm"]
    wshapes = {"Wl1": (F_IN, H), "Wr1": (F_IN, H)}
    t_w = {w: nc.dram_tensor(w, list(wshapes.get(w, (H, H))), bf16,
                             kind="ExternalInput") for w in wnames}
    t_b = {b: nc.dram_tensor(b, [H, 1], f32, kind="ExternalInput")
           for b in ["bl1", "bl2", "bla", "blm"]}
    t_wh = {w: nc.dram_tensor(w, [H, 1], bf16, kind="ExternalInput")
            for w in ["Wa", "Wm"]}
    t_bh = {b: nc.dram_tensor(b, [1, 1], f32, kind="ExternalInput")
            for b in ["ba", "bm"]}
    t_out = nc.dram_tensor("out", [2, NLOC], f32, kind="ExternalOutput")

    NCH = (TILES + CHUNK - 1) // CHUNK
    cw_of = lambda c: min(CHUNK, TILES - c * CHUNK) * 128

    sched_of_tile = {}
    for (t, phase, kn, c, cidx0, xoff) in sched:
        sched_of_tile.setdefault(t, []).append((phase, kn, cidx0, xoff))
    CMAX = max(cols for (_, cols) in chunks)

    with tile.TileContext(nc) as tc:
        with tc.tile_pool(name="const", bufs=1) as cpool, \
             tc.tile_pool(name="hT", bufs=1) as hpool, \
             tc.tile_pool(name="work", bufs=2) as wk, \
             tc.tile_pool(name="psT", bufs=2, space="PSUM") as psT, \
             tc.tile_pool(name="psY", bufs=2, space="PSUM") as psY, \
             tc.tile_pool(name="dram", bufs=1, space="DRAM") as dram:

            ident = cpool.tile([128, 128], f32, name="ident")
            make_identity(nc, ident[:])
            ident16 = cpool.tile([128, 128], bf16, name="ident16")
            make_identity(nc, ident16[:])

            w_sb = {}
            for w in wnames:
                fi = wshapes.get(w, (H, H))[0]
                fh = fi // 128
                ws = cpool.tile([128, fh * H], bf16, name=f"sb_{w}")
                for h in range(fh):
                    nc.sync.dma_start(ws[:, h * H:(h + 1) * H],
                                      t_w[w][h * 128:(h + 1) * 128, :])
                w_sb[w] = ws
            b_sb = {}
            for b in t_b:
                bs = cpool.tile([128, 2], f32, name=f"sb_{b}")
                for h in range(2):
                    nc.sync.dma_start(bs[:, h:h + 1],
                                      t_b[b][h * 128:(h + 1) * 128, :])
                b_sb[b] = bs
            wh_sb = {}
            for w in t_wh:
                ws = cpool.tile([128, 2], bf16, name=f"sb_{w}")
                for h in range(2):
                    nc.sync.dma_start(ws[:, h:h + 1],
                                      t_wh[w][h * 128:(h + 1) * 128, :])
                wh_sb[w] = ws
            bh_sb = {}
            for b in t_bh:
                bs = cpool.tile([1, 1], f32, name=f"sb_{b}")
                nc.sync.dma_start(bs[:], t_bh[b][:])
                bh_sb[b] = bs

            xT_sb = hpool.tile([128, PADN], bf16, name="xT_sb")
            nc.sync.dma_start(xT_sb[:], t_xT[:])
            h1T = hpool.tile([128, 2 * PADN], bf16, name="h1T")
            h2T = hpool.tile([128, 2 * PADN], bf16, name="h2T")

            h1tab = dram.tile([TAB, H], bf16, name="h1tab",
                              addr_space="Shared")
            h2tab = dram.tile([TAB, H], bf16, name="h2tab",
                              addr_space="Shared")
            blk1 = dram.tile([BLOCK, H], bf16, name="blk1")
            blk2 = dram.tile([BLOCK, H], bf16, name="blk2")

            # each core's block ends with a -inf pad row
            padrow = cpool.tile([1, H], bf16, name="padrow")
            nc.vector.memset(padrow[:], NEG)
            nc.sync.dma_start(blk1[NLOC:NLOC + 1, :], padrow[:])
            nc.sync.dma_start(blk2[NLOC:NLOC + 1, :], padrow[:])

            def load_idx_chunk(c, tag):
                off, cols = chunks[c]
                idxc = wk.tile([128, CMAX], i16, name=f"idxc_{tag}",
                               tag="idxc", bufs=4)
                nc.sync.dma_start(
                    idxc[:, :cols],
                    t_idx[off:off + 128 * cols].rearrange("(p s) -> p s",
                                                          p=128))
                return idxc

            def aggregate_tile(t, table, F, tag, idxc):
                """two-phase gather + tree-max for node tile t -> bf16 [128, F].

                table=None: layer-1 mode, rows stream from the host
                pre-gathered t_xell instead of dma_gather."""
                agg16 = wk.tile([128, H], bf16, name=f"agg16_{tag}",
                                tag="agg16", bufs=4)
                first = True
                for (phase, kn, cidx0, xoff) in sched_of_tile[t]:
                    cols = 8 * kn
                    g = wk.tile([128, KCAP * H], bf16, name=f"g_{tag}",
                                tag="gather", bufs=6)
                    if table is None:
                        nc.sync.dma_start(
                            g[:, :kn * F].rearrange("p (k f) -> p k f", f=F),
                            t_xell[xoff * 128:(xoff + kn) * 128, :].rearrange(
                                "(k p) f -> p k f", p=128))
                    else:
                        view = table[0:VSIZE, :] if phase == 0 \
                            else table[V1_START:TAB, :]
                        nc.gpsimd.dma_gather(
                            out_ap=g[:, :kn * F].rearrange("p (k f) -> p k f",
                                                           f=F),
                            in_ap=view, idxs_ap=idxc[:, cidx0:cidx0 + cols],
                            num_idxs=128 * kn, num_idxs_reg=128 * kn,
                            elem_size=F, single_packet=False)
                    _tree_max(nc, g, kn, F)
                    if first:
                        nc.vector.tensor_copy(agg16[:, :F], g[:, :F])
                        first = False
                    else:
                        nc.vector.tensor_tensor(out=agg16[:, :F],
                                                in0=agg16[:, :F],
                                                in1=g[:, :F],
                                                op=mybir.AluOpType.max)
                return agg16

            def transpose_into(srcap, dst, col, tag):
                tp = psT.tile([128, 128], f32, name=f"tp_{tag}", tag="tpf")
                nc.tensor.transpose(tp[:], srcap, ident[:])
                nc.vector.tensor_copy(dst[:, col:col + 128], tp[:])

            def layer(table, selfT, F, Wl, Wr, bl, outT, blkout, tag):
                fh_in = F // 128
                for c in range(NCH):
                    cw = cw_of(c)
                    ntile = cw // 128
                    idxc = load_idx_chunk(c, f"{tag}_{c}") \
                        if table is not None else None
                    aggT = wk.tile([128, fh_in * 512], bf16,
                                   name=f"aggT_{tag}", tag="aggT")
                    for i in range(ntile):
                        t = c * CHUNK + i
                        agg16 = aggregate_tile(t, table, F, f"{tag}_{t}",
                                               idxc)
                        agg32 = wk.tile([128, H], f32, name=f"a32_{tag}",
                                        tag="agg32", bufs=4)
                        nc.any.tensor_copy(agg32[:, :F], agg16[:, :F])
                        for fh in range(fh_in):
                            transpose_into(agg32[:, fh * 128:(fh + 1) * 128],
                                           aggT, fh * 512 + i * 128,
                                           f"{tag}_{t}_{fh}")
                    for hh in range(2):
                        psy = psY.tile([128, 512], f32, name=f"psy_{tag}",
                                       tag="psy")
                        nmm = 2 * fh_in
                        i = 0
                        for fh in range(fh_in):
                            nc.tensor.matmul(
                                psy[:, :cw],
                                w_sb[Wl][:, fh * H + hh * 128:
                                         fh * H + (hh + 1) * 128],
                                aggT[:, fh * 512:fh * 512 + cw],
                                start=(i == 0), stop=(i == nmm - 1))
                            i += 1
                            nc.tensor.matmul(
                                psy[:, :cw],
                                w_sb[Wr][:, fh * H + hh * 128:
                                         fh * H + (hh + 1) * 128],
                                selfT[:, fh * PADN + c * CHUNK * 128:
                                      fh * PADN + c * CHUNK * 128 + cw],
                                start=(i == 0), stop=(i == nmm - 1))
                            i += 1
                        nc.scalar.activation(
                            outT[:, hh * PADN + c * CHUNK * 128:
                                 hh * PADN + c * CHUNK * 128 + cw],
                            psy[:, :cw],
                            mybir.ActivationFunctionType.Relu,
                            bias=b_sb[bl][:, hh:hh + 1])
                    for i in range(ntile):
                        t = c * CHUNK + i
                        ynode = wk.tile([128, H], bf16, name=f"yn_{tag}",
                                        tag="ynode", bufs=3)
                        for hh in range(2):
                            tp = psT.tile([128, 128], bf16,
                                          name=f"tpo_{tag}", tag="tp")
                            nc.tensor.transpose(
                                tp[:],
                                outT[:, hh * PADN + t * 128:
                                     hh * PADN + (t + 1) * 128],
                                ident16[:])
                            nc.vector.tensor_copy(
                                ynode[:, hh * 128:(hh + 1) * 128], tp[:])
                        rows = min(128, NLOC - t * 128)
                        nc.sync.dma_start(blkout[t * 128:t * 128 + rows, :],
                                          ynode[:rows, :])

            layer(None, xT_sb, F_IN, "Wl1", "Wr1", "bl1", h1T, blk1, "l1")
            nc.gpsimd.collective_compute(
                "AllGather", mybir.AluOpType.bypass,
                replica_groups=[list(range(NCOR))],
                ins=[blk1.opt()], outs=[h1tab.opt()])
            layer(h1tab, h1T, H, "Wl2", "Wr2", "bl2", h2T, blk2, "l2")
            nc.gpsimd.collective_compute(
                "AllGather", mybir.AluOpType.bypass,
                replica_groups=[list(range(NCOR))],
                ins=[blk2.opt()], outs=[h2tab.opt()])

            # layer 3: two branches + heads
            for c in range(NCH):
                cw = cw_of(c)
                ntile = cw // 128
                idxc = load_idx_chunk(c, f"l3_{c}")
                aggT = wk.tile([128, 2 * 512], bf16, name="aggT_l3",
                               tag="aggT")
                for i in range(ntile):
                    t = c * CHUNK + i
                    agg16 = aggregate_tile(t, h2tab, H, f"l3_{t}", idxc)
                    agg32 = wk.tile([128, H], f32, name="a32_l3",
                                    tag="agg32", bufs=4)
                    nc.any.tensor_copy(agg32[:, :H], agg16[:, :H])
                    for fh in range(2):
                        transpose_into(agg32[:, fh * 128:(fh + 1) * 128],
                                       aggT, fh * 512 + i * 128,
                                       f"l3_{t}_{fh}")
                out_sbs = [wk.tile([1, 512], f32, name=f"out_sb{bi}",
                                   tag=f"out_sb{bi}") for bi in range(2)]
                for bi, (Wl, Wr, bl, Wh, bh) in enumerate(
                        [("Wla", "Wra", "bla", "Wa", "ba"),
                         ("Wlm", "Wrm", "blm", "Wm", "bm")]):
                    brT = wk.tile([128, 2 * 512], bf16, name=f"brT{bi}",
                                  tag="brT")
                    for hh in range(2):
                        psy = psY.tile([128, 512], f32, name=f"psy3_{bi}",
                                       tag="psy")
                        for fh in range(2):
                            nc.tensor.matmul(
                                psy[:, :cw],
                                w_sb[Wl][:, fh * H + hh * 128:
                                         fh * H + (hh + 1) * 128],
                                aggT[:, fh * 512:fh * 512 + cw],
                                start=(fh == 0), stop=False)
                            nc.tensor.matmul(
                                psy[:, :cw],
                                w_sb[Wr][:, fh * H + hh * 128:
                                         fh * H + (hh + 1) * 128],
                                h2T[:, fh * PADN + c * CHUNK * 128:
                                    fh * PADN + c * CHUNK * 128 + cw],
                                start=False, stop=(fh == 1))
                        nc.scalar.activation(
                            brT[:, hh * 512:hh * 512 + cw], psy[:, :cw],
                            mybir.ActivationFunctionType.Relu,
                            bias=b_sb[bl][:, hh:hh + 1])
                    psh = psY.tile([1, 512], f32, name=f"psh{bi}", tag="psh")
                    for hh in range(2):
                        nc.tensor.matmul(psh[:, :cw],
                                         wh_sb[Wh][:, hh:hh + 1],
                                         brT[:, hh * 512:hh * 512 + cw],
                                         start=(hh == 0), stop=(hh == 1))
                    nc.scalar.activation(out_sbs[bi][:, :cw],
                                         psh[:, :cw],
                                         mybir.ActivationFunctionType.Identity,
                                         bias=bh_sb[bh][:])
                live = min(cw, NLOC - c * CHUNK * 128)
                for bi in range(2):
                    nc.sync.dma_start(
                        t_out[bi:bi + 1,
                              c * CHUNK * 128:c * CHUNK * 128 + live],
                        out_sbs[bi][:, :live])

    nc.compile()
    return nc


# ----------------------------------------------------------------------------
# entry point
# ----------------------------------------------------------------------------

def kernel(x, edge_index, Wl1, bl1, Wr1, Wl2, bl2, Wr2,
           Wla, bla, Wra, Wa, ba, Wlm, blm, Wrm, Wm, bm):
    x = np.asarray(x, np.float32)
    pp = _preprocess(edge_index)
    old_of_new = pp["old_of_new"]

    # x gather table in block layout: per core 6250 rows + one -inf pad row
    xp = x[old_of_new]
    xtab = np.empty((TAB, F_IN), np.float32)
    for m in range(NCOR):
        base = m * BLOCK
        xtab[base:base + NLOC] = xp[m * NLOC:(m + 1) * NLOC]
        xtab[base + NLOC] = NEG
    xtab = xtab.astype(ml_dtypes.bfloat16)

    nc = _build_program(pp["sched"], pp["chunks"], pp["totslot"],
                        pp["totx"])

    def f32(a):
        return np.ascontiguousarray(np.asarray(a, np.float32))

    def b16(a):
        return np.ascontiguousarray(
            np.asarray(a, np.float32).astype(ml_dtypes.bfloat16))

    in_maps = []
    for m in range(NCOR):
        blk = xp[m * NLOC:(m + 1) * NLOC]
        xT = np.zeros((F_IN, PADN), np.float32)
        xT[:, :NLOC] = blk.T
        xell = xtab[pp["ell_abs"][m]]
        in_maps.append({
            "xell": xell, "xT": xT.astype(ml_dtypes.bfloat16),
            "idx": pp["idx_flat"][m],
            "Wl1": b16(Wl1), "Wr1": b16(Wr1),
            "Wl2": b16(Wl2), "Wr2": b16(Wr2),
            "Wla": b16(Wla), "Wra": b16(Wra),
            "Wlm": b16(Wlm), "Wrm": b16(Wrm),
            "bl1": f32(bl1).reshape(H, 1), "bl2": f32(bl2).reshape(H, 1),
            "bla": f32(bla).reshape(H, 1), "blm": f32(blm).reshape(H, 1),
            "Wa": b16(Wa).reshape(H, 1), "Wm": b16(Wm).reshape(H, 1),
            "ba": f32(ba).reshape(1, 1), "bm": f32(bm).reshape(1, 1),
        })

    res = run_bass_kernel_spmd(nc, in_maps, core_ids=list(range(NCOR)))

    rt = np.empty(N, np.float32)
    mv = np.empty(N, np.float32)
    for m in range(NCOR):
        out = res.results[m]["out"]
        rt[m * NLOC:(m + 1) * NLOC] = out[0]
        mv[m * NLOC:(m + 1) * NLOC] = out[1]
    rt_o = np.empty(N, np.float32)
    mv_o = np.empty(N, np.float32)
    rt_o[old_of_new] = rt
    mv_o[old_of_new] = mv

    _LAST.update(nc=nc, in_maps=in_maps, pp=pp)
    return (rt_o, mv_o)


# revision 10
# speedup vs baseline: 1565.6200x; 1.0187x over previous
"""GraphSAGE-max (3 layers + 2 heads) on 8 Trainium2 NeuronCores.

Strategy: data-parallel over dst-node partitions with replicated bf16 feature
tables in DRAM. The critical resource is GPSIMD descriptor generation for
dma_gather (~7.8ns per gathered row), so the schedule minimizes gathered
rows:

  - int16 gather indices can only span 32767 table rows, so the 50008-row
    table is covered by two OVERLAPPING views: view0 = rows [0, 32767),
    view1 = rows [17241, 50008). Sources in the 15526-row overlap can be
    fetched by either phase; high out-degree nodes are steered into the
    overlap (owner cores 3,4) so ~39% of edges are free-choice.
  - Per dst-node the free edges are split between the phases to equalize the
    per-tile max slot counts: K0[t]+K1[t] = max(max_deg, max_f0+max_f1),
    bringing ELL padding from 1.39x down to ~1.20x.

Layer 1's neighbor rows depend only on host inputs, so its ELL stream is
pre-gathered on the host (x[src] in call order) and the device reads it
with plain HWDGE DMAs — no Q7 descriptor generation at all. Layers 2/3
dma_gather from the AllGathered hidden-state tables with calls capped at
8 slot columns (1024 indices), which runs at ~7.8ns/row; 16-column calls
measure ~9.3ns/row.

Each layer, per 128-node tile: fetch neighbor rows -> tree-max on DVE ->
upcast f32 -> PE-transpose to feature-major -> bf16 matmuls (weights
stationary) -> relu -> PE-transpose back to node-major -> store to the
core's block -> AllGather blocks -> next layer's table.
"""

import numpy as np
import ml_dtypes

import concourse.bass as bass
import concourse.bacc as bacc
import concourse.mybir as mybir
import concourse.tile as tile
from concourse.masks import make_identity
from concourse.bass_utils import run_bass_kernel_spmd

N = 50000
E = 800000
F_IN = 128
H = 256
NCOR = 8
NLOC = N // NCOR             # 6250
BLOCK = NLOC + 1             # 6251 rows per core block (last = -inf pad)
TAB = NCOR * BLOCK           # 50008 table rows
VSIZE = 32767                # int16-addressable view size
V1_START = TAB - VSIZE       # 17241; overlap = [V1_START, VSIZE)
PAD0 = NLOC                  # view0 pad idx: B0's pad row (table row 6250)
PAD1 = VSIZE - 1             # view1 pad idx: B7's pad row (50007-17241)
TILES = (NLOC + 127) // 128  # 49
PADN = TILES * 128           # 6272
NEG = float(np.finfo(np.float32).min)
KCAP = 8                     # max gather columns per dma_gather call
CHUNK = 4                    # node tiles per matmul chunk (N free = 512)

_LAST = {}                   # stash for the test harness


# ----------------------------------------------------------------------------
# host-side graph preprocessing
# ----------------------------------------------------------------------------

def _wrap_idx(ilist):
    """ilist [NCOR, num] int -> dma_gather wrapped layout [NCOR, 128*ceil(num/16)]
    (16-partition wrap, replicated to 128 partitions)."""
    num = ilist.shape[1]
    cols = (num + 15) // 16
    w = np.zeros((NCOR, 16, cols), np.int16)
    i = np.arange(num)
    w[:, i % 16, i // 16] = ilist
    w = np.tile(w, (1, 8, 1))                 # [NCOR, 128, cols]
    return w.reshape(NCOR, 128 * cols)


def _preprocess(edge_index):
    src = np.asarray(edge_index[0], np.int64)
    dst = np.asarray(edge_index[1], np.int64)
    deg = np.bincount(dst, minlength=N)
    odeg = np.bincount(src, minlength=N)

    # node -> core: groups of 8 by in-degree desc (keeps per-core in-degree
    # profiles aligned); within each group the highest out-degree nodes go
    # to cores 3,4 whose blocks sit fully inside the view overlap.
    order = np.argsort(-deg, kind="stable")
    CORE_PREF = np.array([3, 4, 2, 5, 1, 6, 0, 7])
    owner = np.empty(N, np.int64)
    for g in range(N // 8):
        grp = order[g * 8:(g + 1) * 8]
        sub = grp[np.argsort(-odeg[grp], kind="stable")]
        owner[sub] = CORE_PREF

    def positions(keys):
        old_of_new = np.empty(N, np.int64)
        for m in range(NCOR):
            nodes = np.where(owner == m)[0]
            k = np.lexsort(tuple(reversed([kk[nodes] for kk in keys])))
            old_of_new[m * NLOC:(m + 1) * NLOC] = nodes[k]
        new_of_old = np.empty(N, np.int64)
        new_of_old[old_of_new] = np.arange(N)
        return old_of_new, new_of_old

    def table_pos(new_of_old):
        return (new_of_old // NLOC) * BLOCK + new_of_old % NLOC

    def forced_counts(tpos):
        sp = tpos[src]
        f0 = np.bincount(dst[sp < V1_START], minlength=N)
        f1 = np.bincount(dst[sp >= VSIZE], minlength=N)
        return f0, f1

    # pass 1: in-degree sort -> forced counts; pass 2: secondary sort by f0
    # clusters similar phase-0 loads into the same tile.
    _, new1 = positions([-deg])
    f0a, _ = forced_counts(table_pos(new1))
    old_of_new, new_of_old = positions([-deg, -f0a])
    tpos = table_pos(new_of_old)
    f0, f1 = forced_counts(tpos)

    # per-tile phase budgets (shared across cores)
    rank = new_of_old % NLOC
    tilei = rank // 128
    K0 = np.zeros(TILES, np.int64)
    K1 = np.zeros(TILES, np.int64)
    for t in range(TILES):
        sel = tilei == t
        mf0, mf1, mk = f0[sel].max(), f1[sel].max(), deg[sel].max()
        Ts = max(mk, mf0 + mf1)
        a = min(max((Ts + 1) // 2, mf0), Ts - mf1)
        K0[t], K1[t] = max(int(a), 1), max(int(Ts - a), 1)

    # per-dst phase-0 quota, then slot assignment
    k0cap = K0[tilei]
    k1cap = K1[tilei]
    freec = deg - f0 - f1
    dlo = np.maximum(np.minimum(f0 + freec, k0cap), deg - k1cap)

    nd = new_of_old[dst]
    sp = tpos[src]
    is_f0 = sp < V1_START
    is_f1 = sp >= VSIZE
    # order edges per dst: forced0 first, then free, then forced1; the first
    # dlo[d] edges go to phase 0.
    cls = np.where(is_f0, 0, np.where(is_f1, 2, 1)).astype(np.int64)
    ekey = nd * 4 + cls
    eorder = np.argsort(ekey, kind="stable")
    nd_s = nd[eorder]
    sp_s = sp[eorder]
    starts = np.searchsorted(nd_s, np.arange(N))
    slot_in_dst = np.arange(E) - starts[nd_s]
    ph_s = (slot_in_dst >= dlo[old_of_new[nd_s]]).astype(np.int64)
    # sanity: forced edges must land in their required phase
    # (guaranteed by construction: dlo >= f0, deg-dlo >= f1, and the
    #  forced0-free-forced1 ordering)
    idx0 = sp_s - 0
    idx1 = sp_s - V1_START
    loc = np.where(ph_s == 0, idx0, idx1)
    slot_in_ph = np.where(ph_s == 0, slot_in_dst,
                          slot_in_dst - dlo[old_of_new[nd_s]])

    # dense ELL per phase [NCOR, PADN, Kmax]
    def ell_of(phase, kcol, padidx):
        ell = np.full((NCOR, PADN, int(kcol.max())), padidx, np.int16)
        sel = ph_s == phase
        nde = nd_s[sel]
        ell[nde // NLOC, nde % NLOC, slot_in_ph[sel]] = loc[sel].astype(np.int16)
        return ell
    ells = {0: ell_of(0, K0, PAD0), 1: ell_of(1, K1, PAD1)}

    # gather call schedule + wrapped int16 index stream, grouped per chunk
    NCH = (TILES + CHUNK - 1) // CHUNK
    sched = []           # (tile, phase, kn, chunk, cidx0, xoff)
    chunks = []          # (flat_off, cols) per chunk
    blocks = []
    ell_abs = []         # absolute table rows per call [NCOR, kn*128]
    off = 0
    xoff = 0
    for c in range(NCH):
        cblocks = []
        ccols = 0
        for t in range(c * CHUNK, min((c + 1) * CHUNK, TILES)):
            for phase, ks in ((0, K0), (1, K1)):
                k0 = 0
                while k0 < int(ks[t]):
                    kn = min(KCAP, int(ks[t]) - k0)
                    blk = ells[phase][:, t * 128:(t + 1) * 128, k0:k0 + kn]
                    ilist = blk.transpose(0, 2, 1).reshape(NCOR, kn * 128)
                    base = 0 if phase == 0 else V1_START
                    ell_abs.append(ilist.astype(np.int64) + base)
                    w = _wrap_idx(ilist).reshape(NCOR, 128, 8 * kn)
                    cblocks.append(w)
                    sched.append((t, phase, kn, c, ccols, xoff))
                    ccols += 8 * kn
                    xoff += kn
                    k0 += kn
        cb = np.concatenate(cblocks, axis=2)      # [NCOR, 128, ccols]
        blocks.append(cb.reshape(NCOR, 128 * ccols))
        chunks.append((off, ccols))
        off += 128 * ccols
    idx_flat = np.concatenate(blocks, axis=1)
    ell_abs = np.concatenate(ell_abs, axis=1)     # [NCOR, totxslots*128]

    return dict(new_of_old=new_of_old, old_of_new=old_of_new,
                sched=sched, chunks=chunks, totslot=off, idx_flat=idx_flat,
                ell_abs=ell_abs, totx=xoff)


# ----------------------------------------------------------------------------
# device program
# ----------------------------------------------------------------------------

def _tree_max(nc, g, k, F):
    """In-place max over k column groups of width F; result in g[:, :F]."""
    while k > 1:
        if k % 2 == 1:
            nc.vector.tensor_tensor(out=g[:, 0:F], in0=g[:, 0:F],
                                    in1=g[:, (k - 1) * F:k * F],
                                    op=mybir.AluOpType.max)
            k -= 1
            if k == 1:
                break
        half = k // 2
        nc.vector.tensor_tensor(out=g[:, 0:half * F], in0=g[:, 0:half * F],
                                in1=g[:, half * F:2 * half * F],
                                op=mybir.AluOpType.max)
        k = half


def _build_program(sched, chunks, totslot, totx):
    nc = bacc.Bacc("TRN2", target_bir_lowering=False, debug=False,
                   num_devices=NCOR)
    f32, bf16, i16 = mybir.dt.float32, mybir.dt.bfloat16, mybir.dt.int16

    t_xell = nc.dram_tensor("xell", [totx * 128, F_IN], bf16,
                            kind="ExternalInput")
    t_xT = nc.dram_tensor("xT", [F_IN, PADN], bf16, kind="ExternalInput")
    t_idx = nc.dram_tensor("idx", [totslot], i16, kind="ExternalInput")
    wnames = ["Wl1", "Wr1", "Wl2", "Wr2", "Wla", "Wra", "Wlm", "Wrm"]
    wshapes = {"Wl1": (F_IN, H), "Wr1": (F_IN, H)}
    t_w = {w: nc.dram_tensor(w, list(wshapes.get(w, (H, H))), bf16,
                             kind="ExternalInput") for w in wnames}
    t_b = {b: nc.dram_tensor(b, [H, 1], f32, kind="ExternalInput")
           for b in ["bl1", "bl2", "bla", "blm"]}
    t_wh = {w: nc.dram_tensor(w, [H, 1], bf16, kind="ExternalInput")
            for w in ["Wa", "Wm"]}
    t_bh = {b: nc.dram_tensor(b, [1, 1], f32, kind="ExternalInput")
            for b in ["ba", "bm"]}
    t_out = nc.dram_tensor("out", [2, NLOC], f32, kind="ExternalOutput")

    NCH = (TILES + CHUNK - 1) // CHUNK
    cw_of = lambda c: min(CHUNK, TILES - c * CHUNK) * 128

    sched_of_tile = {}
    for (t, phase, kn, c, cidx0, xoff) in sched:
        sched_of_tile.setdefault(t, []).append((phase, kn, cidx0, xoff))
    # L1 merged loads: calls of a tile are contiguous in xell
    l1_of_tile = {}
    for t, calls in sched_of_tile.items():
        x0 = calls[0][3]
        ktot = sum(kn for (_, kn, _, _) in calls)
        l1_of_tile[t] = (x0, ktot)
    KTOTMAX = max(k for (_, k) in l1_of_tile.values())
    CMAX = max(cols for (_, cols) in chunks)

    with tile.TileContext(nc) as tc:
        with tc.tile_pool(name="const", bufs=1) as cpool, \
             tc.tile_pool(name="hT", bufs=1) as hpool, \
             tc.tile_pool(name="work", bufs=2) as wk, \
             tc.tile_pool(name="psT", bufs=2, space="PSUM") as psT, \
             tc.tile_pool(name="psY", bufs=2, space="PSUM") as psY, \
             tc.tile_pool(name="dram", bufs=1, space="DRAM") as dram:

            ident = cpool.tile([128, 128], f32, name="ident")
            make_identity(nc, ident[:])
            ident16 = cpool.tile([128, 128], bf16, name="ident16")
            make_identity(nc, ident16[:])

            w_sb = {}
            for w in wnames:
                fi = wshapes.get(w, (H, H))[0]
                fh = fi // 128
                ws = cpool.tile([128, fh * H], bf16, name=f"sb_{w}")
                for h in range(fh):
                    nc.sync.dma_start(ws[:, h * H:(h + 1) * H],
                                      t_w[w][h * 128:(h + 1) * 128, :])
                w_sb[w] = ws
            b_sb = {}
            for b in t_b:
                bs = cpool.tile([128, 2], f32, name=f"sb_{b}")
                for h in range(2):
                    nc.sync.dma_start(bs[:, h:h + 1],
                                      t_b[b][h * 128:(h + 1) * 128, :])
                b_sb[b] = bs
            wh_sb = {}
            for w in t_wh:
                ws = cpool.tile([128, 2], bf16, name=f"sb_{w}")
                for h in range(2):
                    nc.sync.dma_start(ws[:, h:h + 1],
                                      t_wh[w][h * 128:(h + 1) * 128, :])
                wh_sb[w] = ws
            bh_sb = {}
            for b in t_bh:
                bs = cpool.tile([1, 1], f32, name=f"sb_{b}")
                nc.sync.dma_start(bs[:], t_bh[b][:])
                bh_sb[b] = bs

            xT_sb = hpool.tile([128, PADN], bf16, name="xT_sb")
            nc.sync.dma_start(xT_sb[:], t_xT[:])
            h1T = hpool.tile([128, 2 * PADN], bf16, name="h1T")
            h2T = hpool.tile([128, 2 * PADN], bf16, name="h2T")

            h1tab = dram.tile([TAB, H], bf16, name="h1tab",
                              addr_space="Shared")
            h2tab = dram.tile([TAB, H], bf16, name="h2tab",
                              addr_space="Shared")
            blk1 = dram.tile([BLOCK, H], bf16, name="blk1")
            blk2 = dram.tile([BLOCK, H], bf16, name="blk2")

            # each core's block ends with a -inf pad row
            padrow = cpool.tile([1, H], bf16, name="padrow")
            nc.vector.memset(padrow[:], NEG)
            nc.sync.dma_start(blk1[NLOC:NLOC + 1, :], padrow[:])
            nc.sync.dma_start(blk2[NLOC:NLOC + 1, :], padrow[:])

            def load_idx_chunk(c, tag):
                off, cols = chunks[c]
                idxc = wk.tile([128, CMAX], i16, name=f"idxc_{tag}",
                               tag="idxc", bufs=4)
                nc.sync.dma_start(
                    idxc[:, :cols],
                    t_idx[off:off + 128 * cols].rearrange("(p s) -> p s",
                                                          p=128))
                return idxc

            def aggregate_tile(t, table, F, tag, idxc):
                """two-phase gather + tree-max for node tile t -> bf16 [128, F].

                table=None: layer-1 mode, rows stream from the host
                pre-gathered t_xell instead of dma_gather."""
                if table is None:
                    x0, ktot = l1_of_tile[t]
                    g1 = wk.tile([128, KTOTMAX * F_IN], bf16,
                                 name=f"g1_{tag}", tag="gatherL1", bufs=3)
                    nc.scalar.dma_start(
                        g1[:, :ktot * F].rearrange("p (k f) -> p k f", f=F),
                        t_xell[x0 * 128:(x0 + ktot) * 128, :].rearrange(
                            "(k p) f -> p k f", p=128))
                    _tree_max(nc, g1, ktot, F)
                    return g1
                agg16 = wk.tile([128, H], bf16, name=f"agg16_{tag}",
                                tag="agg16", bufs=4)
                first = True
                for (phase, kn, cidx0, xoff) in sched_of_tile[t]:
                    cols = 8 * kn
                    g = wk.tile([128, KCAP * H], bf16, name=f"g_{tag}",
                                tag="gather", bufs=6)
                    view = table[0:VSIZE, :] if phase == 0 \
                        else table[V1_START:TAB, :]
                    nc.gpsimd.dma_gather(
                        out_ap=g[:, :kn * F].rearrange("p (k f) -> p k f",
                                                       f=F),
                        in_ap=view, idxs_ap=idxc[:, cidx0:cidx0 + cols],
                        num_idxs=128 * kn, num_idxs_reg=128 * kn,
                        elem_size=F, single_packet=False)
                    _tree_max(nc, g, kn, F)
                    if first:
                        nc.vector.tensor_copy(agg16[:, :F], g[:, :F])
                        first = False
                    else:
                        nc.vector.tensor_tensor(out=agg16[:, :F],
                                                in0=agg16[:, :F],
                                                in1=g[:, :F],
                                                op=mybir.AluOpType.max)
                return agg16

            def transpose_into(srcap, dst, col, tag):
                tp = psT.tile([128, 128], f32, name=f"tp_{tag}", tag="tpf")
                nc.tensor.transpose(tp[:], srcap, ident[:])
                nc.vector.tensor_copy(dst[:, col:col + 128], tp[:])

            def layer(table, selfT, F, Wl, Wr, bl, outT, blkout, tag):
                fh_in = F // 128
                for c in range(NCH):
                    cw = cw_of(c)
                    ntile = cw // 128
                    idxc = load_idx_chunk(c, f"{tag}_{c}") \
                        if table is not None else None
                    aggT = wk.tile([128, fh_in * 512], bf16,
                                   name=f"aggT_{tag}", tag="aggT")
                    for i in range(ntile):
                        t = c * CHUNK + i
                        agg16 = aggregate_tile(t, table, F, f"{tag}_{t}",
                                               idxc)
                        agg32 = wk.tile([128, H], f32, name=f"a32_{tag}",
                                        tag="agg32", bufs=4)
                        nc.any.tensor_copy(agg32[:, :F], agg16[:, :F])
                        for fh in range(fh_in):
                            transpose_into(agg32[:, fh * 128:(fh + 1) * 128],
                                           aggT, fh * 512 + i * 128,
                                           f"{tag}_{t}_{fh}")
                    for hh in range(2):
                        psy = psY.tile([128, 512], f32, name=f"psy_{tag}",
                                       tag="psy")
                        nmm = 2 * fh_in
                        i = 0
                        for fh in range(fh_in):
                            nc.tensor.matmul(
                                psy[:, :cw],
                                w_sb[Wl][:, fh * H + hh * 128:
                                         fh * H + (hh + 1) * 128],
                                aggT[:, fh * 512:fh * 512 + cw],
                                start=(i == 0), stop=(i == nmm - 1))
                            i += 1
                            nc.tensor.matmul(
                                psy[:, :cw],
                                w_sb[Wr][:, fh * H + hh * 128:
                                         fh * H + (hh + 1) * 128],
                                selfT[:, fh * PADN + c * CHUNK * 128:
                                      fh * PADN + c * CHUNK * 128 + cw],
                                start=(i == 0), stop=(i == nmm - 1))
                            i += 1
                        nc.scalar.activation(
                            outT[:, hh * PADN + c * CHUNK * 128:
                                 hh * PADN + c * CHUNK * 128 + cw],
                            psy[:, :cw],
                            mybir.ActivationFunctionType.Relu,
                            bias=b_sb[bl][:, hh:hh + 1])
                    for i in range(ntile):
                        t = c * CHUNK + i
                        ynode = wk.tile([128, H], bf16, name=f"yn_{tag}",
                                        tag="ynode", bufs=3)
                        for hh in range(2):
                            tp = psT.tile([128, 128], bf16,
                                          name=f"tpo_{tag}", tag="tp")
                            nc.tensor.transpose(
                                tp[:],
                                outT[:, hh * PADN + t * 128:
                                     hh * PADN + (t + 1) * 128],
                                ident16[:])
                            nc.vector.tensor_copy(
                                ynode[:, hh * 128:(hh + 1) * 128], tp[:])
                        rows = min(128, NLOC - t * 128)
                        nc.sync.dma_start(blkout[t * 128:t * 128 + rows, :],
                                          ynode[:rows, :])

            layer(None, xT_sb, F_IN, "Wl1", "Wr1", "bl1", h1T, blk1, "l1")
            nc.gpsimd.collective_compute(
                "AllGather", mybir.AluOpType.bypass,
                replica_groups=[list(range(NCOR))],
                ins=[blk1.opt()], outs=[h1tab.opt()])
            layer(h1tab, h1T, H, "Wl2", "Wr2", "bl2", h2T, blk2, "l2")
            nc.gpsimd.collective_compute(
                "AllGather", mybir.AluOpType.bypass,
                replica_groups=[list(range(NCOR))],
                ins=[blk2.opt()], outs=[h2tab.opt()])

            # layer 3: two branches + heads
            for c in range(NCH):
                cw = cw_of(c)
                ntile = cw // 128
                idxc = load_idx_chunk(c, f"l3_{c}")
                aggT = wk.tile([128, 2 * 512], bf16, name="aggT_l3",
                               tag="aggT")
                for i in range(ntile):
                    t = c * CHUNK + i
                    agg16 = aggregate_tile(t, h2tab, H, f"l3_{t}", idxc)
                    agg32 = wk.tile([128, H], f32, name="a32_l3",
                                    tag="agg32", bufs=4)
                    nc.any.tensor_copy(agg32[:, :H], agg16[:, :H])
                    for fh in range(2):
                        transpose_into(agg32[:, fh * 128:(fh + 1) * 128],
                                       aggT, fh * 512 + i * 128,
                                       f"l3_{t}_{fh}")
                out_sbs = [wk.tile([1, 512], f32, name=f"out_sb{bi}",
                                   tag=f"out_sb{bi}") for bi in range(2)]
                for bi, (Wl, Wr, bl, Wh, bh) in enumerate(
                        [("Wla", "Wra", "bla", "Wa", "ba"),
                         ("Wlm", "Wrm", "blm", "Wm", "bm")]):
                    brT = wk.tile([128, 2 * 512], bf16, name=f"brT{bi}",
                                  tag="brT")
                    for hh in range(2):
                        psy = psY.tile([128, 512], f32, name=f"psy3_{bi}",
                                       tag="psy")
                        for fh in range(2):
                            nc.tensor.matmul(
                                psy[:, :cw],
                                w_sb[Wl][:, fh * H + hh * 128:
                                         fh * H + (hh + 1) * 128],
                                aggT[:, fh * 512:fh * 512 + cw],
                                start=(fh == 0), stop=False)
                            nc.tensor.matmul(
                                psy[:, :cw],
                                w_sb[Wr][:, fh * H + hh * 128:
                                         fh * H + (hh + 1) * 128],
                                h2T[:, fh * PADN + c * CHUNK * 128:
                                    fh * PADN + c * CHUNK * 128 + cw],
                                start=False, stop=(fh == 1))
                        nc.scalar.activation(
                            brT[:, hh * 512:hh * 512 + cw], psy[:, :cw],
                            mybir.ActivationFunctionType.Relu,
                            bias=b_sb[bl][:, hh:hh + 1])
                    psh = psY.tile([1, 512], f32, name=f"psh{bi}", tag="psh")
                    for hh in range(2):
                        nc.tensor.matmul(psh[:, :cw],
                                         wh_sb[Wh][:, hh:hh + 1],
                                         brT[:, hh * 512:hh * 512 + cw],
                                         start=(hh == 0), stop=(hh == 1))
                    nc.scalar.activation(out_sbs[bi][:, :cw],
                                         psh[:, :cw],
                                         mybir.ActivationFunctionType.Identity,
                                         bias=bh_sb[bh][:])
                live = min(cw, NLOC - c * CHUNK * 128)
                for bi in range(2):
                    nc.sync.dma_start(
                        t_out[bi:bi + 1,
                              c * CHUNK * 128:c * CHUNK * 128 + live],
                        out_sbs[bi][:, :live])

    nc.compile()
    return nc


# ----------------------------------------------------------------------------
# entry point
# ----------------------------------------------------------------------------

def kernel(x, edge_index, Wl1, bl1, Wr1, Wl2, bl2, Wr2,
           Wla, bla, Wra, Wa, ba, Wlm, blm, Wrm, Wm, bm):
    x = np.asarray(x, np.float32)
    pp = _preprocess(edge_index)
    old_of_new = pp["old_of_new"]

    # x gather table in block layout: per core 6250 rows + one -inf pad row
    xp = x[old_of_new]
    xtab = np.empty((TAB, F_IN), np.float32)
    for m in range(NCOR):
        base = m * BLOCK
        xtab[base:base + NLOC] = xp[m * NLOC:(m + 1) * NLOC]
        xtab[base + NLOC] = NEG
    xtab = xtab.astype(ml_dtypes.bfloat16)

    nc = _build_program(pp["sched"], pp["chunks"], pp["totslot"],
                        pp["totx"])

    def f32(a):
        return np.ascontiguousarray(np.asarray(a, np.float32))

    def b16(a):
        return np.ascontiguousarray(
            np.asarray(a, np.float32).astype(ml_dtypes.bfloat16))

    in_maps = []
    for m in range(NCOR):
        blk = xp[m * NLOC:(m + 1) * NLOC]
        xT = np.zeros((F_IN, PADN), np.float32)
        xT[:, :NLOC] = blk.T
        xell = xtab[pp["ell_abs"][m]]
        in_maps.append({
            "xell": xell, "xT": xT.astype(ml_dtypes.bfloat16),
            "idx": pp["idx_flat"][m],
            "Wl1": b16(Wl1), "Wr1": b16(Wr1),
            "Wl2": b16(Wl2), "Wr2": b16(Wr2),
            "Wla": b16(Wla), "Wra": b16(Wra),
            "Wlm": b16(Wlm), "Wrm": b16(Wrm),
            "bl1": f32(bl1).reshape(H, 1), "bl2": f32(bl2).reshape(H, 1),
            "bla": f32(bla).reshape(H, 1), "blm": f32(blm).reshape(H, 1),
            "Wa": b16(Wa).reshape(H, 1), "Wm": b16(Wm).reshape(H, 1),
            "ba": f32(ba).reshape(1, 1), "bm": f32(bm).reshape(1, 1),
        })

    res = run_bass_kernel_spmd(nc, in_maps, core_ids=list(range(NCOR)))

    rt = np.empty(N, np.float32)
    mv = np.empty(N, np.float32)
    for m in range(NCOR):
        out = res.results[m]["out"]
        rt[m * NLOC:(m + 1) * NLOC] = out[0]
        mv[m * NLOC:(m + 1) * NLOC] = out[1]
    rt_o = np.empty(N, np.float32)
    mv_o = np.empty(N, np.float32)
    rt_o[old_of_new] = rt
    mv_o[old_of_new] = mv

    _LAST.update(nc=nc, in_maps=in_maps, pp=pp)
    return (rt_o, mv_o)


# revision 11
# speedup vs baseline: 1590.9354x; 1.0162x over previous
"""GraphSAGE-max (3 layers + 2 heads) on 8 Trainium2 NeuronCores.

Strategy: data-parallel over dst-node partitions with replicated bf16 feature
tables in DRAM. The critical resource is GPSIMD descriptor generation for
dma_gather (~7.8ns per gathered row), so the schedule minimizes gathered
rows:

  - int16 gather indices can only span 32767 table rows, so the 50008-row
    table is covered by two OVERLAPPING views: view0 = rows [0, 32767),
    view1 = rows [17241, 50008). Sources in the 15526-row overlap can be
    fetched by either phase; high out-degree nodes are steered into the
    overlap (owner cores 3,4) so ~39% of edges are free-choice.
  - Per dst-node the free edges are split between the phases to equalize the
    per-tile max slot counts: K0[t]+K1[t] = max(max_deg, max_f0+max_f1),
    bringing ELL padding from 1.39x down to ~1.20x.

Layer 1's neighbor rows depend only on host inputs, so its ELL stream is
pre-gathered on the host (x[src] in call order) and the device reads it
with plain HWDGE DMAs — no Q7 descriptor generation at all. Layers 2/3
dma_gather from the AllGathered hidden-state tables with calls capped at
8 slot columns (1024 indices), which runs at ~7.8ns/row; 16-column calls
measure ~9.3ns/row.

Each layer, per 128-node tile: fetch neighbor rows -> tree-max on DVE ->
upcast f32 -> PE-transpose to feature-major -> bf16 matmuls (weights
stationary) -> relu -> PE-transpose back to node-major -> store to the
core's block -> AllGather blocks -> next layer's table.
"""

import numpy as np
import ml_dtypes

import concourse.bass as bass
import concourse.bacc as bacc
import concourse.mybir as mybir
import concourse.tile as tile
from concourse.masks import make_identity
from concourse.bass_utils import run_bass_kernel_spmd

N = 50000
E = 800000
F_IN = 128
H = 256
NCOR = 8
NLOC = N // NCOR             # 6250
BLOCK = NLOC + 1             # 6251 rows per core block (last = -inf pad)
TAB = NCOR * BLOCK           # 50008 table rows
VSIZE = 32767                # int16-addressable view size
V1_START = TAB - VSIZE       # 17241; overlap = [V1_START, VSIZE)
PAD0 = NLOC                  # view0 pad idx: B0's pad row (table row 6250)
PAD1 = VSIZE - 1             # view1 pad idx: B7's pad row (50007-17241)
TILES = (NLOC + 127) // 128  # 49
PADN = TILES * 128           # 6272
NEG = float(np.finfo(np.float32).min)
KCAP = 8                     # max gather columns per dma_gather call
CHUNK = 4                    # node tiles per matmul chunk (N free = 512)

_LAST = {}                   # stash for the test harness


# ----------------------------------------------------------------------------
# host-side graph preprocessing
# ----------------------------------------------------------------------------

def _wrap_idx(ilist):
    """ilist [NCOR, num] int -> dma_gather wrapped layout [NCOR, 128*ceil(num/16)]
    (16-partition wrap, replicated to 128 partitions)."""
    num = ilist.shape[1]
    cols = (num + 15) // 16
    w = np.zeros((NCOR, 16, cols), np.int16)
    i = np.arange(num)
    w[:, i % 16, i // 16] = ilist
    w = np.tile(w, (1, 8, 1))                 # [NCOR, 128, cols]
    return w.reshape(NCOR, 128 * cols)


def _preprocess(edge_index):
    src = np.asarray(edge_index[0], np.int64)
    dst = np.asarray(edge_index[1], np.int64)
    deg = np.bincount(dst, minlength=N)
    odeg = np.bincount(src, minlength=N)

    # node -> core: groups of 8 by in-degree desc (keeps per-core in-degree
    # profiles aligned); within each group the highest out-degree nodes go
    # to cores 3,4 whose blocks sit fully inside the view overlap.
    order = np.argsort(-deg, kind="stable")
    CORE_PREF = np.array([3, 4, 2, 5, 1, 6, 0, 7])
    owner = np.empty(N, np.int64)
    for g in range(N // 8):
        grp = order[g * 8:(g + 1) * 8]
        sub = grp[np.argsort(-odeg[grp], kind="stable")]
        owner[sub] = CORE_PREF

    def positions(keys):
        old_of_new = np.empty(N, np.int64)
        for m in range(NCOR):
            nodes = np.where(owner == m)[0]
            k = np.lexsort(tuple(reversed([kk[nodes] for kk in keys])))
            old_of_new[m * NLOC:(m + 1) * NLOC] = nodes[k]
        new_of_old = np.empty(N, np.int64)
        new_of_old[old_of_new] = np.arange(N)
        return old_of_new, new_of_old

    def table_pos(new_of_old):
        return (new_of_old // NLOC) * BLOCK + new_of_old % NLOC

    def forced_counts(tpos):
        sp = tpos[src]
        f0 = np.bincount(dst[sp < V1_START], minlength=N)
        f1 = np.bincount(dst[sp >= VSIZE], minlength=N)
        return f0, f1

    # pass 1: in-degree sort -> forced counts; pass 2: secondary sort by f0
    # clusters similar phase-0 loads into the same tile.
    _, new1 = positions([-deg])
    f0a, f1a = forced_counts(table_pos(new1))
    old_of_new, new_of_old = positions([-deg, -(f0a - f1a)])
    tpos = table_pos(new_of_old)
    f0, f1 = forced_counts(tpos)

    # per-tile phase budgets (shared across cores)
    rank = new_of_old % NLOC
    tilei = rank // 128
    K0 = np.zeros(TILES, np.int64)
    K1 = np.zeros(TILES, np.int64)
    for t in range(TILES):
        sel = tilei == t
        mf0, mf1, mk = f0[sel].max(), f1[sel].max(), deg[sel].max()
        Ts = max(mk, mf0 + mf1)
        a = min(max((Ts + 1) // 2, mf0), Ts - mf1)
        K0[t], K1[t] = max(int(a), 1), max(int(Ts - a), 1)

    # per-dst phase-0 quota, then slot assignment
    k0cap = K0[tilei]
    k1cap = K1[tilei]
    freec = deg - f0 - f1
    dlo = np.maximum(np.minimum(f0 + freec, k0cap), deg - k1cap)

    nd = new_of_old[dst]
    sp = tpos[src]
    is_f0 = sp < V1_START
    is_f1 = sp >= VSIZE
    # order edges per dst: forced0 first, then free, then forced1; the first
    # dlo[d] edges go to phase 0.
    cls = np.where(is_f0, 0, np.where(is_f1, 2, 1)).astype(np.int64)
    ekey = nd * 4 + cls
    eorder = np.argsort(ekey, kind="stable")
    nd_s = nd[eorder]
    sp_s = sp[eorder]
    starts = np.searchsorted(nd_s, np.arange(N))
    slot_in_dst = np.arange(E) - starts[nd_s]
    ph_s = (slot_in_dst >= dlo[old_of_new[nd_s]]).astype(np.int64)
    # sanity: forced edges must land in their required phase
    # (guaranteed by construction: dlo >= f0, deg-dlo >= f1, and the
    #  forced0-free-forced1 ordering)
    idx0 = sp_s - 0
    idx1 = sp_s - V1_START
    loc = np.where(ph_s == 0, idx0, idx1)
    slot_in_ph = np.where(ph_s == 0, slot_in_dst,
                          slot_in_dst - dlo[old_of_new[nd_s]])

    # dense ELL per phase [NCOR, PADN, Kmax]
    def ell_of(phase, kcol, padidx):
        ell = np.full((NCOR, PADN, int(kcol.max())), padidx, np.int16)
        sel = ph_s == phase
        nde = nd_s[sel]
        ell[nde // NLOC, nde % NLOC, slot_in_ph[sel]] = loc[sel].astype(np.int16)
        return ell
    ells = {0: ell_of(0, K0, PAD0), 1: ell_of(1, K1, PAD1)}

    # gather call schedule + wrapped int16 index stream, grouped per chunk
    NCH = (TILES + CHUNK - 1) // CHUNK
    sched = []           # (tile, phase, kn, chunk, cidx0, xoff)
    chunks = []          # (flat_off, cols) per chunk
    blocks = []
    ell_abs = []         # absolute table rows per call [NCOR, kn*128]
    off = 0
    xoff = 0
    for c in range(NCH):
        cblocks = []
        ccols = 0
        for t in range(c * CHUNK, min((c + 1) * CHUNK, TILES)):
            for phase, ks in ((0, K0), (1, K1)):
                k0 = 0
                while k0 < int(ks[t]):
                    kn = min(KCAP, int(ks[t]) - k0)
                    blk = ells[phase][:, t * 128:(t + 1) * 128, k0:k0 + kn]
                    ilist = blk.transpose(0, 2, 1).reshape(NCOR, kn * 128)
                    base = 0 if phase == 0 else V1_START
                    ell_abs.append(ilist.astype(np.int64) + base)
                    w = _wrap_idx(ilist).reshape(NCOR, 128, 8 * kn)
                    cblocks.append(w)
                    sched.append((t, phase, kn, c, ccols, xoff))
                    ccols += 8 * kn
                    xoff += kn
                    k0 += kn
        cb = np.concatenate(cblocks, axis=2)      # [NCOR, 128, ccols]
        blocks.append(cb.reshape(NCOR, 128 * ccols))
        chunks.append((off, ccols))
        off += 128 * ccols
    idx_flat = np.concatenate(blocks, axis=1)
    ell_abs = np.concatenate(ell_abs, axis=1)     # [NCOR, totxslots*128]

    return dict(new_of_old=new_of_old, old_of_new=old_of_new,
                sched=sched, chunks=chunks, totslot=off, idx_flat=idx_flat,
                ell_abs=ell_abs, totx=xoff)


# ----------------------------------------------------------------------------
# device program
# ----------------------------------------------------------------------------

def _tree_max(nc, g, k, F):
    """In-place max over k column groups of width F; result in g[:, :F]."""
    while k > 1:
        if k % 2 == 1:
            nc.vector.tensor_tensor(out=g[:, 0:F], in0=g[:, 0:F],
                                    in1=g[:, (k - 1) * F:k * F],
                                    op=mybir.AluOpType.max)
            k -= 1
            if k == 1:
                break
        half = k // 2
        nc.vector.tensor_tensor(out=g[:, 0:half * F], in0=g[:, 0:half * F],
                                in1=g[:, half * F:2 * half * F],
                                op=mybir.AluOpType.max)
        k = half


def _build_program(sched, chunks, totslot, totx):
    nc = bacc.Bacc("TRN2", target_bir_lowering=False, debug=False,
                   num_devices=NCOR)
    f32, bf16, i16 = mybir.dt.float32, mybir.dt.bfloat16, mybir.dt.int16

    t_xell = nc.dram_tensor("xell", [totx * 128, F_IN], bf16,
                            kind="ExternalInput")
    t_xT = nc.dram_tensor("xT", [F_IN, PADN], bf16, kind="ExternalInput")
    t_idx = nc.dram_tensor("idx", [totslot], i16, kind="ExternalInput")
    wnames = ["Wl1", "Wr1", "Wl2", "Wr2", "Wla", "Wra", "Wlm", "Wrm"]
    wshapes = {"Wl1": (F_IN, H), "Wr1": (F_IN, H)}
    t_w = {w: nc.dram_tensor(w, list(wshapes.get(w, (H, H))), bf16,
                             kind="ExternalInput") for w in wnames}
    t_b = {b: nc.dram_tensor(b, [H, 1], f32, kind="ExternalInput")
           for b in ["bl1", "bl2", "bla", "blm"]}
    t_wh = {w: nc.dram_tensor(w, [H, 1], bf16, kind="ExternalInput")
            for w in ["Wa", "Wm"]}
    t_bh = {b: nc.dram_tensor(b, [1, 1], f32, kind="ExternalInput")
            for b in ["ba", "bm"]}
    t_out = nc.dram_tensor("out", [2, NLOC], f32, kind="ExternalOutput")

    NCH = (TILES + CHUNK - 1) // CHUNK
    cw_of = lambda c: min(CHUNK, TILES - c * CHUNK) * 128

    sched_of_tile = {}
    for (t, phase, kn, c, cidx0, xoff) in sched:
        sched_of_tile.setdefault(t, []).append((phase, kn, cidx0, xoff))
    # L1 merged loads: calls of a tile are contiguous in xell
    l1_of_tile = {}
    for t, calls in sched_of_tile.items():
        x0 = calls[0][3]
        ktot = sum(kn for (_, kn, _, _) in calls)
        l1_of_tile[t] = (x0, ktot)
    KTOTMAX = max(k for (_, k) in l1_of_tile.values())
    CMAX = max(cols for (_, cols) in chunks)

    with tile.TileContext(nc) as tc:
        with tc.tile_pool(name="const", bufs=1) as cpool, \
             tc.tile_pool(name="hT", bufs=1) as hpool, \
             tc.tile_pool(name="work", bufs=2) as wk, \
             tc.tile_pool(name="psT", bufs=2, space="PSUM") as psT, \
             tc.tile_pool(name="psY", bufs=2, space="PSUM") as psY, \
             tc.tile_pool(name="dram", bufs=1, space="DRAM") as dram:

            ident = cpool.tile([128, 128], f32, name="ident")
            make_identity(nc, ident[:])
            ident16 = cpool.tile([128, 128], bf16, name="ident16")
            make_identity(nc, ident16[:])

            w_sb = {}
            for w in wnames:
                fi = wshapes.get(w, (H, H))[0]
                fh = fi // 128
                ws = cpool.tile([128, fh * H], bf16, name=f"sb_{w}")
                for h in range(fh):
                    nc.sync.dma_start(ws[:, h * H:(h + 1) * H],
                                      t_w[w][h * 128:(h + 1) * 128, :])
                w_sb[w] = ws
            b_sb = {}
            for b in t_b:
                bs = cpool.tile([128, 2], f32, name=f"sb_{b}")
                for h in range(2):
                    nc.sync.dma_start(bs[:, h:h + 1],
                                      t_b[b][h * 128:(h + 1) * 128, :])
                b_sb[b] = bs
            wh_sb = {}
            for w in t_wh:
                ws = cpool.tile([128, 2], bf16, name=f"sb_{w}")
                for h in range(2):
                    nc.sync.dma_start(ws[:, h:h + 1],
                                      t_wh[w][h * 128:(h + 1) * 128, :])
                wh_sb[w] = ws
            bh_sb = {}
            for b in t_bh:
                bs = cpool.tile([1, 1], f32, name=f"sb_{b}")
                nc.sync.dma_start(bs[:], t_bh[b][:])
                bh_sb[b] = bs

            xT_sb = hpool.tile([128, PADN], bf16, name="xT_sb")
            nc.sync.dma_start(xT_sb[:], t_xT[:])
            h1T = hpool.tile([128, 2 * PADN], bf16, name="h1T")
            h2T = hpool.tile([128, 2 * PADN], bf16, name="h2T")

            h1tab = dram.tile([TAB, H], bf16, name="h1tab",
                              addr_space="Shared")
            h2tab = dram.tile([TAB, H], bf16, name="h2tab",
                              addr_space="Shared")
            blk1 = dram.tile([BLOCK, H], bf16, name="blk1")
            blk2 = dram.tile([BLOCK, H], bf16, name="blk2")

            # each core's block ends with a -inf pad row
            padrow = cpool.tile([1, H], bf16, name="padrow")
            nc.vector.memset(padrow[:], NEG)
            nc.sync.dma_start(blk1[NLOC:NLOC + 1, :], padrow[:])
            nc.sync.dma_start(blk2[NLOC:NLOC + 1, :], padrow[:])

            def load_idx_chunk(c, tag):
                off, cols = chunks[c]
                idxc = wk.tile([128, CMAX], i16, name=f"idxc_{tag}",
                               tag="idxc", bufs=4)
                nc.sync.dma_start(
                    idxc[:, :cols],
                    t_idx[off:off + 128 * cols].rearrange("(p s) -> p s",
                                                          p=128))
                return idxc

            def aggregate_tile(t, table, F, tag, idxc):
                """two-phase gather + tree-max for node tile t -> bf16 [128, F].

                table=None: layer-1 mode, rows stream from the host
                pre-gathered t_xell instead of dma_gather."""
                if table is None:
                    x0, ktot = l1_of_tile[t]
                    g1 = wk.tile([128, KTOTMAX * F_IN], bf16,
                                 name=f"g1_{tag}", tag="gatherL1", bufs=4)
                    nc.scalar.dma_start(
                        g1[:, :ktot * F].rearrange("p (k f) -> p k f", f=F),
                        t_xell[x0 * 128:(x0 + ktot) * 128, :].rearrange(
                            "(k p) f -> p k f", p=128))
                    _tree_max(nc, g1, ktot, F)
                    return g1
                agg16 = wk.tile([128, H], bf16, name=f"agg16_{tag}",
                                tag="agg16", bufs=4)
                first = True
                for (phase, kn, cidx0, xoff) in sched_of_tile[t]:
                    cols = 8 * kn
                    g = wk.tile([128, KCAP * H], bf16, name=f"g_{tag}",
                                tag="gather", bufs=6)
                    view = table[0:VSIZE, :] if phase == 0 \
                        else table[V1_START:TAB, :]
                    nc.gpsimd.dma_gather(
                        out_ap=g[:, :kn * F].rearrange("p (k f) -> p k f",
                                                       f=F),
                        in_ap=view, idxs_ap=idxc[:, cidx0:cidx0 + cols],
                        num_idxs=128 * kn, num_idxs_reg=128 * kn,
                        elem_size=F, single_packet=False)
                    _tree_max(nc, g, kn, F)
                    if first:
                        nc.vector.tensor_copy(agg16[:, :F], g[:, :F])
                        first = False
                    else:
                        nc.vector.tensor_tensor(out=agg16[:, :F],
                                                in0=agg16[:, :F],
                                                in1=g[:, :F],
                                                op=mybir.AluOpType.max)
                return agg16

            def transpose_into(srcap, dst, col, tag):
                tp = psT.tile([128, 128], f32, name=f"tp_{tag}", tag="tpf")
                nc.tensor.transpose(tp[:], srcap, ident[:])
                nc.vector.tensor_copy(dst[:, col:col + 128], tp[:])

            def layer(table, selfT, F, Wl, Wr, bl, outT, blkout, tag):
                fh_in = F // 128
                lch = CHUNK if table is not None else 2
                lnch = (TILES + lch - 1) // lch
                for c in range(lnch):
                    cw = min(lch, TILES - c * lch) * 128
                    ntile = cw // 128
                    idxc = load_idx_chunk(c, f"{tag}_{c}") \
                        if table is not None else None
                    aggT = wk.tile([128, fh_in * 512], bf16,
                                   name=f"aggT_{tag}", tag="aggT")
                    for i in range(ntile):
                        t = c * lch + i
                        agg16 = aggregate_tile(t, table, F, f"{tag}_{t}",
                                               idxc)
                        agg32 = wk.tile([128, H], f32, name=f"a32_{tag}",
                                        tag="agg32", bufs=4)
                        nc.any.tensor_copy(agg32[:, :F], agg16[:, :F])
                        for fh in range(fh_in):
                            transpose_into(agg32[:, fh * 128:(fh + 1) * 128],
                                           aggT, fh * 512 + i * 128,
                                           f"{tag}_{t}_{fh}")
                    for hh in range(2):
                        psy = psY.tile([128, 512], f32, name=f"psy_{tag}",
                                       tag="psy")
                        nmm = 2 * fh_in
                        i = 0
                        for fh in range(fh_in):
                            nc.tensor.matmul(
                                psy[:, :cw],
                                w_sb[Wl][:, fh * H + hh * 128:
                                         fh * H + (hh + 1) * 128],
                                aggT[:, fh * 512:fh * 512 + cw],
                                start=(i == 0), stop=(i == nmm - 1))
                            i += 1
                            nc.tensor.matmul(
                                psy[:, :cw],
                                w_sb[Wr][:, fh * H + hh * 128:
                                         fh * H + (hh + 1) * 128],
                                selfT[:, fh * PADN + c * lch * 128:
                                      fh * PADN + c * lch * 128 + cw],
                                start=(i == 0), stop=(i == nmm - 1))
                            i += 1
                        nc.scalar.activation(
                            outT[:, hh * PADN + c * lch * 128:
                                 hh * PADN + c * lch * 128 + cw],
                            psy[:, :cw],
                            mybir.ActivationFunctionType.Relu,
                            bias=b_sb[bl][:, hh:hh + 1])
                    for i in range(ntile):
                        t = c * lch + i
                        ynode = wk.tile([128, H], bf16, name=f"yn_{tag}",
                                        tag="ynode", bufs=3)
                        for hh in range(2):
                            tp = psT.tile([128, 128], bf16,
                                          name=f"tpo_{tag}", tag="tp")
                            nc.tensor.transpose(
                                tp[:],
                                outT[:, hh * PADN + t * 128:
                                     hh * PADN + (t + 1) * 128],
                                ident16[:])
                            nc.vector.tensor_copy(
                                ynode[:, hh * 128:(hh + 1) * 128], tp[:])
                        rows = min(128, NLOC - t * 128)
                        nc.sync.dma_start(blkout[t * 128:t * 128 + rows, :],
                                          ynode[:rows, :])

            layer(None, xT_sb, F_IN, "Wl1", "Wr1", "bl1", h1T, blk1, "l1")
            nc.gpsimd.collective_compute(
                "AllGather", mybir.AluOpType.bypass,
                replica_groups=[list(range(NCOR))],
                ins=[blk1.opt()], outs=[h1tab.opt()])
            layer(h1tab, h1T, H, "Wl2", "Wr2", "bl2", h2T, blk2, "l2")
            nc.gpsimd.collective_compute(
                "AllGather", mybir.AluOpType.bypass,
                replica_groups=[list(range(NCOR))],
                ins=[blk2.opt()], outs=[h2tab.opt()])

            # layer 3: two branches + heads
            for c in range(NCH):
                cw = cw_of(c)
                ntile = cw // 128
                idxc = load_idx_chunk(c, f"l3_{c}")
                aggT = wk.tile([128, 2 * 512], bf16, name="aggT_l3",
                               tag="aggT")
                for i in range(ntile):
                    t = c * CHUNK + i
                    agg16 = aggregate_tile(t, h2tab, H, f"l3_{t}", idxc)
                    agg32 = wk.tile([128, H], f32, name="a32_l3",
                                    tag="agg32", bufs=4)
                    nc.any.tensor_copy(agg32[:, :H], agg16[:, :H])
                    for fh in range(2):
                        transpose_into(agg32[:, fh * 128:(fh + 1) * 128],
                                       aggT, fh * 512 + i * 128,
                                       f"l3_{t}_{fh}")
                out_sbs = [wk.tile([1, 512], f32, name=f"out_sb{bi}",
                                   tag=f"out_sb{bi}") for bi in range(2)]
                for bi, (Wl, Wr, bl, Wh, bh) in enumerate(
                        [("Wla", "Wra", "bla", "Wa", "ba"),
                         ("Wlm", "Wrm", "blm", "Wm", "bm")]):
                    brT = wk.tile([128, 2 * 512], bf16, name=f"brT{bi}",
                                  tag="brT")
                    for hh in range(2):
                        psy = psY.tile([128, 512], f32, name=f"psy3_{bi}",
                                       tag="psy")
                        for fh in range(2):
                            nc.tensor.matmul(
                                psy[:, :cw],
                                w_sb[Wl][:, fh * H + hh * 128:
                                         fh * H + (hh + 1) * 128],
                                aggT[:, fh * 512:fh * 512 + cw],
                                start=(fh == 0), stop=False)
                            nc.tensor.matmul(
                                psy[:, :cw],
                                w_sb[Wr][:, fh * H + hh * 128:
                                         fh * H + (hh + 1) * 128],
                                h2T[:, fh * PADN + c * CHUNK * 128:
                                    fh * PADN + c * CHUNK * 128 + cw],
                                start=False, stop=(fh == 1))
                        nc.scalar.activation(
                            brT[:, hh * 512:hh * 512 + cw], psy[:, :cw],
                            mybir.ActivationFunctionType.Relu,
                            bias=b_sb[bl][:, hh:hh + 1])
                    psh = psY.tile([1, 512], f32, name=f"psh{bi}", tag="psh")
                    for hh in range(2):
                        nc.tensor.matmul(psh[:, :cw],
                                         wh_sb[Wh][:, hh:hh + 1],
                                         brT[:, hh * 512:hh * 512 + cw],
                                         start=(hh == 0), stop=(hh == 1))
                    nc.scalar.activation(out_sbs[bi][:, :cw],
                                         psh[:, :cw],
                                         mybir.ActivationFunctionType.Identity,
                                         bias=bh_sb[bh][:])
                live = min(cw, NLOC - c * CHUNK * 128)
                for bi in range(2):
                    nc.sync.dma_start(
                        t_out[bi:bi + 1,
                              c * CHUNK * 128:c * CHUNK * 128 + live],
                        out_sbs[bi][:, :live])

    nc.compile()
    return nc


# ----------------------------------------------------------------------------
# entry point
# ----------------------------------------------------------------------------

def kernel(x, edge_index, Wl1, bl1, Wr1, Wl2, bl2, Wr2,
           Wla, bla, Wra, Wa, ba, Wlm, blm, Wrm, Wm, bm):
    x = np.asarray(x, np.float32)
    pp = _preprocess(edge_index)
    old_of_new = pp["old_of_new"]

    # x gather table in block layout: per core 6250 rows + one -inf pad row
    xp = x[old_of_new]
    xtab = np.empty((TAB, F_IN), np.float32)
    for m in range(NCOR):
        base = m * BLOCK
        xtab[base:base + NLOC] = xp[m * NLOC:(m + 1) * NLOC]
        xtab[base + NLOC] = NEG
    xtab = xtab.astype(ml_dtypes.bfloat16)

    nc = _build_program(pp["sched"], pp["chunks"], pp["totslot"],
                        pp["totx"])

    def f32(a):
        return np.ascontiguousarray(np.asarray(a, np.float32))

    def b16(a):
        return np.ascontiguousarray(
            np.asarray(a, np.float32).astype(ml_dtypes.bfloat16))

    in_maps = []
    for m in range(NCOR):
        blk = xp[m * NLOC:(m + 1) * NLOC]
        xT = np.zeros((F_IN, PADN), np.float32)
        xT[:, :NLOC] = blk.T
        xell = xtab[pp["ell_abs"][m]]
        in_maps.append({
            "xell": xell, "xT": xT.astype(ml_dtypes.bfloat16),
            "idx": pp["idx_flat"][m],
            "Wl1": b16(Wl1), "Wr1": b16(Wr1),
            "Wl2": b16(Wl2), "Wr2": b16(Wr2),
            "Wla": b16(Wla), "Wra": b16(Wra),
            "Wlm": b16(Wlm), "Wrm": b16(Wrm),
            "bl1": f32(bl1).reshape(H, 1), "bl2": f32(bl2).reshape(H, 1),
            "bla": f32(bla).reshape(H, 1), "blm": f32(blm).reshape(H, 1),
            "Wa": b16(Wa).reshape(H, 1), "Wm": b16(Wm).reshape(H, 1),
            "ba": f32(ba).reshape(1, 1), "bm": f32(bm).reshape(1, 1),
        })

    res = run_bass_kernel_spmd(nc, in_maps, core_ids=list(range(NCOR)))

    rt = np.empty(N, np.float32)
    mv = np.empty(N, np.float32)
    for m in range(NCOR):
        out = res.results[m]["out"]
        rt[m * NLOC:(m + 1) * NLOC] = out[0]
        mv[m * NLOC:(m + 1) * NLOC] = out[1]
    rt_o = np.empty(N, np.float32)
    mv_o = np.empty(N, np.float32)
    rt_o[old_of_new] = rt
    mv_o[old_of_new] = mv

    _LAST.update(nc=nc, in_maps=in_maps, pp=pp)
    return (rt_o, mv_o)


# revision 13
# speedup vs baseline: 1595.8665x; 1.0031x over previous
"""GraphSAGE-max (3 layers + 2 heads) on 8 Trainium2 NeuronCores.

Strategy: data-parallel over dst-node partitions with replicated bf16 feature
tables in DRAM. The critical resource is GPSIMD descriptor generation for
dma_gather (~7.8ns per gathered row), so the schedule minimizes gathered
rows:

  - int16 gather indices can only span 32767 table rows, so the 50008-row
    table is covered by two OVERLAPPING views: view0 = rows [0, 32767),
    view1 = rows [17241, 50008). Sources in the 15526-row overlap can be
    fetched by either phase; high out-degree nodes are steered into the
    overlap (owner cores 3,4) so ~39% of edges are free-choice.
  - Per dst-node the free edges are split between the phases to equalize the
    per-tile max slot counts: K0[t]+K1[t] = max(max_deg, max_f0+max_f1),
    bringing ELL padding from 1.39x down to ~1.20x.

Layer 1's neighbor rows depend only on host inputs, so its ELL stream is
pre-gathered on the host (x[src] in call order) and the device reads it
with plain HWDGE DMAs — no Q7 descriptor generation at all. Layers 2/3
dma_gather from the AllGathered hidden-state tables with calls capped at
8 slot columns (1024 indices), which runs at ~7.8ns/row; 16-column calls
measure ~9.3ns/row.

Each layer, per 128-node tile: fetch neighbor rows -> tree-max on DVE ->
upcast f32 -> PE-transpose to feature-major -> bf16 matmuls (weights
stationary) -> relu -> PE-transpose back to node-major -> store to the
core's block -> AllGather blocks -> next layer's table.
"""

import numpy as np
import ml_dtypes

import concourse.bass as bass
import concourse.bacc as bacc
import concourse.mybir as mybir
import concourse.tile as tile
from concourse.masks import make_identity
from concourse.bass_utils import run_bass_kernel_spmd

N = 50000
E = 800000
F_IN = 128
H = 256
NCOR = 8
NLOC = N // NCOR             # 6250
BLOCK = NLOC + 1             # 6251 rows per core block (last = -inf pad)
TAB = NCOR * BLOCK           # 50008 table rows
VSIZE = 32767                # int16-addressable view size
V1_START = TAB - VSIZE       # 17241; overlap = [V1_START, VSIZE)
PAD0 = NLOC                  # view0 pad idx: B0's pad row (table row 6250)
PAD1 = VSIZE - 1             # view1 pad idx: B7's pad row (50007-17241)
TILES = (NLOC + 127) // 128  # 49
PADN = TILES * 128           # 6272
NEG = float(np.finfo(np.float32).min)
KCAP = 8                     # max gather columns per dma_gather call
CHUNK = 4                    # node tiles per matmul chunk (N free = 512)

_LAST = {}                   # stash for the test harness


# ----------------------------------------------------------------------------
# host-side graph preprocessing
# ----------------------------------------------------------------------------

def _wrap_idx(ilist):
    """ilist [NCOR, num] int -> dma_gather wrapped layout [NCOR, 128*ceil(num/16)]
    (16-partition wrap, replicated to 128 partitions)."""
    num = ilist.shape[1]
    cols = (num + 15) // 16
    w = np.zeros((NCOR, 16, cols), np.int16)
    i = np.arange(num)
    w[:, i % 16, i // 16] = ilist
    w = np.tile(w, (1, 8, 1))                 # [NCOR, 128, cols]
    return w.reshape(NCOR, 128 * cols)


def _preprocess(edge_index):
    src = np.asarray(edge_index[0], np.int64)
    dst = np.asarray(edge_index[1], np.int64)
    deg = np.bincount(dst, minlength=N)
    odeg = np.bincount(src, minlength=N)

    # node -> core: groups of 8 by in-degree desc (keeps per-core in-degree
    # profiles aligned); within each group the highest out-degree nodes go
    # to cores 3,4 whose blocks sit fully inside the view overlap.
    order = np.argsort(-deg, kind="stable")
    CORE_PREF = np.array([3, 4, 2, 5, 1, 6, 0, 7])
    owner = np.empty(N, np.int64)
    for g in range(N // 8):
        grp = order[g * 8:(g + 1) * 8]
        sub = grp[np.argsort(-odeg[grp], kind="stable")]
        owner[sub] = CORE_PREF

    def positions(keys):
        old_of_new = np.empty(N, np.int64)
        for m in range(NCOR):
            nodes = np.where(owner == m)[0]
            k = np.lexsort(tuple(reversed([kk[nodes] for kk in keys])))
            old_of_new[m * NLOC:(m + 1) * NLOC] = nodes[k]
        new_of_old = np.empty(N, np.int64)
        new_of_old[old_of_new] = np.arange(N)
        return old_of_new, new_of_old

    def table_pos(new_of_old):
        return (new_of_old // NLOC) * BLOCK + new_of_old % NLOC

    def forced_counts(tpos):
        sp = tpos[src]
        f0 = np.bincount(dst[sp < V1_START], minlength=N)
        f1 = np.bincount(dst[sp >= VSIZE], minlength=N)
        return f0, f1

    # pass 1: in-degree sort -> forced counts; pass 2: secondary sort by f0
    # clusters similar phase-0 loads into the same tile.
    _, new1 = positions([-deg])
    f0a, f1a = forced_counts(table_pos(new1))
    old_of_new, new_of_old = positions([-deg, -(f0a - f1a)])
    tpos = table_pos(new_of_old)
    f0, f1 = forced_counts(tpos)

    # per-tile phase budgets (shared across cores)
    rank = new_of_old % NLOC
    tilei = rank // 128
    K0 = np.zeros(TILES, np.int64)
    K1 = np.zeros(TILES, np.int64)
    for t in range(TILES):
        sel = tilei == t
        mf0, mf1, mk = f0[sel].max(), f1[sel].max(), deg[sel].max()
        Ts = int(max(mk, mf0 + mf1))
        # rows are Ts*128 regardless of the split; pick a (=K0) minimizing
        # the gather call count ceil(a/KCAP)+ceil((Ts-a)/KCAP), tie-break
        # toward a balanced split.
        lo, hi = max(int(mf0), 1), max(Ts - int(mf1), 1)
        best = None
        for cand in sorted({lo, hi, KCAP, 2 * KCAP, 3 * KCAP,
                            (Ts + 1) // 2}):
            if cand < lo or cand > hi:
                continue
            calls = -(-cand // KCAP) + -(-max(Ts - cand, 1) // KCAP)
            key = (calls, abs(cand - Ts / 2))
            if best is None or key < best[0]:
                best = (key, cand)
        a = best[1]
        K0[t], K1[t] = a, max(Ts - a, 1)

    # per-dst phase-0 quota, then slot assignment
    k0cap = K0[tilei]
    k1cap = K1[tilei]
    freec = deg - f0 - f1
    dlo = np.maximum(np.minimum(f0 + freec, k0cap), deg - k1cap)

    nd = new_of_old[dst]
    sp = tpos[src]
    is_f0 = sp < V1_START
    is_f1 = sp >= VSIZE
    # order edges per dst: forced0 first, then free, then forced1; the first
    # dlo[d] edges go to phase 0.
    cls = np.where(is_f0, 0, np.where(is_f1, 2, 1)).astype(np.int64)
    ekey = nd * 4 + cls
    eorder = np.argsort(ekey, kind="stable")
    nd_s = nd[eorder]
    sp_s = sp[eorder]
    starts = np.searchsorted(nd_s, np.arange(N))
    slot_in_dst = np.arange(E) - starts[nd_s]
    ph_s = (slot_in_dst >= dlo[old_of_new[nd_s]]).astype(np.int64)
    # sanity: forced edges must land in their required phase
    # (guaranteed by construction: dlo >= f0, deg-dlo >= f1, and the
    #  forced0-free-forced1 ordering)
    idx0 = sp_s - 0
    idx1 = sp_s - V1_START
    loc = np.where(ph_s == 0, idx0, idx1)
    slot_in_ph = np.where(ph_s == 0, slot_in_dst,
                          slot_in_dst - dlo[old_of_new[nd_s]])

    # dense ELL per phase [NCOR, PADN, Kmax]
    def ell_of(phase, kcol, padidx):
        ell = np.full((NCOR, PADN, int(kcol.max())), padidx, np.int16)
        sel = ph_s == phase
        nde = nd_s[sel]
        ell[nde // NLOC, nde % NLOC, slot_in_ph[sel]] = loc[sel].astype(np.int16)
        return ell
    ells = {0: ell_of(0, K0, PAD0), 1: ell_of(1, K1, PAD1)}

    # gather call schedule + wrapped int16 index stream, grouped per chunk
    NCH = (TILES + CHUNK - 1) // CHUNK
    sched = []           # (tile, phase, kn, chunk, cidx0, xoff)
    chunks = []          # (flat_off, cols) per chunk
    blocks = []
    ell_abs = []         # absolute table rows per call [NCOR, kn*128]
    off = 0
    xoff = 0
    for c in range(NCH):
        cblocks = []
        ccols = 0
        for t in range(c * CHUNK, min((c + 1) * CHUNK, TILES)):
            for phase, ks in ((0, K0), (1, K1)):
                k0 = 0
                while k0 < int(ks[t]):
                    kn = min(KCAP, int(ks[t]) - k0)
                    blk = ells[phase][:, t * 128:(t + 1) * 128, k0:k0 + kn]
                    ilist = blk.transpose(0, 2, 1).reshape(NCOR, kn * 128)
                    base = 0 if phase == 0 else V1_START
                    ell_abs.append(ilist.astype(np.int64) + base)
                    w = _wrap_idx(ilist).reshape(NCOR, 128, 8 * kn)
                    cblocks.append(w)
                    sched.append((t, phase, kn, c, ccols, xoff))
                    ccols += 8 * kn
                    xoff += kn
                    k0 += kn
        cb = np.concatenate(cblocks, axis=2)      # [NCOR, 128, ccols]
        blocks.append(cb.reshape(NCOR, 128 * ccols))
        chunks.append((off, ccols))
        off += 128 * ccols
    idx_flat = np.concatenate(blocks, axis=1)
    ell_abs = np.concatenate(ell_abs, axis=1)     # [NCOR, totxslots*128]

    return dict(new_of_old=new_of_old, old_of_new=old_of_new,
                sched=sched, chunks=chunks, totslot=off, idx_flat=idx_flat,
                ell_abs=ell_abs, totx=xoff)


# ----------------------------------------------------------------------------
# device program
# ----------------------------------------------------------------------------

def _tree_max(nc, g, k, F):
    """In-place max over k column groups of width F; result in g[:, :F]."""
    while k > 1:
        if k % 2 == 1:
            nc.vector.tensor_tensor(out=g[:, 0:F], in0=g[:, 0:F],
                                    in1=g[:, (k - 1) * F:k * F],
                                    op=mybir.AluOpType.max)
            k -= 1
            if k == 1:
                break
        half = k // 2
        nc.vector.tensor_tensor(out=g[:, 0:half * F], in0=g[:, 0:half * F],
                                in1=g[:, half * F:2 * half * F],
                                op=mybir.AluOpType.max)
        k = half


def _build_program(sched, chunks, totslot, totx):
    nc = bacc.Bacc("TRN2", target_bir_lowering=False, debug=False,
                   num_devices=NCOR)
    f32, bf16, i16 = mybir.dt.float32, mybir.dt.bfloat16, mybir.dt.int16

    t_xell = nc.dram_tensor("xell", [totx * 128, F_IN], bf16,
                            kind="ExternalInput")
    t_xT = nc.dram_tensor("xT", [F_IN, PADN], bf16, kind="ExternalInput")
    t_idx = nc.dram_tensor("idx", [totslot], i16, kind="ExternalInput")
    wnames = ["Wl1", "Wr1", "Wl2", "Wr2", "Wla", "Wra", "Wlm", "Wrm"]
    wshapes = {"Wl1": (F_IN, H), "Wr1": (F_IN, H)}
    t_w = {w: nc.dram_tensor(w, list(wshapes.get(w, (H, H))), bf16,
                             kind="ExternalInput") for w in wnames}
    t_b = {b: nc.dram_tensor(b, [H, 1], f32, kind="ExternalInput")
           for b in ["bl1", "bl2", "bla", "blm"]}
    t_wh = {w: nc.dram_tensor(w, [H, 1], bf16, kind="ExternalInput")
            for w in ["Wa", "Wm"]}
    t_bh = {b: nc.dram_tensor(b, [1, 1], f32, kind="ExternalInput")
            for b in ["ba", "bm"]}
    t_out = nc.dram_tensor("out", [2, NLOC], f32, kind="ExternalOutput")

    NCH = (TILES + CHUNK - 1) // CHUNK
    cw_of = lambda c: min(CHUNK, TILES - c * CHUNK) * 128

    sched_of_tile = {}
    for (t, phase, kn, c, cidx0, xoff) in sched:
        sched_of_tile.setdefault(t, []).append((phase, kn, cidx0, xoff))
    # L1 merged loads: calls of a tile are contiguous in xell
    l1_of_tile = {}
    for t, calls in sched_of_tile.items():
        x0 = calls[0][3]
        ktot = sum(kn for (_, kn, _, _) in calls)
        l1_of_tile[t] = (x0, ktot)
    KTOTMAX = max(k for (_, k) in l1_of_tile.values())
    CMAX = max(cols for (_, cols) in chunks)

    with tile.TileContext(nc) as tc:
        with tc.tile_pool(name="const", bufs=1) as cpool, \
             tc.tile_pool(name="hT", bufs=1) as hpool, \
             tc.tile_pool(name="work", bufs=2) as wk, \
             tc.tile_pool(name="psT", bufs=2, space="PSUM") as psT, \
             tc.tile_pool(name="psY", bufs=2, space="PSUM") as psY, \
             tc.tile_pool(name="dram", bufs=1, space="DRAM") as dram:

            ident = cpool.tile([128, 128], f32, name="ident")
            make_identity(nc, ident[:])
            ident16 = cpool.tile([128, 128], bf16, name="ident16")
            make_identity(nc, ident16[:])

            w_sb = {}
            for w in wnames:
                fi = wshapes.get(w, (H, H))[0]
                fh = fi // 128
                ws = cpool.tile([128, fh * H], bf16, name=f"sb_{w}")
                for h in range(fh):
                    nc.sync.dma_start(ws[:, h * H:(h + 1) * H],
                                      t_w[w][h * 128:(h + 1) * 128, :])
                w_sb[w] = ws
            b_sb = {}
            for b in t_b:
                bs = cpool.tile([128, 2], f32, name=f"sb_{b}")
                for h in range(2):
                    nc.sync.dma_start(bs[:, h:h + 1],
                                      t_b[b][h * 128:(h + 1) * 128, :])
                b_sb[b] = bs
            wh_sb = {}
            for w in t_wh:
                ws = cpool.tile([128, 2], bf16, name=f"sb_{w}")
                for h in range(2):
                    nc.sync.dma_start(ws[:, h:h + 1],
                                      t_wh[w][h * 128:(h + 1) * 128, :])
                wh_sb[w] = ws
            bh_sb = {}
            for b in t_bh:
                bs = cpool.tile([1, 1], f32, name=f"sb_{b}")
                nc.sync.dma_start(bs[:], t_bh[b][:])
                bh_sb[b] = bs

            xT_sb = hpool.tile([128, PADN], bf16, name="xT_sb")
            nc.sync.dma_start(xT_sb[:], t_xT[:])
            h1T = hpool.tile([128, 2 * PADN], bf16, name="h1T")
            h2T = hpool.tile([128, 2 * PADN], bf16, name="h2T")

            h1tab = dram.tile([TAB, H], bf16, name="h1tab",
                              addr_space="Shared")
            h2tab = dram.tile([TAB, H], bf16, name="h2tab",
                              addr_space="Shared")
            blk1 = dram.tile([BLOCK, H], bf16, name="blk1")
            blk2 = dram.tile([BLOCK, H], bf16, name="blk2")

            # each core's block ends with a -inf pad row
            padrow = cpool.tile([1, H], bf16, name="padrow")
            nc.vector.memset(padrow[:], NEG)
            nc.sync.dma_start(blk1[NLOC:NLOC + 1, :], padrow[:])
            nc.sync.dma_start(blk2[NLOC:NLOC + 1, :], padrow[:])

            def load_idx_chunk(c, tag):
                off, cols = chunks[c]
                idxc = wk.tile([128, CMAX], i16, name=f"idxc_{tag}",
                               tag="idxc", bufs=4)
                nc.sync.dma_start(
                    idxc[:, :cols],
                    t_idx[off:off + 128 * cols].rearrange("(p s) -> p s",
                                                          p=128))
                return idxc

            def aggregate_tile(t, table, F, tag, idxc):
                """two-phase gather + tree-max for node tile t -> bf16 [128, F].

                table=None: layer-1 mode, rows stream from the host
                pre-gathered t_xell instead of dma_gather."""
                if table is None:
                    x0, ktot = l1_of_tile[t]
                    g1 = wk.tile([128, KTOTMAX * F_IN], bf16,
                                 name=f"g1_{tag}", tag="gatherL1", bufs=4)
                    nc.scalar.dma_start(
                        g1[:, :ktot * F].rearrange("p (k f) -> p k f", f=F),
                        t_xell[x0 * 128:(x0 + ktot) * 128, :].rearrange(
                            "(k p) f -> p k f", p=128))
                    _tree_max(nc, g1, ktot, F)
                    return g1
                agg16 = wk.tile([128, H], bf16, name=f"agg16_{tag}",
                                tag="agg16", bufs=6)
                first = True
                for (phase, kn, cidx0, xoff) in sched_of_tile[t]:
                    cols = 8 * kn
                    g = wk.tile([128, KCAP * H], bf16, name=f"g_{tag}",
                                tag="gather", bufs=8)
                    view = table[0:VSIZE, :] if phase == 0 \
                        else table[V1_START:TAB, :]
                    nc.gpsimd.dma_gather(
                        out_ap=g[:, :kn * F].rearrange("p (k f) -> p k f",
                                                       f=F),
                        in_ap=view, idxs_ap=idxc[:, cidx0:cidx0 + cols],
                        num_idxs=128 * kn, num_idxs_reg=128 * kn,
                        elem_size=F, single_packet=False)
                    _tree_max(nc, g, kn, F)
                    if first:
                        nc.vector.tensor_copy(agg16[:, :F], g[:, :F])
                        first = False
                    else:
                        nc.vector.tensor_tensor(out=agg16[:, :F],
                                                in0=agg16[:, :F],
                                                in1=g[:, :F],
                                                op=mybir.AluOpType.max)
                return agg16

            def transpose_into(srcap, dst, col, tag):
                tp = psT.tile([128, 128], f32, name=f"tp_{tag}", tag="tpf")
                nc.tensor.transpose(tp[:], srcap, ident[:])
                nc.vector.tensor_copy(dst[:, col:col + 128], tp[:])

            def layer(table, selfT, F, Wl, Wr, bl, outT, blkout, tag):
                fh_in = F // 128
                lch = CHUNK if table is not None else 2
                lnch = (TILES + lch - 1) // lch
                for c in range(lnch):
                    cw = min(lch, TILES - c * lch) * 128
                    ntile = cw // 128
                    idxc = load_idx_chunk(c, f"{tag}_{c}") \
                        if table is not None else None
                    aggT = wk.tile([128, fh_in * 512], bf16,
                                   name=f"aggT_{tag}", tag="aggT")
                    for i in range(ntile):
                        t = c * lch + i
                        agg16 = aggregate_tile(t, table, F, f"{tag}_{t}",
                                               idxc)
                        agg32 = wk.tile([128, H], f32, name=f"a32_{tag}",
                                        tag="agg32", bufs=4)
                        nc.any.tensor_copy(agg32[:, :F], agg16[:, :F])
                        for fh in range(fh_in):
                            transpose_into(agg32[:, fh * 128:(fh + 1) * 128],
                                           aggT, fh * 512 + i * 128,
                                           f"{tag}_{t}_{fh}")
                    for hh in range(2):
                        psy = psY.tile([128, 512], f32, name=f"psy_{tag}",
                                       tag="psy")
                        nmm = 2 * fh_in
                        i = 0
                        for fh in range(fh_in):
                            nc.tensor.matmul(
                                psy[:, :cw],
                                w_sb[Wl][:, fh * H + hh * 128:
                                         fh * H + (hh + 1) * 128],
                                aggT[:, fh * 512:fh * 512 + cw],
                                start=(i == 0), stop=(i == nmm - 1))
                            i += 1
                            nc.tensor.matmul(
                                psy[:, :cw],
                                w_sb[Wr][:, fh * H + hh * 128:
                                         fh * H + (hh + 1) * 128],
                                selfT[:, fh * PADN + c * lch * 128:
                                      fh * PADN + c * lch * 128 + cw],
                                start=(i == 0), stop=(i == nmm - 1))
                            i += 1
                        nc.scalar.activation(
                            outT[:, hh * PADN + c * lch * 128:
                                 hh * PADN + c * lch * 128 + cw],
                            psy[:, :cw],
                            mybir.ActivationFunctionType.Relu,
                            bias=b_sb[bl][:, hh:hh + 1])
                    for i in range(ntile):
                        t = c * lch + i
                        ynode = wk.tile([128, H], bf16, name=f"yn_{tag}",
                                        tag="ynode", bufs=3)
                        for hh in range(2):
                            tp = psT.tile([128, 128], bf16,
                                          name=f"tpo_{tag}", tag="tp")
                            nc.tensor.transpose(
                                tp[:],
                                outT[:, hh * PADN + t * 128:
                                     hh * PADN + (t + 1) * 128],
                                ident16[:])
                            nc.vector.tensor_copy(
                                ynode[:, hh * 128:(hh + 1) * 128], tp[:])
                        rows = min(128, NLOC - t * 128)
                        nc.sync.dma_start(blkout[t * 128:t * 128 + rows, :],
                                          ynode[:rows, :])

            layer(None, xT_sb, F_IN, "Wl1", "Wr1", "bl1", h1T, blk1, "l1")
            nc.gpsimd.collective_compute(
                "AllGather", mybir.AluOpType.bypass,
                replica_groups=[list(range(NCOR))],
                ins=[blk1.opt()], outs=[h1tab.opt()])
            layer(h1tab, h1T, H, "Wl2", "Wr2", "bl2", h2T, blk2, "l2")
            nc.gpsimd.collective_compute(
                "AllGather", mybir.AluOpType.bypass,
                replica_groups=[list(range(NCOR))],
                ins=[blk2.opt()], outs=[h2tab.opt()])

            # layer 3: two branches + heads
            for c in range(NCH):
                cw = cw_of(c)
                ntile = cw // 128
                idxc = load_idx_chunk(c, f"l3_{c}")
                aggT = wk.tile([128, 2 * 512], bf16, name="aggT_l3",
                               tag="aggT")
                for i in range(ntile):
                    t = c * CHUNK + i
                    agg16 = aggregate_tile(t, h2tab, H, f"l3_{t}", idxc)
                    agg32 = wk.tile([128, H], f32, name="a32_l3",
                                    tag="agg32", bufs=4)
                    nc.any.tensor_copy(agg32[:, :H], agg16[:, :H])
                    for fh in range(2):
                        transpose_into(agg32[:, fh * 128:(fh + 1) * 128],
                                       aggT, fh * 512 + i * 128,
                                       f"l3_{t}_{fh}")
                out_sbs = [wk.tile([1, 512], f32, name=f"out_sb{bi}",
                                   tag=f"out_sb{bi}") for bi in range(2)]
                for bi, (Wl, Wr, bl, Wh, bh) in enumerate(
                        [("Wla", "Wra", "bla", "Wa", "ba"),
                         ("Wlm", "Wrm", "blm", "Wm", "bm")]):
                    brT = wk.tile([128, 2 * 512], bf16, name=f"brT{bi}",
                                  tag="brT")
                    for hh in range(2):
                        psy = psY.tile([128, 512], f32, name=f"psy3_{bi}",
                                       tag="psy")
                        for fh in range(2):
                            nc.tensor.matmul(
                                psy[:, :cw],
                                w_sb[Wl][:, fh * H + hh * 128:
                                         fh * H + (hh + 1) * 128],
                                aggT[:, fh * 512:fh * 512 + cw],
                                start=(fh == 0), stop=False)
                            nc.tensor.matmul(
                                psy[:, :cw],
                                w_sb[Wr][:, fh * H + hh * 128:
                                         fh * H + (hh + 1) * 128],
                                h2T[:, fh * PADN + c * CHUNK * 128:
                                    fh * PADN + c * CHUNK * 128 + cw],
                                start=False, stop=(fh == 1))
                        nc.scalar.activation(
                            brT[:, hh * 512:hh * 512 + cw], psy[:, :cw],
                            mybir.ActivationFunctionType.Relu,
                            bias=b_sb[bl][:, hh:hh + 1])
                    psh = psY.tile([1, 512], f32, name=f"psh{bi}", tag="psh")
                    for hh in range(2):
                        nc.tensor.matmul(psh[:, :cw],
                                         wh_sb[Wh][:, hh:hh + 1],
                                         brT[:, hh * 512:hh * 512 + cw],
                                         start=(hh == 0), stop=(hh == 1))
                    nc.scalar.activation(out_sbs[bi][:, :cw],
                                         psh[:, :cw],
                                         mybir.ActivationFunctionType.Identity,
                                         bias=bh_sb[bh][:])
                live = min(cw, NLOC - c * CHUNK * 128)
                for bi in range(2):
                    nc.sync.dma_start(
                        t_out[bi:bi + 1,
                              c * CHUNK * 128:c * CHUNK * 128 + live],
                        out_sbs[bi][:, :live])

    nc.compile()
    return nc


# ----------------------------------------------------------------------------
# entry point
# ----------------------------------------------------------------------------

def kernel(x, edge_index, Wl1, bl1, Wr1, Wl2, bl2, Wr2,
           Wla, bla, Wra, Wa, ba, Wlm, blm, Wrm, Wm, bm):
    x = np.asarray(x, np.float32)
    pp = _preprocess(edge_index)
    old_of_new = pp["old_of_new"]

    # x gather table in block layout: per core 6250 rows + one -inf pad row
    xp = x[old_of_new]
    xtab = np.empty((TAB, F_IN), np.float32)
    for m in range(NCOR):
        base = m * BLOCK
        xtab[base:base + NLOC] = xp[m * NLOC:(m + 1) * NLOC]
        xtab[base + NLOC] = NEG
    xtab = xtab.astype(ml_dtypes.bfloat16)

    nc = _build_program(pp["sched"], pp["chunks"], pp["totslot"],
                        pp["totx"])

    def f32(a):
        return np.ascontiguousarray(np.asarray(a, np.float32))

    def b16(a):
        return np.ascontiguousarray(
            np.asarray(a, np.float32).astype(ml_dtypes.bfloat16))

    in_maps = []
    for m in range(NCOR):
        blk = xp[m * NLOC:(m + 1) * NLOC]
        xT = np.zeros((F_IN, PADN), np.float32)
        xT[:, :NLOC] = blk.T
        xell = xtab[pp["ell_abs"][m]]
        in_maps.append({
            "xell": xell, "xT": xT.astype(ml_dtypes.bfloat16),
            "idx": pp["idx_flat"][m],
            "Wl1": b16(Wl1), "Wr1": b16(Wr1),
            "Wl2": b16(Wl2), "Wr2": b16(Wr2),
            "Wla": b16(Wla), "Wra": b16(Wra),
            "Wlm": b16(Wlm), "Wrm": b16(Wrm),
            "bl1": f32(bl1).reshape(H, 1), "bl2": f32(bl2).reshape(H, 1),
            "bla": f32(bla).reshape(H, 1), "blm": f32(blm).reshape(H, 1),
            "Wa": b16(Wa).reshape(H, 1), "Wm": b16(Wm).reshape(H, 1),
            "ba": f32(ba).reshape(1, 1), "bm": f32(bm).reshape(1, 1),
        })

    res = run_bass_kernel_spmd(nc, in_maps, core_ids=list(range(NCOR)))

    rt = np.empty(N, np.float32)
    mv = np.empty(N, np.float32)
    for m in range(NCOR):
        out = res.results[m]["out"]
        rt[m * NLOC:(m + 1) * NLOC] = out[0]
        mv[m * NLOC:(m + 1) * NLOC] = out[1]
    rt_o = np.empty(N, np.float32)
    mv_o = np.empty(N, np.float32)
    rt_o[old_of_new] = rt
    mv_o[old_of_new] = mv

    _LAST.update(nc=nc, in_maps=in_maps, pp=pp)
    return (rt_o, mv_o)
